# revision 47
# baseline (speedup 1.0000x reference)
"""Trainium2 Bass kernel for one dense transformer block (B=2, S=2048, D=1024,
16 q-heads / 4 kv-heads GQA, squared-ReLU MLP), data-parallel over 8 NeuronCores.

Sharding: core c = (b, j), b = c // 4, j = c % 4, owns q-token tiles
{j, j+4, j+8, j+12} (128 tokens each) of batch b. K/V are computed for the full
sequence on every core (no collectives). The kv range for own q-tile t is
padded to 512*(t+1); causality enforced with per-core 0/1 masks on the last
512-wide kv chunk.

Numerical identities used (exact up to negligible eps rescaling):
  - per-head q/k rmsnorm is scale-invariant per token, so the block input
    rmsnorm cancels inside it -> Q/K project from raw (norm-weight-folded) x
  - the MLP input rmsnorm cancels through relu()^2 -> proj -> post-rmsnorm
  - V is projected from raw x and rescaled by 1/rms1(x) per token
  - no softmax max-subtraction (logits bounded by |q||k|/8 = 8)
  - softmax denominator = ones-column appended to V in the AV matmul
  - K's 1/rms is applied as a per-partition AP scale inside the exp
    activation (kv tokens are partitions in the score tiles); Q's 1/rms and
    q_gain/8 ride a replicate matmul onto qT

v2 perf changes vs baseline:
  - no DVE reciprocal with f32r destination (was ~7.7ns/elem); all recips are
    fp32->fp32 on DVE, replicates via small fp32 matmuls
  - rope via a feature-swap permutation matmul + 3 full-width DVE ops
    (was 12 narrow DVE ops)
  - bf16 weights + x + V/p/mask/y/h2 paths (half DMA, FWL weight loads,
    2x DVE); q/k/scores stay f32r
"""

import os

import numpy as np
import ml_dtypes

import concourse.bass as bass
from concourse import bacc
import concourse.tile as tile
import concourse.mybir as mybir
from concourse.bass_utils import run_bass_kernel_spmd

f32 = mybir.dt.float32
f32r = mybir.dt.float32r
bf16 = mybir.dt.bfloat16
AF = mybir.ActivationFunctionType
ALU = mybir.AluOpType

B, S, D = 2, 2048, 1024
H, HKV, HD = 16, 4, 64
MLP_HID = 4 * D
KV = HKV * HD
NT = 16
OWN = 512
EPS_BLOCK = 1e-6
EPS_QK = float(np.finfo(np.float32).eps)
ROPE_BASE = 10000.0

PAIRS = [(0, 4), (1, 5), (2, 6), (3, 7), (8, 12), (9, 13), (10, 14), (11, 15)]

PHASE_ORDER = ["ab", "c", "d", "e", "f"]


def build(q_gain):
    max_ph = os.environ.get("KERNEL_PHASES", "f")
    ph_on = lambda p: PHASE_ORDER.index(p) <= PHASE_ORDER.index(max_ph)
    bacc.Bacc.move_matmul_waits_to_ldweights = lambda self: None
    nc = bacc.Bacc(None)

    def dram_in(name, shape, dt):
        return nc.dram_tensor(name, list(shape), dt, kind="ExternalInput")

    xT = dram_in("xT", (128, 8, S), bf16)
    xq = dram_in("xq", (128, 8, OWN), bf16)
    xres = dram_in("xres", (128, 8, OWN), f32)
    wq = dram_in("wq", (8, 128, 8, 128), bf16)
    wk = dram_in("wk", (128, 8, KV), bf16)
    wv = dram_in("wv", (128, 8, KV), bf16)
    wo = dram_in("wo", (8, 128, 8, 128), bf16)
    wfc = dram_in("wfc", (32, 128, 8, 128), bf16)
    wprojq = dram_in("wprojq", (32, 2, 128, 4, 128), bf16)
    cosF = dram_in("cosF", (128, S), f32)
    sinF = dram_in("sinF", (128, S), f32)   # sign-folded: +sin rows 0-31/64-95, -sin rows 32-63/96-127
    cosO = dram_in("cosO", (128, OWN), f32)
    sinO = dram_in("sinO", (128, OWN), f32)
    maskM = dram_in("maskM", (128, 4, 4, 128), bf16)
    permM = dram_in("permM", (128, 128), f32r)     # swap rows i <-> i^32
    oc_h = dram_in("oc_h", (128, 2), bf16)         # col0: top-64 ones; col1: bottom-64 ones
    onescb = dram_in("onescb", (128, 2), bf16)     # all ones
    selg2 = dram_in("selg2", (2, 8, 128), f32)     # row0 -> cols 0-63 * gA/8, row1 -> cols 64-127 * gB/8
    identM = dram_in("identM", (128, 128), bf16)   # 128x128 identity
    selk = dram_in("selk", (2, 128), f32)          # row0 -> cols 0-63 ones, row1 -> cols 64-127 ones
    o10 = dram_in("o10", (2, 128), f32)            # row0 ones, row1 zeros
    g_attn = dram_in("g_attn", (128, 8), f32)
    g_mlp = dram_in("g_mlp", (128, 8), f32)
    b_mlp = dram_in("b_mlp", (128, 8), f32)

    out_t = nc.dram_tensor("out", [128, 8, OWN], f32, kind="ExternalOutput")

    with tile.TileContext(nc) as tc, \
         tc.tile_pool(name="cst", bufs=1) as cst, \
         tc.tile_pool(name="big", bufs=1) as big:
        och = cst.tile([128, 2], bf16, tag="och")
        nc.sync.dma_start(och[:], oc_h[:])
        ocb = cst.tile([128, 2], bf16, tag="ocb")
        nc.sync.dma_start(ocb[:], onescb[:])
        selg = cst.tile([2, 8, 128], f32, tag="selg")
        nc.sync.dma_start(selg[:], selg2[:])
        idt = cst.tile([128, 128], bf16, tag="idt")
        nc.sync.dma_start(idt[:], identM[:])
        selkt = cst.tile([2, 128], f32, tag="selkt")
        nc.sync.dma_start(selkt[:], selk[:])
        o10t = cst.tile([2, 128], f32, tag="o10t")
        nc.sync.dma_start(o10t[:], o10[:])
        perm = cst.tile([128, 128], f32r, tag="perm")
        nc.sync.dma_start(perm[:], permM[:])
        eps6 = cst.tile([128, 1], f32, tag="eps6")
        nc.vector.memset(eps6[:], EPS_BLOCK)
        epsq = cst.tile([128, 1], f32, tag="epsq")
        nc.vector.memset(epsq[:], EPS_QK)
        gat = cst.tile([128, 8], f32, tag="gat")
        nc.sync.dma_start(gat[:], g_attn[:])
        gml = cst.tile([128, 8], f32, tag="gml")
        nc.sync.dma_start(gml[:], g_mlp[:])
        bml = cst.tile([128, 8], f32, tag="bml")
        nc.sync.dma_start(bml[:], b_mlp[:])
        from contextlib import ExitStack
        rope_stack = ExitStack()
        ropep = rope_stack.enter_context(tc.tile_pool(name="ropep", bufs=1))
        cosf = ropep.tile([128, S], f32, tag="cosf")
        nc.sync.dma_start(cosf[:], cosF[:])
        sinf = ropep.tile([128, S], f32, tag="sinf")
        nc.sync.dma_start(sinf[:], sinF[:])
        coso = ropep.tile([128, OWN], f32, tag="coso")
        nc.sync.dma_start(coso[:], cosO[:])
        sino = ropep.tile([128, OWN], f32, tag="sino")
        nc.sync.dma_start(sino[:], sinO[:])

        kT = big.tile([128, 2, S], f32r, tag="kT")
        v_all = big.tile([128, 4, NT, 66], bf16, tag="v_all")
        qT = big.tile([128, 8, OWN], f32r, tag="qT")
        y_all = big.tile([128, 8, OWN], bf16, tag="y_all")
        xrs = big.tile([128, 8, OWN], f32, tag="xrs_mout")
        nc.sync.dma_start(xrs[:], xres[:])
        invr1 = big.tile([128, NT], f32, tag="invr1")
        rms_st = big.tile([128, NT], f32, tag="rms_st")

        # ------------- Phase AB: rms1, K, V over full sequence ------------
        absub = int(os.environ.get("KERNEL_AB_SUB", "99"))
        if ph_on("ab"):
            with tc.tile_pool(name="pab_x", bufs=3) as pab_x, \
                 tc.tile_pool(name="pab_sb", bufs=2) as pab_sb, \
                 tc.tile_pool(name="pab_w", bufs=1) as pab_w, \
                 tc.tile_pool(name="pab_ps", bufs=2, space="PSUM") as pab_ps, \
                 tc.tile_pool(name="pab_ps1", bufs=1, space="PSUM") as pab_ps1:
                wvs = pab_w.tile([128, 8, KV], bf16, tag="wvs")
                nc.sync.dma_start(wvs[:], wv[:])
                wks = pab_w.tile([128, 8, KV], bf16, tag="wks")
                nc.sync.dma_start(wks[:], wk[:])
                for ci in range(4):
                    sl = slice(ci * 512, (ci + 1) * 512)
                    xc = pab_x.tile([128, 8, 512], bf16, tag="xc")
                    nc.sync.dma_start(xc[:], xT[:, :, sl])
                    # token-major sumsq -> invr1 for the 4 token tiles of the chunk
                    for kt in range(4):
                        x2 = pab_sb.tile([128, 8, 128], bf16, tag="x2")
                        nc.scalar.activation(x2[:], xc[:, :, kt * 128:(kt + 1) * 128],
                                             AF.Square)
                        ssp = pab_ps1.tile([128, 2], f32, tag="sstk")
                        for k in range(8):
                            nc.tensor.matmul(ssp[:], x2[:, k, :], ocb[:, 0:2],
                                             start=(k == 0), stop=(k == 7))
                        nc.scalar.activation(rms_st[:, ci * 4 + kt, None], ssp[:, 0:1],
                                             AF.Sqrt, scale=1.0 / D, bias=eps6[:])
                    nc.vector.reciprocal(invr1[:, ci * 4:(ci + 1) * 4],
                                         rms_st[:, ci * 4:(ci + 1) * 4])
                    # V token-major for the 4 token tiles
                    for kt in range(4 if absub >= 2 else 0):
                        gkt = ci * 4 + kt
                        vps = pab_ps.tile([128, KV], f32, tag="vps")
                        for k in range(8):
                            nc.tensor.matmul(vps[:], xc[:, k, kt * 128:(kt + 1) * 128],
                                             wvs[:, k, :], start=(k == 0), stop=(k == 7))
                        nc.vector.tensor_scalar_mul(
                            v_all[:, :, gkt, 0:64],
                            vps[:].rearrange("p (g d) -> p g d", g=4),
                            invr1[:, gkt, None])
                    # K feature-major for both kv pairs
                    for kp in range(2 if absub >= 3 else 0):
                        kps = pab_ps.tile([128, 512], f32, tag="kps")
                        for k in range(8):
                            nc.tensor.matmul(kps[:], wks[:, k, kp * 128:(kp + 1) * 128],
                                             xc[:, k, :], start=(k == 0), stop=(k == 7))
                        kraw = pab_sb.tile([128, 512], f32r, tag="kraw")
                        nc.any.tensor_copy(kraw[:], kps[:])
                        ksw = pab_ps1.tile([128, 512], f32, tag="ksw")
                        nc.tensor.matmul(ksw[:], perm[:], kraw[:], start=True, stop=True)
                        if absub < 4:
                            continue
                        k2 = pab_sb.tile([128, 512], bf16, tag="k2")
                        nc.scalar.activation(k2[:], kps[:], AF.Square)
                        # per-token sumsq token-major, then transpose to row-major
                        sstk = pab_ps1.tile([128, 4, 2], f32, tag="sstk")
                        for kt in range(4):
                            ksl = slice(kt * 128, (kt + 1) * 128)
                            nc.tensor.matmul(sstk[:, kt, :], k2[:, ksl], och[:, 0:2],
                                             start=True, stop=True)
                        sstk_sb = pab_sb.tile([128, 4, 2], bf16, tag="sstk_sb")
                        nc.any.tensor_copy(sstk_sb[:], sstk[:])
                        ssrow = pab_ps1.tile([2, 512], f32, tag="ssrow")
                        for kt in range(4):
                            ksl = slice(kt * 128, (kt + 1) * 128)
                            nc.tensor.matmul(ssrow[0:2, ksl], sstk_sb[:, kt, :],
                                             idt[:], start=True, stop=True)
                        rmsk = pab_sb.tile([2, 512], f32, tag="rmsk")
                        nc.scalar.activation(rmsk[:], ssrow[0:2, :], AF.Sqrt,
                                             scale=1.0 / HD, bias=epsq[0:2, :])
                        invk = pab_sb.tile([2, 512], f32, tag="invk")
                        nc.vector.reciprocal_approx_fast(invk[:], rmsk[:])
                        repk = pab_ps1.tile([128, 512], f32, tag="repk")
                        nc.tensor.matmul(repk[:], selkt[:], invk[:],
                                         start=True, stop=True)
                        if absub < 5:
                            continue
                        t1 = pab_sb.tile([128, 512], f32, tag="t1k")
                        nc.vector.tensor_tensor(t1[:], kraw[:], cosf[:, sl], ALU.mult)
                        t2 = pab_sb.tile([128, 512], f32, tag="t2k")
                        nc.vector.tensor_tensor(t2[:], ksw[:], sinf[:, sl], ALU.mult)
                        t3 = pab_sb.tile([128, 512], f32, tag="t3k")
                        nc.vector.tensor_tensor(t3[:], t1[:], t2[:], ALU.add)
                        nc.vector.tensor_tensor(kT[:, kp, sl], t3[:], repk[:], ALU.mult)
                # ones column of V
                nc.vector.tensor_copy(
                    v_all[:, :, :, 64:66],
                    ocb[:, 0, None, None].to_broadcast([128, 4, NT, 2]))

        # ------------- Phase C: Q for own tokens --------------------------
        if ph_on("c"):
            with tc.tile_pool(name="pc_x", bufs=1) as pc_x, \
                 tc.tile_pool(name="pc_sb", bufs=3) as pc_sb, \
                 tc.tile_pool(name="pc_w", bufs=3) as pc_w, \
                 tc.tile_pool(name="pc_ps", bufs=2, space="PSUM") as pc_ps, \
                 tc.tile_pool(name="pc_ps1", bufs=1, space="PSUM") as pc_ps1:
                xqs = pc_x.tile([128, 8, OWN], bf16, tag="xqs")
                nc.sync.dma_start(xqs[:], xq[:])
                for p in range(8):
                    wqs = pc_w.tile([128, 8, 128], bf16, tag="wqs")
                    nc.sync.dma_start(wqs[:], wq[p])
                    qps = pc_ps.tile([128, OWN], f32, tag="qps")
                    for k in range(8):
                        nc.tensor.matmul(qps[:], wqs[:, k, :], xqs[:, k, :],
                                         start=(k == 0), stop=(k == 7))
                    qraw = pc_sb.tile([128, OWN], f32r, tag="qraw")
                    nc.any.tensor_copy(qraw[:], qps[:])
                    qsw = pc_ps.tile([128, OWN], f32, tag="qsw")
                    nc.tensor.matmul(qsw[:], perm[:], qraw[:], start=True, stop=True)
                    q2 = pc_sb.tile([128, OWN], bf16, tag="q2")
                    nc.scalar.activation(q2[:], qps[:], AF.Square)
                    sstq = pc_ps1.tile([128, 4, 2], f32, tag="sstq")
                    for kt in range(4):
                        ksl = slice(kt * 128, (kt + 1) * 128)
                        nc.tensor.matmul(sstq[:, kt, :], q2[:, ksl], och[:, 0:2],
                                         start=True, stop=True)
                    sstq_sb = pc_sb.tile([128, 4, 2], bf16, tag="sstq_sb")
                    nc.any.tensor_copy(sstq_sb[:], sstq[:])
                    ssqrow = pc_ps1.tile([2, OWN], f32, tag="ssqrow")
                    for kt in range(4):
                        ksl = slice(kt * 128, (kt + 1) * 128)
                        nc.tensor.matmul(ssqrow[0:2, ksl], sstq_sb[:, kt, :],
                                         idt[:], start=True, stop=True)
                    rmsq = pc_sb.tile([2, OWN], f32, tag="rmsq")
                    nc.scalar.activation(rmsq[:], ssqrow[0:2, :], AF.Sqrt,
                                         scale=1.0 / HD, bias=epsq[0:2, :])
                    invq = pc_sb.tile([2, OWN], f32, tag="invq")
                    nc.vector.reciprocal_approx_fast(invq[:], rmsq[:])
                    repq = pc_ps1.tile([128, OWN], f32, tag="repq")
                    nc.tensor.matmul(repq[:], selg[:, p, :], invq[:],
                                     start=True, stop=True)
                    t1 = pc_sb.tile([128, OWN], f32, tag="t1q")
                    nc.vector.tensor_tensor(t1[:], qraw[:], coso[:], ALU.mult)
                    t2 = pc_sb.tile([128, OWN], f32, tag="t2q")
                    nc.vector.tensor_tensor(t2[:], qsw[:], sino[:], ALU.mult)
                    t3 = pc_sb.tile([128, OWN], f32, tag="t3q")
                    nc.vector.tensor_tensor(t3[:], t1[:], t2[:], ALU.add)
                    nc.vector.tensor_tensor(qT[:, p, :], t3[:], repq[:], ALU.mult)

            rope_stack.close()

        # ------------- Phase D: attention ---------------------------------
        if ph_on("d"):
            xpr = big.tile([128, 8, OWN], bf16, tag="xpr")
            xpb = big.tile([128, 8, OWN], f32, tag="xpb")
            with tc.tile_pool(name="pd_m", bufs=1) as pd_m, \
                 tc.tile_pool(name="pd_pt", bufs=6) as pd_pt, \
                 tc.tile_pool(name="pd_sb", bufs=2) as pd_sb, \
                 tc.tile_pool(name="pd_s", bufs=2, space="PSUM") as pd_s, \
                 tc.tile_pool(name="pd_y", bufs=1, space="PSUM") as pd_y, \
                 tc.tile_pool(name="pd_r", bufs=1, space="PSUM") as pd_r:
                masks = pd_m.tile([128, 4, 4, 128], bf16, tag="masks")
                nc.sync.dma_start(masks[:], maskM[:])
                for t in range(4):
                    qsl = slice(t * 128, (t + 1) * 128)
                    n_chunks = t + 1
                    n_kvt = 4 * n_chunks
                    for half in range(2):
                        gA, gB = 2 * half, 2 * half + 1
                        yA = pd_y.tile([66, 4, 128], f32, tag="yA")
                        yB = pd_y.tile([66, 4, 128], f32, tag="yB")
                        qsA = qT[0:64, 4 * half:4 * half + 4, qsl]
                        qsB = qT[64:128, 4 * half:4 * half + 4, qsl]
                        for c in range(n_chunks):
                            pts = []
                            for i in range(4):
                                ks = slice((4 * c + i) * 128, (4 * c + i + 1) * 128)
                                psAB = pd_s.tile([128, 2, 4, 128], f32, tag="psAB")
                                nc.tensor.matmul(psAB[:, 0, :, :],
                                                 kT[0:64, half, ks], qsA,
                                                 start=True, stop=True,
                                                 tile_position=(0, 0))
                                nc.tensor.matmul(psAB[:, 1, :, :],
                                                 kT[64:128, half, ks], qsB,
                                                 start=True, stop=True,
                                                 tile_position=(64, 0))
                                ptAB = pd_pt.tile([128, 2, 4, 128], bf16, tag="ptAB")
                                nc.scalar.activation(ptAB[:], psAB[:], AF.Exp)
                                if c == t:
                                    mbc = masks[:, t, i, None, None, :].to_broadcast(
                                        [128, 2, 4, 128])
                                    eng = nc.vector if i % 2 == 0 else nc.gpsimd
                                    eng.tensor_tensor(ptAB[:], ptAB[:], mbc, ALU.mult)
                                pts.append(ptAB)
                            for i in range(4):
                                kvt = 4 * c + i
                                nc.tensor.matmul(yA[:], v_all[:, gA, kvt, :],
                                                 pts[i][:, 0, :, :], start=(kvt == 0),
                                                 stop=(kvt == n_kvt - 1))
                                nc.tensor.matmul(yB[:], v_all[:, gB, kvt, :],
                                                 pts[i][:, 1, :, :], start=(kvt == 0),
                                                 stop=(kvt == n_kvt - 1))
                        for g, y in ((gA, yA), (gB, yB)):
                            dsb = pd_sb.tile([2, 4, 128], f32, tag="dsb")
                            nc.vector.tensor_copy(dsb[:], y[64:66, :, :])
                            invs = pd_sb.tile([2, 4, 128], f32, tag="invs")
                            nc.vector.reciprocal_approx_fast(invs[:], dsb[:])
                            ysb = pd_sb.tile([64, 4, 128], f32, tag="ysb")
                            nc.vector.tensor_copy(ysb[:], y[0:64, :, :])
                            repy = pd_r.tile([64, 4, 128], f32, tag="repy")
                            nc.tensor.matmul(repy[:].rearrange("p a b -> p (a b)"),
                                             o10t[:, 0:64],
                                             invs[:].rearrange("p a b -> p (a b)"),
                                             start=True, stop=True)
                            for i in range(4):
                                h = 4 * g + i
                                chunk, part = h // 2, (h % 2) * 64
                                nc.vector.tensor_tensor(
                                    y_all[part:part + 64, chunk, qsl],
                                    ysb[:, i, :], repy[:, i, :], ALU.mult)

        # ------------- Phase E: Wo + post-norm + residual -----------------
        if ph_on("e"):
            with tc.tile_pool(name="pe_sb", bufs=2) as pe_sb, \
                 tc.tile_pool(name="pe_ao", bufs=1) as pe_ao, \
                 tc.tile_pool(name="pe_w", bufs=1) as pe_w, \
                 tc.tile_pool(name="pe_ps", bufs=2, space="PSUM") as pe_ps, \
                 tc.tile_pool(name="pe_ss", bufs=1, space="PSUM") as pe_ss:
                ao = pe_ao.tile([128, 8, OWN], f32, tag="ao")
                ssa = pe_ss.tile([2, OWN], f32, tag="ssa")
                wos_l = []
                for o in range(8):
                    wos = pe_w.tile([128, 8, 128], bf16, tag=f"wos{o}")
                    nc.sync.dma_start(wos[:], wo[o])
                    wos_l.append(wos)
                for tt in range(4):
                    tsl = slice(tt * 128, (tt + 1) * 128)
                    for o in range(8):
                        aps = pe_ps.tile([128, 128], f32, tag="aps")
                        for k in range(8):
                            nc.tensor.matmul(aps[:], wos_l[o][:, k, :],
                                             y_all[:, k, tsl],
                                             start=(k == 0), stop=(k == 7))
                        nc.any.tensor_copy(ao[:, o, tsl], aps[:])
                        a2 = pe_sb.tile([128, 128], bf16, tag="a2")
                        nc.scalar.activation(a2[:], aps[:], AF.Square)
                        nc.tensor.matmul(ssa[0:2, tsl], ocb[:, 0:2], a2[:],
                                         start=(o == 0), stop=(o == 7))
                rmsa = pe_sb.tile([2, OWN], f32, tag="rmsa")
                nc.scalar.activation(rmsa[:], ssa[0:2, :], AF.Sqrt,
                                     scale=1.0 / D, bias=eps6[0:2, :])
                inva = pe_sb.tile([2, OWN], f32, tag="inva")
                nc.vector.reciprocal_approx_fast(inva[:], rmsa[:])
                repa = pe_ss.tile([128, OWN], f32, tag="repa")
                nc.tensor.matmul(repa[:], o10t[:], inva[:], start=True, stop=True)
                for o in range(8):
                    t1 = pe_sb.tile([128, OWN], f32, tag="t1e")
                    nc.vector.tensor_tensor(t1[:], ao[:, o, :], repa[:], ALU.mult)
                    nc.vector.scalar_tensor_tensor(
                        xpb[:, o, :], t1[:], gat[:, o, None], xrs[:, o, :],
                        ALU.mult, ALU.add)
                    nc.any.tensor_copy(xpr[:, o, :], xpb[:, o, :])
                    nc.vector.tensor_scalar_add(xpb[:, o, :], xpb[:, o, :],
                                                bml[:, o, None])

        # ------------- Phase F: MLP ---------------------------------------
        if ph_on("f"):
            mout = big.tile([128, 8, OWN], f32, tag="xrs_mout")
            with tc.tile_pool(name="pf_h2", bufs=1) as pf_h2, \
                 tc.tile_pool(name="pf_sb", bufs=2) as pf_sb, \
                 tc.tile_pool(name="pf_wf", bufs=3) as pf_wf, \
                 tc.tile_pool(name="pf_wp", bufs=3) as pf_wp, \
                 tc.tile_pool(name="pf_ps", bufs=2, space="PSUM") as pf_ps, \
                 tc.tile_pool(name="pf_mo", bufs=1, space="PSUM") as pf_mo:
                h2 = pf_h2.tile([128, 32, OWN], bf16, tag="h2")
                for hc in range(32):
                    wfs = pf_wf.tile([128, 8, 128], bf16, tag="wfs")
                    nc.sync.dma_start(wfs[:], wfc[hc])
                    hps = pf_ps.tile([128, OWN], f32, tag="hps")
                    for k in range(8):
                        nc.tensor.matmul(hps[:], wfs[:, k, :], xpr[:, k, :],
                                         start=(k == 0), stop=(k == 7))
                    hr = pf_sb.tile([128, OWN], bf16, tag="hr")
                    nc.scalar.activation(hr[:], hps[:], AF.Relu)
                    nc.vector.tensor_tensor(h2[:, hc, :], hr[:], hr[:], ALU.mult)
                ssm = pf_ps.tile([2, OWN], f32, tag="ssm")
                for ohalf in range(2):
                    mo_ps = [pf_mo.tile([128, OWN], f32, name=f"mo{oi}", tag=f"mo{oi}")
                             for oi in range(4)]
                    for hc in range(32):
                        wps = pf_wp.tile([128, 4, 128], bf16, tag="wps")
                        nc.sync.dma_start(wps[:], wprojq[hc, ohalf])
                        for oi in range(4):
                            nc.tensor.matmul(mo_ps[oi][:], wps[:, oi, :], h2[:, hc, :],
                                             start=(hc == 0), stop=(hc == 31))
                    for oi in range(4):
                        o = ohalf * 4 + oi
                        nc.any.tensor_copy(mout[:, o, :], mo_ps[oi][:])
                        m2 = pf_sb.tile([128, OWN], bf16, tag="m2")
                        nc.scalar.activation(m2[:], mo_ps[oi][:], AF.Square)
                        nc.tensor.matmul(ssm[:], ocb[:, 0:2], m2[:],
                                         start=(o == 0), stop=(o == 7))
                rmsm = pf_sb.tile([2, OWN], f32, tag="rmsm")
                nc.scalar.activation(rmsm[:], ssm[0:2, :], AF.Sqrt, scale=1.0 / D,
                                     bias=eps6[0:2, :])
                invm = pf_sb.tile([2, OWN], f32, tag="invm")
                nc.vector.reciprocal_approx_fast(invm[:], rmsm[:])
                repm = pf_ps.tile([128, OWN], f32, tag="hps")
                nc.tensor.matmul(repm[:], o10t[:], invm[:], start=True, stop=True)
                for o in range(8):
                    t1 = pf_sb.tile([128, OWN], f32, tag="t1f")
                    nc.vector.tensor_tensor(t1[:], mout[:, o, :], repm[:], ALU.mult)
                    outv = pf_sb.tile([128, OWN], f32, tag="outv")
                    nc.vector.scalar_tensor_tensor(
                        outv[:], t1[:], gml[:, o, None], xpb[:, o, :],
                        ALU.mult, ALU.add)
                    nc.sync.dma_start(out_t[:, o, :], outv[:])

        if not ph_on("f"):
            with tc.tile_pool(name="dummy", bufs=1) as dp:
                zout = dp.tile([128, 8, OWN], f32, tag="zout")
                nc.vector.memset(zout[:], 0.0)
                nc.sync.dma_start(out_t[:], zout[:])
            rope_stack.close()

    nc.finalize()
    return nc


def _feat_major(a):
    """[F, T] -> device layout [128, F//128, T]."""
    F, T = a.shape
    return np.ascontiguousarray(a.reshape(F // 128, 128, T).transpose(1, 0, 2))


def _vec_dev(v):
    return np.ascontiguousarray(v.reshape(-1, 128).T)


def _bf(a):
    return np.ascontiguousarray(a.astype(ml_dtypes.bfloat16))


_CACHE = {}
_RUN_KW = {}


def kernel(x, attn_norm_w, mlp_norm_w, attn_post_norm_w, mlp_post_norm_w,
           attn_scale, mlp_scale, attn_mod_gain, attn_mod_bias,
           mlp_mod_gain, mlp_mod_bias, Wq, Wk, Wv, Wo, q_gain, fc_w, proj_w):
    x = np.asarray(x, np.float32)
    q_gain = np.asarray(q_gain, np.float32)

    if "nc" not in _CACHE:
        _CACHE["nc"] = build(q_gain)
    nc = _CACHE["nc"]

    anw = np.asarray(attn_norm_w, np.float32)
    mnw = np.asarray(mlp_norm_w, np.float32)
    wq_eff = np.asarray(Wq, np.float32) * anw[None, :]
    wk_eff = np.asarray(Wk, np.float32) * anw[None, :]
    wv_eff = np.asarray(Wv, np.float32) * anw[None, :]
    fc_eff = np.asarray(fc_w, np.float32) * mnw[None, :]

    perm = np.zeros(D, np.int64)
    for p, (a, b) in enumerate(PAIRS):
        perm[p * 128:p * 128 + 64] = np.arange(a * 64, a * 64 + 64)
        perm[p * 128 + 64:(p + 1) * 128] = np.arange(b * 64, b * 64 + 64)
    WqTp = wq_eff.T[:, perm]                                  # [D_in, D_out-perm]
    wq_dev = _bf(np.stack([_feat_major(WqTp[:, p * 128:(p + 1) * 128]) for p in range(8)]))
    wk_dev = _bf(_feat_major(wk_eff.T))
    wv_dev = _bf(_feat_major(wv_eff.T))
    WoT = np.asarray(Wo, np.float32).T
    wo_dev = _bf(np.stack([_feat_major(WoT[:, o * 128:(o + 1) * 128]) for o in range(8)]))
    fcT = fc_eff.T
    wfc_dev = _bf(np.stack([_feat_major(fcT[:, h * 128:(h + 1) * 128]) for h in range(32)]))
    projT = np.asarray(proj_w, np.float32).T                  # [4096, 1024]
    wproj_dev = _bf(np.ascontiguousarray(
        projT.reshape(32, 128, 2, 4, 128).transpose(0, 2, 1, 3, 4)))

    inv_freq = 1.0 / (ROPE_BASE ** (np.arange(0, HD, 2, dtype=np.float32) / HD))
    tpos = np.arange(S, dtype=np.float32)
    freqs = np.outer(tpos, inv_freq).astype(np.float32)
    cosT = np.ascontiguousarray(np.tile(np.cos(freqs).T, (4, 1)))   # [128, S]
    sin1 = np.sin(freqs).T                                          # [32, S]
    sinS = np.ascontiguousarray(
        np.concatenate([sin1, -sin1, sin1, -sin1], axis=0))         # [128, S] sign-folded

    # swap permutation i <-> i^32 (within each 64-wide head)
    permM_h = np.zeros((128, 128), np.float32)
    for i in range(128):
        permM_h[i, i ^ 32] = 1.0

    oc_h_v = np.zeros((128, 2), np.float32)
    oc_h_v[0:64, 0] = 1.0
    oc_h_v[64:128, 1] = 1.0
    selg2_v = np.zeros((2, 8, 128), np.float32)
    for p, (a, b) in enumerate(PAIRS):
        selg2_v[0, p, 0:64] = q_gain[a] / 8.0
        selg2_v[1, p, 64:128] = q_gain[b] / 8.0
    ones10_v = np.concatenate([np.ones((1, 128), np.float32),
                               np.zeros((1, 128), np.float32)])

    gat_v = (np.asarray(attn_post_norm_w, np.float32)
             * np.asarray(attn_mod_gain, np.float32)
             * np.asarray(attn_scale, np.float32))
    bat_v = np.asarray(attn_mod_bias, np.float32) * np.asarray(attn_scale, np.float32)
    gml_v = (np.asarray(mlp_post_norm_w, np.float32)
             * np.asarray(mlp_mod_gain, np.float32)
             * np.asarray(mlp_scale, np.float32))
    bml_v = np.asarray(mlp_mod_bias, np.float32) * np.asarray(mlp_scale, np.float32)

    shared = {
        "wq": wq_dev, "wk": wk_dev, "wv": wv_dev, "wo": wo_dev,
        "wfc": wfc_dev, "wprojq": wproj_dev,
        "cosF": cosT, "sinF": sinS,
        "permM": permM_h,
        "oc_h": _bf(oc_h_v),
        "onescb": _bf(np.ones((128, 2), np.float32)),
        "selg2": selg2_v,
        "identM": _bf(np.eye(128, dtype=np.float32)),
        "selk": np.stack([np.concatenate([np.ones(64, np.float32), np.zeros(64, np.float32)]),
                          np.concatenate([np.zeros(64, np.float32), np.ones(64, np.float32)])]),
        "o10": ones10_v,
        "g_attn": _vec_dev(gat_v), "g_mlp": _vec_dev(gml_v),
        "b_mlp": _vec_dev(bml_v),
    }

    in_maps = []
    owners = []
    for c in range(8):
        b, j = c // 4, c % 4
        rows = np.concatenate(
            [np.arange((j + 4 * t) * 128, (j + 4 * t + 1) * 128) for t in range(4)])
        owners.append((b, rows))
        xb = x[b].T
        x_own = xb[:, rows]
        mask = np.zeros((4, 4, 128, 128), np.float32)
        for t in range(4):
            m = j + 4 * t
            q_idx = m * 128 + np.arange(128)
            for ktl in range(4):
                kv_idx = 512 * t + 128 * ktl + np.arange(128)
                mask[t, ktl] = (kv_idx[:, None] <= q_idx[None, :])
        m_in = {
            "xT": _bf(_feat_major(xb)),
            "xq": _bf(_feat_major(x_own)),
            "xres": _feat_major(x_own + bat_v[:, None]),
            "cosO": np.ascontiguousarray(cosT[:, rows]),
            "sinO": np.ascontiguousarray(sinS[:, rows]),
            "maskM": _bf(np.ascontiguousarray(mask.transpose(2, 0, 1, 3))),
        }
        m_in.update(shared)
        in_maps.append(m_in)

    res = run_bass_kernel_spmd(nc, in_maps, core_ids=list(range(8)),
                               **_RUN_KW)
    _CACHE["last_result"] = res

    out = np.empty((B, S, D), np.float32)
    for c in range(8):
        b, rows = owners[c]
        o = res.results[c]["out"]
        out[b, rows, :] = o.transpose(2, 1, 0).reshape(OWN, D)
    return out


# revision 48
# speedup vs baseline: 1.0158x; 1.0158x over previous
"""Trainium2 Bass kernel for one dense transformer block (B=2, S=2048, D=1024,
16 q-heads / 4 kv-heads GQA, squared-ReLU MLP), data-parallel over 8 NeuronCores.

Sharding: core c = (b, j), b = c // 4, j = c % 4, owns q-token tiles
{j, j+4, j+8, j+12} (128 tokens each) of batch b. K/V are computed for the full
sequence on every core (no collectives). The kv range for own q-tile t is
padded to 512*(t+1); causality enforced with per-core 0/1 masks on the last
512-wide kv chunk.

Numerical identities used (exact up to negligible eps rescaling):
  - per-head q/k rmsnorm is scale-invariant per token, so the block input
    rmsnorm cancels inside it -> Q/K project from raw (norm-weight-folded) x
  - the MLP input rmsnorm cancels through relu()^2 -> proj -> post-rmsnorm
  - V is projected from raw x and rescaled by 1/rms1(x) per token
  - no softmax max-subtraction (logits bounded by |q||k|/8 = 8)
  - softmax denominator = ones-column appended to V in the AV matmul
  - K's 1/rms is applied as a per-partition AP scale inside the exp
    activation (kv tokens are partitions in the score tiles); Q's 1/rms and
    q_gain/8 ride a replicate matmul onto qT

v2 perf changes vs baseline:
  - no DVE reciprocal with f32r destination (was ~7.7ns/elem); all recips are
    fp32->fp32 on DVE, replicates via small fp32 matmuls
  - rope via a feature-swap permutation matmul + 3 full-width DVE ops
    (was 12 narrow DVE ops)
  - bf16 weights + x + V/p/mask/y/h2 paths (half DMA, FWL weight loads,
    2x DVE); q/k/scores stay f32r
"""

import os

import numpy as np
import ml_dtypes

import concourse.bass as bass
from concourse import bacc
import concourse.tile as tile
import concourse.mybir as mybir
from concourse.bass_utils import run_bass_kernel_spmd

f32 = mybir.dt.float32
f32r = mybir.dt.float32r
bf16 = mybir.dt.bfloat16
AF = mybir.ActivationFunctionType
ALU = mybir.AluOpType

B, S, D = 2, 2048, 1024
H, HKV, HD = 16, 4, 64
MLP_HID = 4 * D
KV = HKV * HD
NT = 16
OWN = 512
EPS_BLOCK = 1e-6
EPS_QK = float(np.finfo(np.float32).eps)
ROPE_BASE = 10000.0

PAIRS = [(0, 4), (1, 5), (2, 6), (3, 7), (8, 12), (9, 13), (10, 14), (11, 15)]

PHASE_ORDER = ["ab", "c", "d", "e", "f"]


def build(q_gain):
    max_ph = os.environ.get("KERNEL_PHASES", "f")
    ph_on = lambda p: PHASE_ORDER.index(p) <= PHASE_ORDER.index(max_ph)
    bacc.Bacc.move_matmul_waits_to_ldweights = lambda self: None
    nc = bacc.Bacc(None)

    def dram_in(name, shape, dt):
        return nc.dram_tensor(name, list(shape), dt, kind="ExternalInput")

    xT = dram_in("xT", (128, 8, S), bf16)
    xq = dram_in("xq", (128, 8, OWN), bf16)
    xres = dram_in("xres", (128, 8, OWN), f32)
    wq = dram_in("wq", (8, 128, 8, 128), bf16)
    wk = dram_in("wk", (128, 8, KV), bf16)
    wv = dram_in("wv", (128, 8, KV), bf16)
    wo = dram_in("wo", (8, 128, 8, 128), bf16)
    wfc = dram_in("wfc", (32, 128, 8, 128), bf16)
    wprojq = dram_in("wprojq", (32, 2, 128, 4, 128), bf16)
    cosF = dram_in("cosF", (128, S), f32)
    sinF = dram_in("sinF", (128, S), f32)   # sign-folded: +sin rows 0-31/64-95, -sin rows 32-63/96-127
    cosO = dram_in("cosO", (128, OWN), f32)
    sinO = dram_in("sinO", (128, OWN), f32)
    maskM = dram_in("maskM", (128, 4, 4, 128), bf16)
    permM = dram_in("permM", (128, 128), f32r)     # swap rows i <-> i^32
    oc_h = dram_in("oc_h", (128, 2), bf16)         # col0: top-64 ones; col1: bottom-64 ones
    onescb = dram_in("onescb", (128, 2), bf16)     # all ones
    selg2 = dram_in("selg2", (2, 8, 128), f32)     # row0 -> cols 0-63 * gA/8, row1 -> cols 64-127 * gB/8
    identM = dram_in("identM", (128, 128), bf16)   # 128x128 identity
    selk = dram_in("selk", (2, 128), f32)          # row0 -> cols 0-63 ones, row1 -> cols 64-127 ones
    o10 = dram_in("o10", (2, 128), f32)            # row0 ones, row1 zeros
    g_attn = dram_in("g_attn", (128, 8), f32)
    g_mlp = dram_in("g_mlp", (128, 8), f32)
    b_mlp = dram_in("b_mlp", (128, 8), f32)

    out_t = nc.dram_tensor("out", [128, 8, OWN], f32, kind="ExternalOutput")

    with tile.TileContext(nc) as tc, \
         tc.tile_pool(name="cst", bufs=1) as cst, \
         tc.tile_pool(name="big", bufs=1) as big:
        och = cst.tile([128, 2], bf16, tag="och")
        nc.sync.dma_start(och[:], oc_h[:])
        ocb = cst.tile([128, 2], bf16, tag="ocb")
        nc.sync.dma_start(ocb[:], onescb[:])
        selg = cst.tile([2, 8, 128], f32, tag="selg")
        nc.sync.dma_start(selg[:], selg2[:])
        idt = cst.tile([128, 128], bf16, tag="idt")
        nc.sync.dma_start(idt[:], identM[:])
        selkt = cst.tile([2, 128], f32, tag="selkt")
        nc.sync.dma_start(selkt[:], selk[:])
        o10t = cst.tile([2, 128], f32, tag="o10t")
        nc.sync.dma_start(o10t[:], o10[:])
        perm = cst.tile([128, 128], f32r, tag="perm")
        nc.sync.dma_start(perm[:], permM[:])
        eps6 = cst.tile([128, 1], f32, tag="eps6")
        nc.vector.memset(eps6[:], EPS_BLOCK)
        epsq = cst.tile([128, 1], f32, tag="epsq")
        nc.vector.memset(epsq[:], EPS_QK)
        gat = cst.tile([128, 8], f32, tag="gat")
        nc.sync.dma_start(gat[:], g_attn[:])
        gml = cst.tile([128, 8], f32, tag="gml")
        nc.sync.dma_start(gml[:], g_mlp[:])
        bml = cst.tile([128, 8], f32, tag="bml")
        nc.sync.dma_start(bml[:], b_mlp[:])
        from contextlib import ExitStack
        rope_stack = ExitStack()
        ropep = rope_stack.enter_context(tc.tile_pool(name="ropep", bufs=1))
        cosf = ropep.tile([128, S], f32, tag="cosf")
        nc.sync.dma_start(cosf[:], cosF[:])
        sinf = ropep.tile([128, S], f32, tag="sinf")
        nc.sync.dma_start(sinf[:], sinF[:])
        coso = ropep.tile([128, OWN], f32, tag="coso")
        nc.sync.dma_start(coso[:], cosO[:])
        sino = ropep.tile([128, OWN], f32, tag="sino")
        nc.sync.dma_start(sino[:], sinO[:])

        kT = big.tile([128, 2, S], f32r, tag="kT")
        v_all = big.tile([128, 4, NT, 66], bf16, tag="v_all")
        qT = big.tile([128, 8, OWN], f32r, tag="qT")
        y_all = big.tile([128, 8, OWN], bf16, tag="y_all")
        xrs = big.tile([128, 8, OWN], f32, tag="xrs_mout")
        nc.sync.dma_start(xrs[:], xres[:])
        invr1 = big.tile([128, NT], f32, tag="invr1")
        rms_st = big.tile([128, NT], f32, tag="rms_st")

        # ------------- Phase AB: rms1, K, V over full sequence ------------
        absub = int(os.environ.get("KERNEL_AB_SUB", "99"))
        if ph_on("ab"):
            with tc.tile_pool(name="pab_x", bufs=3) as pab_x, \
                 tc.tile_pool(name="pab_sb", bufs=3) as pab_sb, \
                 tc.tile_pool(name="pab_w", bufs=1) as pab_w, \
                 tc.tile_pool(name="pab_ps", bufs=2, space="PSUM") as pab_ps, \
                 tc.tile_pool(name="pab_ps1", bufs=1, space="PSUM") as pab_ps1:
                wvs = pab_w.tile([128, 8, KV], bf16, tag="wvs")
                nc.sync.dma_start(wvs[:], wv[:])
                wks = pab_w.tile([128, 8, KV], bf16, tag="wks")
                nc.sync.dma_start(wks[:], wk[:])
                for ci in range(4):
                    sl = slice(ci * 512, (ci + 1) * 512)
                    xc = pab_x.tile([128, 8, 512], bf16, tag="xc")
                    nc.sync.dma_start(xc[:], xT[:, :, sl])
                    # token-major sumsq -> invr1 for the 4 token tiles of the chunk
                    for kt in range(4):
                        x2 = pab_sb.tile([128, 8, 128], bf16, tag="x2")
                        nc.scalar.activation(x2[:], xc[:, :, kt * 128:(kt + 1) * 128],
                                             AF.Square)
                        ssp = pab_ps1.tile([128, 2], f32, tag="sstk")
                        for k in range(8):
                            nc.tensor.matmul(ssp[:], x2[:, k, :], ocb[:, 0:2],
                                             start=(k == 0), stop=(k == 7))
                        nc.scalar.activation(rms_st[:, ci * 4 + kt, None], ssp[:, 0:1],
                                             AF.Sqrt, scale=1.0 / D, bias=eps6[:])
                    nc.vector.reciprocal(invr1[:, ci * 4:(ci + 1) * 4],
                                         rms_st[:, ci * 4:(ci + 1) * 4])
                    # V token-major for the 4 token tiles
                    for kt in range(4 if absub >= 2 else 0):
                        gkt = ci * 4 + kt
                        vps = pab_ps.tile([128, KV], f32, tag="vps")
                        for k in range(8):
                            nc.tensor.matmul(vps[:], xc[:, k, kt * 128:(kt + 1) * 128],
                                             wvs[:, k, :], start=(k == 0), stop=(k == 7))
                        nc.vector.tensor_scalar_mul(
                            v_all[:, :, gkt, 0:64],
                            vps[:].rearrange("p (g d) -> p g d", g=4),
                            invr1[:, gkt, None])
                    # K feature-major for both kv pairs
                    for kp in range(2 if absub >= 3 else 0):
                        kps = pab_ps.tile([128, 512], f32, tag="kps")
                        for k in range(8):
                            nc.tensor.matmul(kps[:], wks[:, k, kp * 128:(kp + 1) * 128],
                                             xc[:, k, :], start=(k == 0), stop=(k == 7))
                        kraw = pab_sb.tile([128, 512], f32r, tag="kraw")
                        nc.any.tensor_copy(kraw[:], kps[:])
                        ksw = pab_ps1.tile([128, 512], f32, tag="ksw")
                        nc.tensor.matmul(ksw[:], perm[:], kraw[:], start=True, stop=True)
                        if absub < 4:
                            continue
                        k2 = pab_sb.tile([128, 512], bf16, tag="k2")
                        nc.scalar.activation(k2[:], kps[:], AF.Square)
                        # per-token sumsq token-major, then transpose to row-major
                        sstk = pab_ps1.tile([128, 4, 2], f32, tag="sstk")
                        for kt in range(4):
                            ksl = slice(kt * 128, (kt + 1) * 128)
                            nc.tensor.matmul(sstk[:, kt, :], k2[:, ksl], och[:, 0:2],
                                             start=True, stop=True)
                        sstk_sb = pab_sb.tile([128, 4, 2], bf16, tag="sstk_sb")
                        nc.any.tensor_copy(sstk_sb[:], sstk[:])
                        ssrow = pab_ps1.tile([2, 512], f32, tag="ssrow")
                        for kt in range(4):
                            ksl = slice(kt * 128, (kt + 1) * 128)
                            nc.tensor.matmul(ssrow[0:2, ksl], sstk_sb[:, kt, :],
                                             idt[:], start=True, stop=True)
                        rmsk = pab_sb.tile([2, 512], f32, tag="rmsk")
                        nc.scalar.activation(rmsk[:], ssrow[0:2, :], AF.Sqrt,
                                             scale=1.0 / HD, bias=epsq[0:2, :])
                        invk = pab_sb.tile([2, 512], f32, tag="invk")
                        nc.vector.reciprocal_approx_fast(invk[:], rmsk[:])
                        repk = pab_ps1.tile([128, 512], f32, tag="repk")
                        nc.tensor.matmul(repk[:], selkt[:], invk[:],
                                         start=True, stop=True)
                        if absub < 5:
                            continue
                        t1 = pab_sb.tile([128, 512], f32, tag="t1k")
                        nc.vector.tensor_tensor(t1[:], kraw[:], cosf[:, sl], ALU.mult)
                        t2 = pab_sb.tile([128, 512], f32, tag="t2k")
                        nc.vector.tensor_tensor(t2[:], ksw[:], sinf[:, sl], ALU.mult)
                        t3 = pab_sb.tile([128, 512], f32, tag="t3k")
                        nc.vector.tensor_tensor(t3[:], t1[:], t2[:], ALU.add)
                        nc.vector.tensor_tensor(kT[:, kp, sl], t3[:], repk[:], ALU.mult)
                # ones column of V
                nc.vector.tensor_copy(
                    v_all[:, :, :, 64:66],
                    ocb[:, 0, None, None].to_broadcast([128, 4, NT, 2]))

        # ------------- Phase C: Q for own tokens --------------------------
        if ph_on("c"):
            with tc.tile_pool(name="pc_x", bufs=1) as pc_x, \
                 tc.tile_pool(name="pc_sb", bufs=3) as pc_sb, \
                 tc.tile_pool(name="pc_w", bufs=3) as pc_w, \
                 tc.tile_pool(name="pc_ps", bufs=2, space="PSUM") as pc_ps, \
                 tc.tile_pool(name="pc_ps1", bufs=1, space="PSUM") as pc_ps1:
                xqs = pc_x.tile([128, 8, OWN], bf16, tag="xqs")
                nc.sync.dma_start(xqs[:], xq[:])
                for p in range(8):
                    wqs = pc_w.tile([128, 8, 128], bf16, tag="wqs")
                    nc.sync.dma_start(wqs[:], wq[p])
                    qps = pc_ps.tile([128, OWN], f32, tag="qps")
                    for k in range(8):
                        nc.tensor.matmul(qps[:], wqs[:, k, :], xqs[:, k, :],
                                         start=(k == 0), stop=(k == 7))
                    qraw = pc_sb.tile([128, OWN], f32r, tag="qraw")
                    nc.any.tensor_copy(qraw[:], qps[:])
                    qsw = pc_ps.tile([128, OWN], f32, tag="qsw")
                    nc.tensor.matmul(qsw[:], perm[:], qraw[:], start=True, stop=True)
                    q2 = pc_sb.tile([128, OWN], bf16, tag="q2")
                    nc.scalar.activation(q2[:], qps[:], AF.Square)
                    sstq = pc_ps1.tile([128, 4, 2], f32, tag="sstq")
                    for kt in range(4):
                        ksl = slice(kt * 128, (kt + 1) * 128)
                        nc.tensor.matmul(sstq[:, kt, :], q2[:, ksl], och[:, 0:2],
                                         start=True, stop=True)
                    sstq_sb = pc_sb.tile([128, 4, 2], bf16, tag="sstq_sb")
                    nc.any.tensor_copy(sstq_sb[:], sstq[:])
                    ssqrow = pc_ps1.tile([2, OWN], f32, tag="ssqrow")
                    for kt in range(4):
                        ksl = slice(kt * 128, (kt + 1) * 128)
                        nc.tensor.matmul(ssqrow[0:2, ksl], sstq_sb[:, kt, :],
                                         idt[:], start=True, stop=True)
                    rmsq = pc_sb.tile([2, OWN], f32, tag="rmsq")
                    nc.scalar.activation(rmsq[:], ssqrow[0:2, :], AF.Sqrt,
                                         scale=1.0 / HD, bias=epsq[0:2, :])
                    invq = pc_sb.tile([2, OWN], f32, tag="invq")
                    nc.vector.reciprocal_approx_fast(invq[:], rmsq[:])
                    repq = pc_ps1.tile([128, OWN], f32, tag="repq")
                    nc.tensor.matmul(repq[:], selg[:, p, :], invq[:],
                                     start=True, stop=True)
                    t1 = pc_sb.tile([128, OWN], f32, tag="t1q")
                    nc.vector.tensor_tensor(t1[:], qraw[:], coso[:], ALU.mult)
                    t2 = pc_sb.tile([128, OWN], f32, tag="t2q")
                    nc.vector.tensor_tensor(t2[:], qsw[:], sino[:], ALU.mult)
                    t3 = pc_sb.tile([128, OWN], f32, tag="t3q")
                    nc.vector.tensor_tensor(t3[:], t1[:], t2[:], ALU.add)
                    nc.vector.tensor_tensor(qT[:, p, :], t3[:], repq[:], ALU.mult)

            rope_stack.close()

        # ------------- Phase D: attention ---------------------------------
        if ph_on("d"):
            xpr = big.tile([128, 8, OWN], bf16, tag="xpr")
            xpb = big.tile([128, 8, OWN], f32, tag="xpb")
            with tc.tile_pool(name="pd_m", bufs=1) as pd_m, \
                 tc.tile_pool(name="pd_pt", bufs=9) as pd_pt, \
                 tc.tile_pool(name="pd_sb", bufs=2) as pd_sb, \
                 tc.tile_pool(name="pd_s", bufs=2, space="PSUM") as pd_s, \
                 tc.tile_pool(name="pd_y", bufs=1, space="PSUM") as pd_y, \
                 tc.tile_pool(name="pd_r", bufs=1, space="PSUM") as pd_r:
                masks = pd_m.tile([128, 4, 4, 128], bf16, tag="masks")
                nc.sync.dma_start(masks[:], maskM[:])
                for t in range(4):
                    qsl = slice(t * 128, (t + 1) * 128)
                    n_chunks = t + 1
                    n_kvt = 4 * n_chunks
                    for half in range(2):
                        gA, gB = 2 * half, 2 * half + 1
                        yA = pd_y.tile([66, 4, 128], f32, tag="yA")
                        yB = pd_y.tile([66, 4, 128], f32, tag="yB")
                        qsA = qT[0:64, 4 * half:4 * half + 4, qsl]
                        qsB = qT[64:128, 4 * half:4 * half + 4, qsl]
                        for c in range(n_chunks):
                            pts = []
                            for i in range(4):
                                ks = slice((4 * c + i) * 128, (4 * c + i + 1) * 128)
                                psAB = pd_s.tile([128, 2, 4, 128], f32, tag="psAB")
                                nc.tensor.matmul(psAB[:, 0, :, :],
                                                 kT[0:64, half, ks], qsA,
                                                 start=True, stop=True,
                                                 tile_position=(0, 0))
                                nc.tensor.matmul(psAB[:, 1, :, :],
                                                 kT[64:128, half, ks], qsB,
                                                 start=True, stop=True,
                                                 tile_position=(64, 0))
                                ptAB = pd_pt.tile([128, 2, 4, 128], bf16, tag="ptAB")
                                nc.scalar.activation(ptAB[:], psAB[:], AF.Exp)
                                if c == t:
                                    mbc = masks[:, t, i, None, None, :].to_broadcast(
                                        [128, 2, 4, 128])
                                    eng = nc.vector if i % 2 == 0 else nc.gpsimd
                                    eng.tensor_tensor(ptAB[:], ptAB[:], mbc, ALU.mult)
                                pts.append(ptAB)
                            for i in range(4):
                                kvt = 4 * c + i
                                nc.tensor.matmul(yA[:], v_all[:, gA, kvt, :],
                                                 pts[i][:, 0, :, :], start=(kvt == 0),
                                                 stop=(kvt == n_kvt - 1))
                                nc.tensor.matmul(yB[:], v_all[:, gB, kvt, :],
                                                 pts[i][:, 1, :, :], start=(kvt == 0),
                                                 stop=(kvt == n_kvt - 1))
                        for g, y in ((gA, yA), (gB, yB)):
                            dsb = pd_sb.tile([2, 4, 128], f32, tag="dsb")
                            nc.any.tensor_copy(dsb[:], y[64:66, :, :])
                            invs = pd_sb.tile([2, 4, 128], f32, tag="invs")
                            nc.vector.reciprocal_approx_fast(invs[:], dsb[:])
                            ysb = pd_sb.tile([64, 4, 128], f32, tag="ysb")
                            nc.any.tensor_copy(ysb[:], y[0:64, :, :])
                            repy = pd_r.tile([64, 4, 128], f32, tag="repy")
                            nc.tensor.matmul(repy[:].rearrange("p a b -> p (a b)"),
                                             o10t[:, 0:64],
                                             invs[:].rearrange("p a b -> p (a b)"),
                                             start=True, stop=True)
                            for i in range(4):
                                h = 4 * g + i
                                chunk, part = h // 2, (h % 2) * 64
                                nc.vector.tensor_tensor(
                                    y_all[part:part + 64, chunk, qsl],
                                    ysb[:, i, :], repy[:, i, :], ALU.mult)

        # ------------- Phase E: Wo + post-norm + residual -----------------
        if ph_on("e"):
            with tc.tile_pool(name="pe_sb", bufs=2) as pe_sb, \
                 tc.tile_pool(name="pe_ao", bufs=1) as pe_ao, \
                 tc.tile_pool(name="pe_w", bufs=3) as pe_w, \
                 tc.tile_pool(name="pe_ps", bufs=2, space="PSUM") as pe_ps, \
                 tc.tile_pool(name="pe_ss", bufs=1, space="PSUM") as pe_ss:
                ao = pe_ao.tile([128, 8, OWN], f32, tag="ao")
                ssa = pe_ss.tile([2, OWN], f32, tag="ssa")
                for o in range(8):
                    wos = pe_w.tile([128, 8, 128], bf16, tag="wos")
                    nc.sync.dma_start(wos[:], wo[o])
                    aps = pe_ps.tile([128, OWN], f32, tag="aps")
                    for k in range(8):
                        nc.tensor.matmul(aps[:], wos[:, k, :], y_all[:, k, :],
                                         start=(k == 0), stop=(k == 7))
                    nc.any.tensor_copy(ao[:, o, :], aps[:])
                    a2 = pe_sb.tile([128, OWN], bf16, tag="a2")
                    nc.scalar.activation(a2[:], aps[:], AF.Square)
                    nc.tensor.matmul(ssa[:], ocb[:, 0:2], a2[:],
                                     start=(o == 0), stop=(o == 7))
                rmsa = pe_sb.tile([2, OWN], f32, tag="rmsa")
                nc.scalar.activation(rmsa[:], ssa[0:2, :], AF.Sqrt,
                                     scale=1.0 / D, bias=eps6[0:2, :])
                inva = pe_sb.tile([2, OWN], f32, tag="inva")
                nc.vector.reciprocal_approx_fast(inva[:], rmsa[:])
                repa = pe_ss.tile([128, OWN], f32, tag="repa")
                nc.tensor.matmul(repa[:], o10t[:], inva[:], start=True, stop=True)
                for o in range(8):
                    t1 = pe_sb.tile([128, OWN], f32, tag="t1e")
                    nc.vector.tensor_tensor(t1[:], ao[:, o, :], repa[:], ALU.mult)
                    nc.vector.scalar_tensor_tensor(
                        xpb[:, o, :], t1[:], gat[:, o, None], xrs[:, o, :],
                        ALU.mult, ALU.add)
                    nc.any.tensor_copy(xpr[:, o, :], xpb[:, o, :])
                    nc.vector.tensor_scalar_add(xpb[:, o, :], xpb[:, o, :],
                                                bml[:, o, None])

        # ------------- Phase F: MLP ---------------------------------------
        if ph_on("f"):
            mout = big.tile([128, 8, OWN], f32, tag="xrs_mout")
            with tc.tile_pool(name="pf_h2", bufs=1) as pf_h2, \
                 tc.tile_pool(name="pf_sb", bufs=2) as pf_sb, \
                 tc.tile_pool(name="pf_wf", bufs=3) as pf_wf, \
                 tc.tile_pool(name="pf_wp", bufs=3) as pf_wp, \
                 tc.tile_pool(name="pf_ps", bufs=2, space="PSUM") as pf_ps, \
                 tc.tile_pool(name="pf_mo", bufs=1, space="PSUM") as pf_mo:
                h2 = pf_h2.tile([128, 32, OWN], bf16, tag="h2")
                for hc in range(32):
                    wfs = pf_wf.tile([128, 8, 128], bf16, tag="wfs")
                    nc.sync.dma_start(wfs[:], wfc[hc])
                    hps = pf_ps.tile([128, OWN], f32, tag="hps")
                    for k in range(8):
                        nc.tensor.matmul(hps[:], wfs[:, k, :], xpr[:, k, :],
                                         start=(k == 0), stop=(k == 7))
                    hr = pf_sb.tile([128, OWN], bf16, tag="hr")
                    nc.scalar.activation(hr[:], hps[:], AF.Relu)
                    nc.vector.tensor_tensor(h2[:, hc, :], hr[:], hr[:], ALU.mult)
                ssm = pf_ps.tile([2, OWN], f32, tag="ssm")
                for ohalf in range(2):
                    mo_ps = [pf_mo.tile([128, OWN], f32, name=f"mo{oi}", tag=f"mo{oi}")
                             for oi in range(4)]
                    for hc in range(32):
                        wps = pf_wp.tile([128, 4, 128], bf16, tag="wps")
                        nc.sync.dma_start(wps[:], wprojq[hc, ohalf])
                        for oi in range(4):
                            nc.tensor.matmul(mo_ps[oi][:], wps[:, oi, :], h2[:, hc, :],
                                             start=(hc == 0), stop=(hc == 31))
                    for oi in range(4):
                        o = ohalf * 4 + oi
                        nc.any.tensor_copy(mout[:, o, :], mo_ps[oi][:])
                        m2 = pf_sb.tile([128, OWN], bf16, tag="m2")
                        nc.scalar.activation(m2[:], mo_ps[oi][:], AF.Square)
                        nc.tensor.matmul(ssm[:], ocb[:, 0:2], m2[:],
                                         start=(o == 0), stop=(o == 7))
                rmsm = pf_sb.tile([2, OWN], f32, tag="rmsm")
                nc.scalar.activation(rmsm[:], ssm[0:2, :], AF.Sqrt, scale=1.0 / D,
                                     bias=eps6[0:2, :])
                invm = pf_sb.tile([2, OWN], f32, tag="invm")
                nc.vector.reciprocal_approx_fast(invm[:], rmsm[:])
                repm = pf_ps.tile([128, OWN], f32, tag="hps")
                nc.tensor.matmul(repm[:], o10t[:], invm[:], start=True, stop=True)
                for o in range(8):
                    t1 = pf_sb.tile([128, OWN], f32, tag="t1f")
                    nc.vector.tensor_tensor(t1[:], mout[:, o, :], repm[:], ALU.mult)
                    outv = pf_sb.tile([128, OWN], f32, tag="outv")
                    nc.vector.scalar_tensor_tensor(
                        outv[:], t1[:], gml[:, o, None], xpb[:, o, :],
                        ALU.mult, ALU.add)
                    nc.sync.dma_start(out_t[:, o, :], outv[:])

        if not ph_on("f"):
            with tc.tile_pool(name="dummy", bufs=1) as dp:
                zout = dp.tile([128, 8, OWN], f32, tag="zout")
                nc.vector.memset(zout[:], 0.0)
                nc.sync.dma_start(out_t[:], zout[:])
            rope_stack.close()

    nc.finalize()
    return nc


def _feat_major(a):
    """[F, T] -> device layout [128, F//128, T]."""
    F, T = a.shape
    return np.ascontiguousarray(a.reshape(F // 128, 128, T).transpose(1, 0, 2))


def _vec_dev(v):
    return np.ascontiguousarray(v.reshape(-1, 128).T)


def _bf(a):
    return np.ascontiguousarray(a.astype(ml_dtypes.bfloat16))


_CACHE = {}
_RUN_KW = {}


def kernel(x, attn_norm_w, mlp_norm_w, attn_post_norm_w, mlp_post_norm_w,
           attn_scale, mlp_scale, attn_mod_gain, attn_mod_bias,
           mlp_mod_gain, mlp_mod_bias, Wq, Wk, Wv, Wo, q_gain, fc_w, proj_w):
    x = np.asarray(x, np.float32)
    q_gain = np.asarray(q_gain, np.float32)

    if "nc" not in _CACHE:
        _CACHE["nc"] = build(q_gain)
    nc = _CACHE["nc"]

    anw = np.asarray(attn_norm_w, np.float32)
    mnw = np.asarray(mlp_norm_w, np.float32)
    wq_eff = np.asarray(Wq, np.float32) * anw[None, :]
    wk_eff = np.asarray(Wk, np.float32) * anw[None, :]
    wv_eff = np.asarray(Wv, np.float32) * anw[None, :]
    fc_eff = np.asarray(fc_w, np.float32) * mnw[None, :]

    perm = np.zeros(D, np.int64)
    for p, (a, b) in enumerate(PAIRS):
        perm[p * 128:p * 128 + 64] = np.arange(a * 64, a * 64 + 64)
        perm[p * 128 + 64:(p + 1) * 128] = np.arange(b * 64, b * 64 + 64)
    WqTp = wq_eff.T[:, perm]                                  # [D_in, D_out-perm]
    wq_dev = _bf(np.stack([_feat_major(WqTp[:, p * 128:(p + 1) * 128]) for p in range(8)]))
    wk_dev = _bf(_feat_major(wk_eff.T))
    wv_dev = _bf(_feat_major(wv_eff.T))
    WoT = np.asarray(Wo, np.float32).T
    wo_dev = _bf(np.stack([_feat_major(WoT[:, o * 128:(o + 1) * 128]) for o in range(8)]))
    fcT = fc_eff.T
    wfc_dev = _bf(np.stack([_feat_major(fcT[:, h * 128:(h + 1) * 128]) for h in range(32)]))
    projT = np.asarray(proj_w, np.float32).T                  # [4096, 1024]
    wproj_dev = _bf(np.ascontiguousarray(
        projT.reshape(32, 128, 2, 4, 128).transpose(0, 2, 1, 3, 4)))

    inv_freq = 1.0 / (ROPE_BASE ** (np.arange(0, HD, 2, dtype=np.float32) / HD))
    tpos = np.arange(S, dtype=np.float32)
    freqs = np.outer(tpos, inv_freq).astype(np.float32)
    cosT = np.ascontiguousarray(np.tile(np.cos(freqs).T, (4, 1)))   # [128, S]
    sin1 = np.sin(freqs).T                                          # [32, S]
    sinS = np.ascontiguousarray(
        np.concatenate([sin1, -sin1, sin1, -sin1], axis=0))         # [128, S] sign-folded

    # swap permutation i <-> i^32 (within each 64-wide head)
    permM_h = np.zeros((128, 128), np.float32)
    for i in range(128):
        permM_h[i, i ^ 32] = 1.0

    oc_h_v = np.zeros((128, 2), np.float32)
    oc_h_v[0:64, 0] = 1.0
    oc_h_v[64:128, 1] = 1.0
    selg2_v = np.zeros((2, 8, 128), np.float32)
    for p, (a, b) in enumerate(PAIRS):
        selg2_v[0, p, 0:64] = q_gain[a] / 8.0
        selg2_v[1, p, 64:128] = q_gain[b] / 8.0
    ones10_v = np.concatenate([np.ones((1, 128), np.float32),
                               np.zeros((1, 128), np.float32)])

    gat_v = (np.asarray(attn_post_norm_w, np.float32)
             * np.asarray(attn_mod_gain, np.float32)
             * np.asarray(attn_scale, np.float32))
    bat_v = np.asarray(attn_mod_bias, np.float32) * np.asarray(attn_scale, np.float32)
    gml_v = (np.asarray(mlp_post_norm_w, np.float32)
             * np.asarray(mlp_mod_gain, np.float32)
             * np.asarray(mlp_scale, np.float32))
    bml_v = np.asarray(mlp_mod_bias, np.float32) * np.asarray(mlp_scale, np.float32)

    shared = {
        "wq": wq_dev, "wk": wk_dev, "wv": wv_dev, "wo": wo_dev,
        "wfc": wfc_dev, "wprojq": wproj_dev,
        "cosF": cosT, "sinF": sinS,
        "permM": permM_h,
        "oc_h": _bf(oc_h_v),
        "onescb": _bf(np.ones((128, 2), np.float32)),
        "selg2": selg2_v,
        "identM": _bf(np.eye(128, dtype=np.float32)),
        "selk": np.stack([np.concatenate([np.ones(64, np.float32), np.zeros(64, np.float32)]),
                          np.concatenate([np.zeros(64, np.float32), np.ones(64, np.float32)])]),
        "o10": ones10_v,
        "g_attn": _vec_dev(gat_v), "g_mlp": _vec_dev(gml_v),
        "b_mlp": _vec_dev(bml_v),
    }

    in_maps = []
    owners = []
    for c in range(8):
        b, j = c // 4, c % 4
        rows = np.concatenate(
            [np.arange((j + 4 * t) * 128, (j + 4 * t + 1) * 128) for t in range(4)])
        owners.append((b, rows))
        xb = x[b].T
        x_own = xb[:, rows]
        mask = np.zeros((4, 4, 128, 128), np.float32)
        for t in range(4):
            m = j + 4 * t
            q_idx = m * 128 + np.arange(128)
            for ktl in range(4):
                kv_idx = 512 * t + 128 * ktl + np.arange(128)
                mask[t, ktl] = (kv_idx[:, None] <= q_idx[None, :])
        m_in = {
            "xT": _bf(_feat_major(xb)),
            "xq": _bf(_feat_major(x_own)),
            "xres": _feat_major(x_own + bat_v[:, None]),
            "cosO": np.ascontiguousarray(cosT[:, rows]),
            "sinO": np.ascontiguousarray(sinS[:, rows]),
            "maskM": _bf(np.ascontiguousarray(mask.transpose(2, 0, 1, 3))),
        }
        m_in.update(shared)
        in_maps.append(m_in)

    res = run_bass_kernel_spmd(nc, in_maps, core_ids=list(range(8)),
                               **_RUN_KW)
    _CACHE["last_result"] = res

    out = np.empty((B, S, D), np.float32)
    for c in range(8):
        b, rows = owners[c]
        o = res.results[c]["out"]
        out[b, rows, :] = o.transpose(2, 1, 0).reshape(OWN, D)
    return out


# revision 49
# speedup vs baseline: 1.1311x; 1.1135x over previous
"""Trainium2 Bass kernel for one dense transformer block (B=2, S=2048, D=1024,
16 q-heads / 4 kv-heads GQA, squared-ReLU MLP), data-parallel over 8 NeuronCores.

Sharding: core c = (b, j), b = c // 4, j = c % 4, owns q-token tiles
{j, j+4, j+8, j+12} (128 tokens each) of batch b. K/V are computed for the full
sequence on every core (no collectives). The kv range for own q-tile t is
padded to 512*(t+1); causality enforced with per-core 0/1 masks on the last
512-wide kv chunk.

Numerical identities used (exact up to negligible eps rescaling):
  - per-head q/k rmsnorm is scale-invariant per token, so the block input
    rmsnorm cancels inside it -> Q/K project from raw (norm-weight-folded) x
  - the MLP input rmsnorm cancels through relu()^2 -> proj -> post-rmsnorm
  - V is projected from raw x and rescaled by 1/rms1(x) per token
  - no softmax max-subtraction (logits bounded by |q||k|/8 = 8)
  - softmax denominator = ones-column appended to V in the AV matmul
  - K's 1/rms is applied as a per-partition AP scale inside the exp
    activation (kv tokens are partitions in the score tiles); Q's 1/rms and
    q_gain/8 ride a replicate matmul onto qT

v2 perf changes vs baseline:
  - no DVE reciprocal with f32r destination (was ~7.7ns/elem); all recips are
    fp32->fp32 on DVE, replicates via small fp32 matmuls
  - rope via a feature-swap permutation matmul + 3 full-width DVE ops
    (was 12 narrow DVE ops)
  - bf16 weights + x + V/p/mask/y/h2 paths (half DMA, FWL weight loads,
    2x DVE); q/k/scores stay f32r
"""

import os

import numpy as np
import ml_dtypes

import concourse.bass as bass
from concourse import bacc
import concourse.tile as tile
import concourse.mybir as mybir
from concourse.bass_utils import run_bass_kernel_spmd

f32 = mybir.dt.float32
f32r = mybir.dt.float32r
bf16 = mybir.dt.bfloat16
AF = mybir.ActivationFunctionType
ALU = mybir.AluOpType

B, S, D = 2, 2048, 1024
H, HKV, HD = 16, 4, 64
MLP_HID = 4 * D
KV = HKV * HD
NT = 16
OWN = 512
EPS_BLOCK = 1e-6
EPS_QK = float(np.finfo(np.float32).eps)
ROPE_BASE = 10000.0

PAIRS = [(0, 4), (1, 5), (2, 6), (3, 7), (8, 12), (9, 13), (10, 14), (11, 15)]

PHASE_ORDER = ["ab", "c", "d", "e", "f"]


def build(q_gain):
    max_ph = os.environ.get("KERNEL_PHASES", "f")
    ph_on = lambda p: PHASE_ORDER.index(p) <= PHASE_ORDER.index(max_ph)
    bacc.Bacc.move_matmul_waits_to_ldweights = lambda self: None
    nc = bacc.Bacc(None)

    def dram_in(name, shape, dt):
        return nc.dram_tensor(name, list(shape), dt, kind="ExternalInput")

    xT = dram_in("xT", (128, 8, S), bf16)
    xq = dram_in("xq", (128, 8, OWN), bf16)
    xres = dram_in("xres", (128, 8, OWN), f32)
    wq = dram_in("wq", (8, 128, 8, 128), bf16)
    wk = dram_in("wk", (128, 8, KV), bf16)
    wv = dram_in("wv", (128, 8, KV), bf16)
    wo = dram_in("wo", (8, 128, 8, 128), bf16)
    wfc = dram_in("wfc", (32, 128, 8, 128), bf16)
    wprojq = dram_in("wprojq", (32, 2, 128, 4, 128), bf16)
    cosF = dram_in("cosF", (128, S), f32)
    sinF = dram_in("sinF", (128, S), f32)   # sign-folded: +sin rows 0-31/64-95, -sin rows 32-63/96-127
    cosO = dram_in("cosO", (128, OWN), f32)
    sinO = dram_in("sinO", (128, OWN), f32)
    maskM = dram_in("maskM", (128, 4, 4, 128), bf16)
    permM = dram_in("permM", (128, 128), f32r)     # swap rows i <-> i^32
    oc_h = dram_in("oc_h", (128, 2), bf16)         # col0: top-64 ones; col1: bottom-64 ones
    onescb = dram_in("onescb", (128, 2), bf16)     # all ones
    selg2 = dram_in("selg2", (2, 8, 128), f32)     # row0 -> cols 0-63 * gA/8, row1 -> cols 64-127 * gB/8
    identM = dram_in("identM", (128, 128), bf16)   # 128x128 identity
    selk = dram_in("selk", (2, 128), f32)          # row0 -> cols 0-63 ones, row1 -> cols 64-127 ones
    o10 = dram_in("o10", (2, 128), f32)            # row0 ones, row1 zeros
    g_attn = dram_in("g_attn", (128, 8), f32)
    g_mlp = dram_in("g_mlp", (128, 8), f32)
    b_mlp = dram_in("b_mlp", (128, 8), f32)

    out_t = nc.dram_tensor("out", [128, 8, OWN], f32, kind="ExternalOutput")

    with tile.TileContext(nc) as tc, \
         tc.tile_pool(name="cst", bufs=1) as cst, \
         tc.tile_pool(name="big", bufs=1) as big:
        och = cst.tile([128, 2], bf16, tag="och")
        nc.sync.dma_start(och[:], oc_h[:])
        ocb = cst.tile([128, 2], bf16, tag="ocb")
        nc.sync.dma_start(ocb[:], onescb[:])
        selg = cst.tile([2, 8, 128], f32, tag="selg")
        nc.sync.dma_start(selg[:], selg2[:])
        idt = cst.tile([128, 128], bf16, tag="idt")
        nc.sync.dma_start(idt[:], identM[:])
        selkt = cst.tile([2, 128], f32, tag="selkt")
        nc.sync.dma_start(selkt[:], selk[:])
        o10t = cst.tile([2, 128], f32, tag="o10t")
        nc.sync.dma_start(o10t[:], o10[:])
        perm = cst.tile([128, 128], f32r, tag="perm")
        nc.sync.dma_start(perm[:], permM[:])
        eps6 = cst.tile([128, 1], f32, tag="eps6")
        nc.vector.memset(eps6[:], EPS_BLOCK)
        epsq = cst.tile([128, 1], f32, tag="epsq")
        nc.vector.memset(epsq[:], EPS_QK)
        gat = cst.tile([128, 8], f32, tag="gat")
        nc.sync.dma_start(gat[:], g_attn[:])
        gml = cst.tile([128, 8], f32, tag="gml")
        nc.sync.dma_start(gml[:], g_mlp[:])
        bml = cst.tile([128, 8], f32, tag="bml")
        nc.sync.dma_start(bml[:], b_mlp[:])
        from contextlib import ExitStack
        rope_stack = ExitStack()
        ropep = rope_stack.enter_context(tc.tile_pool(name="ropep", bufs=1))
        cosf = ropep.tile([128, S], f32, tag="cosf")
        nc.sync.dma_start(cosf[:], cosF[:])
        sinf = ropep.tile([128, S], f32, tag="sinf")
        nc.sync.dma_start(sinf[:], sinF[:])
        coso = ropep.tile([128, OWN], f32, tag="coso")
        nc.sync.dma_start(coso[:], cosO[:])
        sino = ropep.tile([128, OWN], f32, tag="sino")
        nc.sync.dma_start(sino[:], sinO[:])

        kT = big.tile([128, 2, S], f32r, tag="kT")
        v_all = big.tile([128, 4, NT, 66], bf16, tag="v_all")
        qT = big.tile([128, 8, OWN], f32r, tag="qT")
        y_all = big.tile([128, 8, OWN], bf16, tag="y_all")
        xrs = big.tile([128, 8, OWN], f32, tag="xrs_mout")
        nc.sync.dma_start(xrs[:], xres[:])
        invr1 = big.tile([128, NT], f32, tag="invr1")
        rms_st = big.tile([128, NT], f32, tag="rms_st")

        # ------------- Phase AB: rms1, K, V over full sequence ------------
        absub = int(os.environ.get("KERNEL_AB_SUB", "99"))
        if ph_on("ab"):
            with tc.tile_pool(name="pab_x", bufs=3) as pab_x, \
                 tc.tile_pool(name="pab_sb", bufs=2) as pab_sb, \
                 tc.tile_pool(name="pab_w", bufs=1) as pab_w, \
                 tc.tile_pool(name="pab_ps", bufs=2, space="PSUM") as pab_ps, \
                 tc.tile_pool(name="pab_ps1", bufs=1, space="PSUM") as pab_ps1:
                wvs = pab_w.tile([128, 8, KV], bf16, tag="wvs")
                nc.sync.dma_start(wvs[:], wv[:])
                wks = pab_w.tile([128, 8, KV], bf16, tag="wks")
                nc.sync.dma_start(wks[:], wk[:])
                for ci in range(4):
                    sl = slice(ci * 512, (ci + 1) * 512)
                    xc = pab_x.tile([128, 8, 512], bf16, tag="xc")
                    nc.sync.dma_start(xc[:], xT[:, :, sl])
                    # token-major sumsq -> invr1 for the 4 token tiles of the chunk
                    for kt in range(4):
                        x2 = pab_sb.tile([128, 8, 128], bf16, tag="x2")
                        nc.scalar.activation(x2[:], xc[:, :, kt * 128:(kt + 1) * 128],
                                             AF.Square)
                        ssp = pab_ps1.tile([128, 2], f32, tag="sstk")
                        for k in range(8):
                            nc.tensor.matmul(ssp[:], x2[:, k, :], ocb[:, 0:2],
                                             start=(k == 0), stop=(k == 7))
                        nc.scalar.activation(rms_st[:, ci * 4 + kt, None], ssp[:, 0:1],
                                             AF.Sqrt, scale=1.0 / D, bias=eps6[:])
                    nc.vector.reciprocal(invr1[:, ci * 4:(ci + 1) * 4],
                                         rms_st[:, ci * 4:(ci + 1) * 4])
                    # V token-major for the 4 token tiles
                    for kt in range(4 if absub >= 2 else 0):
                        gkt = ci * 4 + kt
                        vps = pab_ps.tile([128, KV], f32, tag="vps")
                        for k in range(8):
                            nc.tensor.matmul(vps[:], xc[:, k, kt * 128:(kt + 1) * 128],
                                             wvs[:, k, :], start=(k == 0), stop=(k == 7))
                        nc.vector.tensor_scalar_mul(
                            v_all[:, :, gkt, 0:64],
                            vps[:].rearrange("p (g d) -> p g d", g=4),
                            invr1[:, gkt, None])
                    # K feature-major for both kv pairs
                    for kp in range(2 if absub >= 3 else 0):
                        kps = pab_ps.tile([128, 512], f32, tag="kps")
                        for k in range(8):
                            nc.tensor.matmul(kps[:], wks[:, k, kp * 128:(kp + 1) * 128],
                                             xc[:, k, :], start=(k == 0), stop=(k == 7))
                        kraw = pab_sb.tile([128, 512], f32r, tag="kraw")
                        nc.any.tensor_copy(kraw[:], kps[:])
                        ksw = pab_ps1.tile([128, 512], f32, tag="ksw")
                        nc.tensor.matmul(ksw[:], perm[:], kraw[:], start=True, stop=True)
                        if absub < 4:
                            continue
                        k2 = pab_sb.tile([128, 512], bf16, tag="k2")
                        nc.scalar.activation(k2[:], kps[:], AF.Square)
                        # per-token sumsq token-major, then transpose to row-major
                        sstk = pab_ps1.tile([128, 4, 2], f32, tag="sstk")
                        for kt in range(4):
                            ksl = slice(kt * 128, (kt + 1) * 128)
                            nc.tensor.matmul(sstk[:, kt, :], k2[:, ksl], och[:, 0:2],
                                             start=True, stop=True)
                        sstk_sb = pab_sb.tile([128, 4, 2], bf16, tag="sstk_sb")
                        nc.any.tensor_copy(sstk_sb[:], sstk[:])
                        ssrow = pab_ps1.tile([2, 512], f32, tag="ssrow")
                        for kt in range(4):
                            ksl = slice(kt * 128, (kt + 1) * 128)
                            nc.tensor.matmul(ssrow[0:2, ksl], sstk_sb[:, kt, :],
                                             idt[:], start=True, stop=True)
                        rmsk = pab_sb.tile([2, 512], f32, tag="rmsk")
                        nc.scalar.activation(rmsk[:], ssrow[0:2, :], AF.Sqrt,
                                             scale=1.0 / HD, bias=epsq[0:2, :])
                        invk = pab_sb.tile([2, 512], f32, tag="invk")
                        nc.vector.reciprocal_approx_fast(invk[:], rmsk[:])
                        repk = pab_ps1.tile([128, 512], f32, tag="repk")
                        nc.tensor.matmul(repk[:], selkt[:], invk[:],
                                         start=True, stop=True)
                        if absub < 5:
                            continue
                        t1 = pab_sb.tile([128, 512], f32, tag="t1k")
                        nc.vector.tensor_tensor(t1[:], kraw[:], cosf[:, sl], ALU.mult)
                        t2 = pab_sb.tile([128, 512], f32, tag="t2k")
                        nc.vector.tensor_tensor(t2[:], ksw[:], sinf[:, sl], ALU.mult)
                        t3 = pab_sb.tile([128, 512], f32, tag="t3k")
                        nc.vector.tensor_tensor(t3[:], t1[:], t2[:], ALU.add)
                        nc.vector.tensor_tensor(kT[:, kp, sl], t3[:], repk[:], ALU.mult)
                # ones column of V
                nc.vector.tensor_copy(
                    v_all[:, :, :, 64:66],
                    ocb[:, 0, None, None].to_broadcast([128, 4, NT, 2]))

        # ------------- Phase C: Q for own tokens --------------------------
        if ph_on("c"):
            with tc.tile_pool(name="pc_x", bufs=1) as pc_x, \
                 tc.tile_pool(name="pc_sb", bufs=3) as pc_sb, \
                 tc.tile_pool(name="pc_w", bufs=3) as pc_w, \
                 tc.tile_pool(name="pc_ps", bufs=2, space="PSUM") as pc_ps, \
                 tc.tile_pool(name="pc_ps1", bufs=1, space="PSUM") as pc_ps1:
                xqs = pc_x.tile([128, 8, OWN], bf16, tag="xqs")
                nc.sync.dma_start(xqs[:], xq[:])
                for p in range(8):
                    wqs = pc_w.tile([128, 8, 128], bf16, tag="wqs")
                    nc.sync.dma_start(wqs[:], wq[p])
                    qps = pc_ps.tile([128, OWN], f32, tag="qps")
                    for k in range(8):
                        nc.tensor.matmul(qps[:], wqs[:, k, :], xqs[:, k, :],
                                         start=(k == 0), stop=(k == 7))
                    qraw = pc_sb.tile([128, OWN], f32r, tag="qraw")
                    nc.any.tensor_copy(qraw[:], qps[:])
                    qsw = pc_ps.tile([128, OWN], f32, tag="qsw")
                    nc.tensor.matmul(qsw[:], perm[:], qraw[:], start=True, stop=True)
                    q2 = pc_sb.tile([128, OWN], bf16, tag="q2")
                    nc.scalar.activation(q2[:], qps[:], AF.Square)
                    sstq = pc_ps1.tile([128, 4, 2], f32, tag="sstq")
                    for kt in range(4):
                        ksl = slice(kt * 128, (kt + 1) * 128)
                        nc.tensor.matmul(sstq[:, kt, :], q2[:, ksl], och[:, 0:2],
                                         start=True, stop=True)
                    sstq_sb = pc_sb.tile([128, 4, 2], bf16, tag="sstq_sb")
                    nc.any.tensor_copy(sstq_sb[:], sstq[:])
                    ssqrow = pc_ps1.tile([2, OWN], f32, tag="ssqrow")
                    for kt in range(4):
                        ksl = slice(kt * 128, (kt + 1) * 128)
                        nc.tensor.matmul(ssqrow[0:2, ksl], sstq_sb[:, kt, :],
                                         idt[:], start=True, stop=True)
                    rmsq = pc_sb.tile([2, OWN], f32, tag="rmsq")
                    nc.scalar.activation(rmsq[:], ssqrow[0:2, :], AF.Sqrt,
                                         scale=1.0 / HD, bias=epsq[0:2, :])
                    invq = pc_sb.tile([2, OWN], f32, tag="invq")
                    nc.vector.reciprocal_approx_fast(invq[:], rmsq[:])
                    repq = pc_ps1.tile([128, OWN], f32, tag="repq")
                    nc.tensor.matmul(repq[:], selg[:, p, :], invq[:],
                                     start=True, stop=True)
                    t1 = pc_sb.tile([128, OWN], f32, tag="t1q")
                    nc.vector.tensor_tensor(t1[:], qraw[:], coso[:], ALU.mult)
                    t2 = pc_sb.tile([128, OWN], f32, tag="t2q")
                    nc.vector.tensor_tensor(t2[:], qsw[:], sino[:], ALU.mult)
                    t3 = pc_sb.tile([128, OWN], f32, tag="t3q")
                    nc.vector.tensor_tensor(t3[:], t1[:], t2[:], ALU.add)
                    nc.vector.tensor_tensor(qT[:, p, :], t3[:], repq[:], ALU.mult)

            rope_stack.close()

        # ------------- Phase D: attention ---------------------------------
        if ph_on("d"):
            xpr = big.tile([128, 8, OWN], bf16, tag="xpr")
            xpb = big.tile([128, 8, OWN], f32, tag="xpb")
            with tc.tile_pool(name="pd_m", bufs=1) as pd_m, \
                 tc.tile_pool(name="pd_pt", bufs=6) as pd_pt, \
                 tc.tile_pool(name="pd_sb", bufs=2) as pd_sb, \
                 tc.tile_pool(name="pd_s", bufs=2, space="PSUM") as pd_s, \
                 tc.tile_pool(name="pd_y", bufs=1, space="PSUM") as pd_y, \
                 tc.tile_pool(name="pd_r", bufs=1, space="PSUM") as pd_r:
                masks = pd_m.tile([128, 4, 4, 128], bf16, tag="masks")
                nc.sync.dma_start(masks[:], maskM[:])
                for t in range(4):
                    qsl = slice(t * 128, (t + 1) * 128)
                    n_chunks = t + 1
                    n_kvt = 4 * n_chunks
                    for half in range(2):
                        gA, gB = 2 * half, 2 * half + 1
                        yA = pd_y.tile([66, 4, 128], f32, tag="yA")
                        yB = pd_y.tile([66, 4, 128], f32, tag="yB")
                        qsA = qT[0:64, 4 * half:4 * half + 4, qsl]
                        qsB = qT[64:128, 4 * half:4 * half + 4, qsl]
                        for c in range(n_chunks):
                            pts = []
                            for i in range(4):
                                ks = slice((4 * c + i) * 128, (4 * c + i + 1) * 128)
                                psAB = pd_s.tile([128, 2, 4, 128], f32, tag="psAB")
                                nc.tensor.matmul(psAB[:, 0, :, :],
                                                 kT[0:64, half, ks], qsA,
                                                 start=True, stop=True,
                                                 tile_position=(0, 0))
                                nc.tensor.matmul(psAB[:, 1, :, :],
                                                 kT[64:128, half, ks], qsB,
                                                 start=True, stop=True,
                                                 tile_position=(64, 0))
                                ptAB = pd_pt.tile([128, 2, 4, 128], bf16, tag="ptAB")
                                nc.scalar.activation(ptAB[:], psAB[:], AF.Exp)
                                if c == t:
                                    mbc = masks[:, t, i, None, None, :].to_broadcast(
                                        [128, 2, 4, 128])
                                    eng = nc.vector if i % 2 == 0 else nc.gpsimd
                                    eng.tensor_tensor(ptAB[:], ptAB[:], mbc, ALU.mult)
                                pts.append(ptAB)
                            for i in range(4):
                                kvt = 4 * c + i
                                nc.tensor.matmul(yA[:], v_all[:, gA, kvt, :],
                                                 pts[i][:, 0, :, :], start=(kvt == 0),
                                                 stop=(kvt == n_kvt - 1))
                                nc.tensor.matmul(yB[:], v_all[:, gB, kvt, :],
                                                 pts[i][:, 1, :, :], start=(kvt == 0),
                                                 stop=(kvt == n_kvt - 1))
                        for g, y in ((gA, yA), (gB, yB)):
                            dsb = pd_sb.tile([2, 4, 128], f32, tag="dsb")
                            nc.vector.tensor_copy(dsb[:], y[64:66, :, :])
                            invs = pd_sb.tile([2, 4, 128], f32, tag="invs")
                            nc.vector.reciprocal_approx_fast(invs[:], dsb[:])
                            ysb = pd_sb.tile([64, 4, 128], f32, tag="ysb")
                            nc.vector.tensor_copy(ysb[:], y[0:64, :, :])
                            repy = pd_r.tile([64, 4, 128], f32, tag="repy")
                            nc.tensor.matmul(repy[:].rearrange("p a b -> p (a b)"),
                                             o10t[:, 0:64],
                                             invs[:].rearrange("p a b -> p (a b)"),
                                             start=True, stop=True)
                            for i in range(4):
                                h = 4 * g + i
                                chunk, part = h // 2, (h % 2) * 64
                                nc.vector.tensor_tensor(
                                    y_all[part:part + 64, chunk, qsl],
                                    ysb[:, i, :], repy[:, i, :], ALU.mult)

        # ------------- Phase E: Wo + post-norm + residual -----------------
        if ph_on("e"):
            with tc.tile_pool(name="pe_sb", bufs=2) as pe_sb, \
                 tc.tile_pool(name="pe_ao", bufs=1) as pe_ao, \
                 tc.tile_pool(name="pe_w", bufs=3) as pe_w, \
                 tc.tile_pool(name="pe_ps", bufs=2, space="PSUM") as pe_ps, \
                 tc.tile_pool(name="pe_ss", bufs=1, space="PSUM") as pe_ss:
                ao = pe_ao.tile([128, 8, OWN], f32, tag="ao")
                ssa = pe_ss.tile([2, OWN], f32, tag="ssa")
                for o in range(8):
                    wos = pe_w.tile([128, 8, 128], bf16, tag="wos")
                    nc.sync.dma_start(wos[:], wo[o])
                    aps = pe_ps.tile([128, OWN], f32, tag="aps")
                    for k in range(8):
                        nc.tensor.matmul(aps[:], wos[:, k, :], y_all[:, k, :],
                                         start=(k == 0), stop=(k == 7))
                    nc.any.tensor_copy(ao[:, o, :], aps[:])
                    a2 = pe_sb.tile([128, OWN], bf16, tag="a2")
                    nc.scalar.activation(a2[:], aps[:], AF.Square)
                    nc.tensor.matmul(ssa[:], ocb[:, 0:2], a2[:],
                                     start=(o == 0), stop=(o == 7))
                rmsa = pe_sb.tile([2, OWN], f32, tag="rmsa")
                nc.scalar.activation(rmsa[:], ssa[0:2, :], AF.Sqrt,
                                     scale=1.0 / D, bias=eps6[0:2, :])
                inva = pe_sb.tile([2, OWN], f32, tag="inva")
                nc.vector.reciprocal_approx_fast(inva[:], rmsa[:])
                repa = pe_ss.tile([128, OWN], f32, tag="repa")
                nc.tensor.matmul(repa[:], o10t[:], inva[:], start=True, stop=True)
                for o in range(8):
                    t1 = pe_sb.tile([128, OWN], f32, tag="t1e")
                    nc.vector.tensor_tensor(t1[:], ao[:, o, :], repa[:], ALU.mult)
                    nc.vector.scalar_tensor_tensor(
                        xpb[:, o, :], t1[:], gat[:, o, None], xrs[:, o, :],
                        ALU.mult, ALU.add)
                    nc.any.tensor_copy(xpr[:, o, :], xpb[:, o, :])
                    nc.vector.tensor_scalar_add(xpb[:, o, :], xpb[:, o, :],
                                                bml[:, o, None])

        # ------------- Phase F: MLP ---------------------------------------
        if ph_on("f"):
            mout = big.tile([128, 8, OWN], f32, tag="xrs_mout")
            with tc.tile_pool(name="pf_h2", bufs=1) as pf_h2, \
                 tc.tile_pool(name="pf_sb", bufs=2) as pf_sb, \
                 tc.tile_pool(name="pf_wf", bufs=3) as pf_wf, \
                 tc.tile_pool(name="pf_wp", bufs=3) as pf_wp, \
                 tc.tile_pool(name="pf_ps", bufs=2, space="PSUM") as pf_ps, \
                 tc.tile_pool(name="pf_mo", bufs=1, space="PSUM") as pf_mo:
                h2 = pf_h2.tile([128, 32, OWN], bf16, tag="h2")
                for hc in range(32):
                    wfs = pf_wf.tile([128, 8, 128], bf16, tag="wfs")
                    nc.sync.dma_start(wfs[:], wfc[hc])
                    hps = pf_ps.tile([128, OWN], f32, tag="hps")
                    for k in range(8):
                        nc.tensor.matmul(hps[:], wfs[:, k, :], xpr[:, k, :],
                                         start=(k == 0), stop=(k == 7))
                    hr = pf_sb.tile([128, OWN], bf16, tag="hr")
                    nc.scalar.activation(hr[:], hps[:], AF.Relu)
                    nc.vector.tensor_tensor(h2[:, hc, :], hr[:], hr[:], ALU.mult)
                ssm = pf_ps.tile([2, OWN], f32, tag="ssm")
                for ohalf in range(2):
                    mo_ps = [pf_mo.tile([128, OWN], f32, name=f"mo{oi}", tag=f"mo{oi}")
                             for oi in range(4)]
                    for hc in range(32):
                        wps = pf_wp.tile([128, 4, 128], bf16, tag="wps")
                        nc.sync.dma_start(wps[:], wprojq[hc, ohalf])
                        for oi in range(4):
                            nc.tensor.matmul(mo_ps[oi][:], wps[:, oi, :], h2[:, hc, :],
                                             start=(hc == 0), stop=(hc == 31))
                    for oi in range(4):
                        o = ohalf * 4 + oi
                        nc.any.tensor_copy(mout[:, o, :], mo_ps[oi][:])
                        m2 = pf_sb.tile([128, OWN], bf16, tag="m2")
                        nc.scalar.activation(m2[:], mo_ps[oi][:], AF.Square)
                        nc.tensor.matmul(ssm[:], ocb[:, 0:2], m2[:],
                                         start=(o == 0), stop=(o == 7))
                rmsm = pf_sb.tile([2, OWN], f32, tag="rmsm")
                nc.scalar.activation(rmsm[:], ssm[0:2, :], AF.Sqrt, scale=1.0 / D,
                                     bias=eps6[0:2, :])
                invm = pf_sb.tile([2, OWN], f32, tag="invm")
                nc.vector.reciprocal_approx_fast(invm[:], rmsm[:])
                repm = pf_ps.tile([128, OWN], f32, tag="hps")
                nc.tensor.matmul(repm[:], o10t[:], invm[:], start=True, stop=True)
                for o in range(8):
                    t1 = pf_sb.tile([128, OWN], f32, tag="t1f")
                    nc.vector.tensor_tensor(t1[:], mout[:, o, :], repm[:], ALU.mult)
                    outv = pf_sb.tile([128, OWN], f32, tag="outv")
                    nc.vector.scalar_tensor_tensor(
                        outv[:], t1[:], gml[:, o, None], xpb[:, o, :],
                        ALU.mult, ALU.add)
                    nc.sync.dma_start(out_t[:, o, :], outv[:])

        if not ph_on("f"):
            with tc.tile_pool(name="dummy", bufs=1) as dp:
                zout = dp.tile([128, 8, OWN], f32, tag="zout")
                nc.vector.memset(zout[:], 0.0)
                nc.sync.dma_start(out_t[:], zout[:])
            rope_stack.close()

    nc.finalize()
    return nc


def _feat_major(a):
    """[F, T] -> device layout [128, F//128, T]."""
    F, T = a.shape
    return np.ascontiguousarray(a.reshape(F // 128, 128, T).transpose(1, 0, 2))


def _vec_dev(v):
    return np.ascontiguousarray(v.reshape(-1, 128).T)


def _bf(a):
    return np.ascontiguousarray(a.astype(ml_dtypes.bfloat16))


_CACHE = {}
_RUN_KW = {}


def kernel(x, attn_norm_w, mlp_norm_w, attn_post_norm_w, mlp_post_norm_w,
           attn_scale, mlp_scale, attn_mod_gain, attn_mod_bias,
           mlp_mod_gain, mlp_mod_bias, Wq, Wk, Wv, Wo, q_gain, fc_w, proj_w):
    x = np.asarray(x, np.float32)
    q_gain = np.asarray(q_gain, np.float32)

    if "nc" not in _CACHE:
        _CACHE["nc"] = build(q_gain)
    nc = _CACHE["nc"]

    anw = np.asarray(attn_norm_w, np.float32)
    mnw = np.asarray(mlp_norm_w, np.float32)
    wq_eff = np.asarray(Wq, np.float32) * anw[None, :]
    wk_eff = np.asarray(Wk, np.float32) * anw[None, :]
    wv_eff = np.asarray(Wv, np.float32) * anw[None, :]
    fc_eff = np.asarray(fc_w, np.float32) * mnw[None, :]

    perm = np.zeros(D, np.int64)
    for p, (a, b) in enumerate(PAIRS):
        perm[p * 128:p * 128 + 64] = np.arange(a * 64, a * 64 + 64)
        perm[p * 128 + 64:(p + 1) * 128] = np.arange(b * 64, b * 64 + 64)
    WqTp = wq_eff.T[:, perm]                                  # [D_in, D_out-perm]
    wq_dev = _bf(np.stack([_feat_major(WqTp[:, p * 128:(p + 1) * 128]) for p in range(8)]))
    wk_dev = _bf(_feat_major(wk_eff.T))
    wv_dev = _bf(_feat_major(wv_eff.T))
    WoT = np.asarray(Wo, np.float32).T
    wo_dev = _bf(np.stack([_feat_major(WoT[:, o * 128:(o + 1) * 128]) for o in range(8)]))
    fcT = fc_eff.T
    wfc_dev = _bf(np.stack([_feat_major(fcT[:, h * 128:(h + 1) * 128]) for h in range(32)]))
    projT = np.asarray(proj_w, np.float32).T                  # [4096, 1024]
    wproj_dev = _bf(np.ascontiguousarray(
        projT.reshape(32, 128, 2, 4, 128).transpose(0, 2, 1, 3, 4)))

    inv_freq = 1.0 / (ROPE_BASE ** (np.arange(0, HD, 2, dtype=np.float32) / HD))
    tpos = np.arange(S, dtype=np.float32)
    freqs = np.outer(tpos, inv_freq).astype(np.float32)
    cosT = np.ascontiguousarray(np.tile(np.cos(freqs).T, (4, 1)))   # [128, S]
    sin1 = np.sin(freqs).T                                          # [32, S]
    sinS = np.ascontiguousarray(
        np.concatenate([sin1, -sin1, sin1, -sin1], axis=0))         # [128, S] sign-folded

    # swap permutation i <-> i^32 (within each 64-wide head)
    permM_h = np.zeros((128, 128), np.float32)
    for i in range(128):
        permM_h[i, i ^ 32] = 1.0

    oc_h_v = np.zeros((128, 2), np.float32)
    oc_h_v[0:64, 0] = 1.0
    oc_h_v[64:128, 1] = 1.0
    selg2_v = np.zeros((2, 8, 128), np.float32)
    for p, (a, b) in enumerate(PAIRS):
        selg2_v[0, p, 0:64] = q_gain[a] / 8.0
        selg2_v[1, p, 64:128] = q_gain[b] / 8.0
    ones10_v = np.concatenate([np.ones((1, 128), np.float32),
                               np.zeros((1, 128), np.float32)])

    gat_v = (np.asarray(attn_post_norm_w, np.float32)
             * np.asarray(attn_mod_gain, np.float32)
             * np.asarray(attn_scale, np.float32))
    bat_v = np.asarray(attn_mod_bias, np.float32) * np.asarray(attn_scale, np.float32)
    gml_v = (np.asarray(mlp_post_norm_w, np.float32)
             * np.asarray(mlp_mod_gain, np.float32)
             * np.asarray(mlp_scale, np.float32))
    bml_v = np.asarray(mlp_mod_bias, np.float32) * np.asarray(mlp_scale, np.float32)

    shared = {
        "wq": wq_dev, "wk": wk_dev, "wv": wv_dev, "wo": wo_dev,
        "wfc": wfc_dev, "wprojq": wproj_dev,
        "cosF": cosT, "sinF": sinS,
        "permM": permM_h,
        "oc_h": _bf(oc_h_v),
        "onescb": _bf(np.ones((128, 2), np.float32)),
        "selg2": selg2_v,
        "identM": _bf(np.eye(128, dtype=np.float32)),
        "selk": np.stack([np.concatenate([np.ones(64, np.float32), np.zeros(64, np.float32)]),
                          np.concatenate([np.zeros(64, np.float32), np.ones(64, np.float32)])]),
        "o10": ones10_v,
        "g_attn": _vec_dev(gat_v), "g_mlp": _vec_dev(gml_v),
        "b_mlp": _vec_dev(bml_v),
    }

    in_maps = []
    owners = []
    for c in range(8):
        b, j = c // 4, c % 4
        rows = np.concatenate(
            [np.arange((j + 4 * t) * 128, (j + 4 * t + 1) * 128) for t in range(4)])
        owners.append((b, rows))
        xb = x[b].T
        x_own = xb[:, rows]
        mask = np.zeros((4, 4, 128, 128), np.float32)
        for t in range(4):
            m = j + 4 * t
            q_idx = m * 128 + np.arange(128)
            for ktl in range(4):
                kv_idx = 512 * t + 128 * ktl + np.arange(128)
                mask[t, ktl] = (kv_idx[:, None] <= q_idx[None, :])
        m_in = {
            "xT": _bf(_feat_major(xb)),
            "xq": _bf(_feat_major(x_own)),
            "xres": _feat_major(x_own + bat_v[:, None]),
            "cosO": np.ascontiguousarray(cosT[:, rows]),
            "sinO": np.ascontiguousarray(sinS[:, rows]),
            "maskM": _bf(np.ascontiguousarray(mask.transpose(2, 0, 1, 3))),
        }
        m_in.update(shared)
        in_maps.append(m_in)

    res = run_bass_kernel_spmd(nc, in_maps, core_ids=list(range(8)),
                               **_RUN_KW)
    _CACHE["last_result"] = res

    out = np.empty((B, S, D), np.float32)
    for c in range(8):
        b, rows = owners[c]
        o = res.results[c]["out"]
        out[b, rows, :] = o.transpose(2, 1, 0).reshape(OWN, D)
    return out


# revision 50
# speedup vs baseline: 1.1335x; 1.0022x over previous
"""Trainium2 Bass kernel for one dense transformer block (B=2, S=2048, D=1024,
16 q-heads / 4 kv-heads GQA, squared-ReLU MLP), data-parallel over 8 NeuronCores.

Sharding: core c = (b, j), b = c // 4, j = c % 4, owns q-token tiles
{j, j+4, j+8, j+12} (128 tokens each) of batch b. K/V are computed for the full
sequence on every core (no collectives). The kv range for own q-tile t is
padded to 512*(t+1); causality enforced with per-core 0/1 masks on the last
512-wide kv chunk.

Numerical identities used (exact up to negligible eps rescaling):
  - per-head q/k rmsnorm is scale-invariant per token, so the block input
    rmsnorm cancels inside it -> Q/K project from raw (norm-weight-folded) x
  - the MLP input rmsnorm cancels through relu()^2 -> proj -> post-rmsnorm
  - V is projected from raw x and rescaled by 1/rms1(x) per token
  - no softmax max-subtraction (logits bounded by |q||k|/8 = 8)
  - softmax denominator = ones-column appended to V in the AV matmul
  - K's 1/rms is applied as a per-partition AP scale inside the exp
    activation (kv tokens are partitions in the score tiles); Q's 1/rms and
    q_gain/8 ride a replicate matmul onto qT

v2 perf changes vs baseline:
  - no DVE reciprocal with f32r destination (was ~7.7ns/elem); all recips are
    fp32->fp32 on DVE, replicates via small fp32 matmuls
  - rope via a feature-swap permutation matmul + 3 full-width DVE ops
    (was 12 narrow DVE ops)
  - bf16 weights + x + V/p/mask/y/h2 paths (half DMA, FWL weight loads,
    2x DVE); q/k/scores stay f32r
"""

import os

import numpy as np
import ml_dtypes

import concourse.bass as bass
from concourse import bacc
import concourse.tile as tile
import concourse.mybir as mybir
from concourse.bass_utils import run_bass_kernel_spmd

f32 = mybir.dt.float32
f32r = mybir.dt.float32r
bf16 = mybir.dt.bfloat16
AF = mybir.ActivationFunctionType
ALU = mybir.AluOpType

B, S, D = 2, 2048, 1024
H, HKV, HD = 16, 4, 64
MLP_HID = 4 * D
KV = HKV * HD
NT = 16
OWN = 512
EPS_BLOCK = 1e-6
EPS_QK = float(np.finfo(np.float32).eps)
ROPE_BASE = 10000.0

PAIRS = [(0, 4), (1, 5), (2, 6), (3, 7), (8, 12), (9, 13), (10, 14), (11, 15)]

PHASE_ORDER = ["ab", "c", "d", "e", "f"]


def build(q_gain):
    max_ph = os.environ.get("KERNEL_PHASES", "f")
    ph_on = lambda p: PHASE_ORDER.index(p) <= PHASE_ORDER.index(max_ph)
    bacc.Bacc.move_matmul_waits_to_ldweights = lambda self: None
    nc = bacc.Bacc(None)

    def dram_in(name, shape, dt):
        return nc.dram_tensor(name, list(shape), dt, kind="ExternalInput")

    xT = dram_in("xT", (128, 8, S), bf16)
    xq = dram_in("xq", (128, 8, OWN), bf16)
    xres = dram_in("xres", (128, 8, OWN), f32)
    wq = dram_in("wq", (8, 128, 8, 128), bf16)
    wk = dram_in("wk", (128, 8, KV), bf16)
    wv = dram_in("wv", (128, 8, KV), bf16)
    wo = dram_in("wo", (8, 128, 8, 128), bf16)
    wfc = dram_in("wfc", (32, 128, 8, 128), bf16)
    wprojq = dram_in("wprojq", (32, 2, 128, 4, 128), bf16)
    cosF = dram_in("cosF", (128, S), f32)
    sinF = dram_in("sinF", (128, S), f32)   # sign-folded: +sin rows 0-31/64-95, -sin rows 32-63/96-127
    cosO = dram_in("cosO", (128, OWN), f32)
    sinO = dram_in("sinO", (128, OWN), f32)
    maskM = dram_in("maskM", (128, 4, 4, 128), bf16)
    permM = dram_in("permM", (128, 128), f32r)     # swap rows i <-> i^32
    oc_h = dram_in("oc_h", (128, 2), bf16)         # col0: top-64 ones; col1: bottom-64 ones
    onescb = dram_in("onescb", (128, 2), bf16)     # all ones
    selg2 = dram_in("selg2", (2, 8, 128), f32)     # row0 -> cols 0-63 * gA/8, row1 -> cols 64-127 * gB/8
    identM = dram_in("identM", (128, 128), bf16)   # 128x128 identity
    selk = dram_in("selk", (2, 128), f32)          # row0 -> cols 0-63 ones, row1 -> cols 64-127 ones
    o10 = dram_in("o10", (2, 128), f32)            # row0 ones, row1 zeros
    g_attn = dram_in("g_attn", (128, 8), f32)
    g_mlp = dram_in("g_mlp", (128, 8), f32)
    b_mlp = dram_in("b_mlp", (128, 8), f32)

    out_t = nc.dram_tensor("out", [128, 8, OWN], f32, kind="ExternalOutput")

    with tile.TileContext(nc) as tc, \
         tc.tile_pool(name="cst", bufs=1) as cst, \
         tc.tile_pool(name="big", bufs=1) as big:
        och = cst.tile([128, 2], bf16, tag="och")
        nc.sync.dma_start(och[:], oc_h[:])
        ocb = cst.tile([128, 2], bf16, tag="ocb")
        nc.sync.dma_start(ocb[:], onescb[:])
        selg = cst.tile([2, 8, 128], f32, tag="selg")
        nc.sync.dma_start(selg[:], selg2[:])
        idt = cst.tile([128, 128], bf16, tag="idt")
        nc.sync.dma_start(idt[:], identM[:])
        selkt = cst.tile([2, 128], f32, tag="selkt")
        nc.sync.dma_start(selkt[:], selk[:])
        o10t = cst.tile([2, 128], f32, tag="o10t")
        nc.sync.dma_start(o10t[:], o10[:])
        perm = cst.tile([128, 128], f32r, tag="perm")
        nc.sync.dma_start(perm[:], permM[:])
        eps6 = cst.tile([128, 1], f32, tag="eps6")
        nc.vector.memset(eps6[:], EPS_BLOCK)
        epsq = cst.tile([128, 1], f32, tag="epsq")
        nc.vector.memset(epsq[:], EPS_QK)
        gat = cst.tile([128, 8], f32, tag="gat")
        nc.sync.dma_start(gat[:], g_attn[:])
        gml = cst.tile([128, 8], f32, tag="gml")
        nc.sync.dma_start(gml[:], g_mlp[:])
        bml = cst.tile([128, 8], f32, tag="bml")
        nc.sync.dma_start(bml[:], b_mlp[:])
        from contextlib import ExitStack
        rope_stack = ExitStack()
        ropep = rope_stack.enter_context(tc.tile_pool(name="ropep", bufs=1))
        cosf = ropep.tile([128, S], f32, tag="cosf")
        sinf = ropep.tile([128, S], f32, tag="sinf")
        coso = ropep.tile([128, OWN], f32, tag="coso")
        sino = ropep.tile([128, OWN], f32, tag="sino")

        kT = big.tile([128, 2, S], f32r, tag="kT")
        v_all = big.tile([128, 4, NT, 66], bf16, tag="v_all")
        qT = big.tile([128, 8, OWN], f32r, tag="qT")
        y_all = big.tile([128, 8, OWN], bf16, tag="y_all")
        xrs = big.tile([128, 8, OWN], f32, tag="xrs_mout")
        invr1 = big.tile([128, NT], f32, tag="invr1")
        rms_st = big.tile([128, NT], f32, tag="rms_st")

        # ------------- Phase AB: rms1, K, V over full sequence ------------
        absub = int(os.environ.get("KERNEL_AB_SUB", "99"))
        if ph_on("ab"):
            with tc.tile_pool(name="pab_x", bufs=3) as pab_x, \
                 tc.tile_pool(name="pab_sb", bufs=2) as pab_sb, \
                 tc.tile_pool(name="pab_w", bufs=1) as pab_w, \
                 tc.tile_pool(name="pab_ps", bufs=2, space="PSUM") as pab_ps, \
                 tc.tile_pool(name="pab_ps1", bufs=1, space="PSUM") as pab_ps1:
                wvs = pab_w.tile([128, 8, KV], bf16, tag="wvs")
                nc.sync.dma_start(wvs[:], wv[:])
                wks = pab_w.tile([128, 8, KV], bf16, tag="wks")
                nc.sync.dma_start(wks[:], wk[:])
                nc.sync.dma_start(cosf[:], cosF[:])
                nc.sync.dma_start(sinf[:], sinF[:])
                nc.sync.dma_start(coso[:], cosO[:])
                nc.sync.dma_start(sino[:], sinO[:])
                nc.sync.dma_start(xrs[:], xres[:])
                for ci in range(4):
                    sl = slice(ci * 512, (ci + 1) * 512)
                    xc = pab_x.tile([128, 8, 512], bf16, tag="xc")
                    nc.sync.dma_start(xc[:], xT[:, :, sl])
                    # token-major sumsq -> invr1 for the 4 token tiles of the chunk
                    for kt in range(4):
                        x2 = pab_sb.tile([128, 8, 128], bf16, tag="x2")
                        nc.scalar.activation(x2[:], xc[:, :, kt * 128:(kt + 1) * 128],
                                             AF.Square)
                        ssp = pab_ps1.tile([128, 2], f32, tag="sstk")
                        for k in range(8):
                            nc.tensor.matmul(ssp[:], x2[:, k, :], ocb[:, 0:2],
                                             start=(k == 0), stop=(k == 7))
                        nc.scalar.activation(rms_st[:, ci * 4 + kt, None], ssp[:, 0:1],
                                             AF.Sqrt, scale=1.0 / D, bias=eps6[:])
                    nc.vector.reciprocal(invr1[:, ci * 4:(ci + 1) * 4],
                                         rms_st[:, ci * 4:(ci + 1) * 4])
                    # V token-major for the 4 token tiles
                    for kt in range(4 if absub >= 2 else 0):
                        gkt = ci * 4 + kt
                        vps = pab_ps.tile([128, KV], f32, tag="vps")
                        for k in range(8):
                            nc.tensor.matmul(vps[:], xc[:, k, kt * 128:(kt + 1) * 128],
                                             wvs[:, k, :], start=(k == 0), stop=(k == 7))
                        nc.vector.tensor_scalar_mul(
                            v_all[:, :, gkt, 0:64],
                            vps[:].rearrange("p (g d) -> p g d", g=4),
                            invr1[:, gkt, None])
                    # K feature-major for both kv pairs
                    for kp in range(2 if absub >= 3 else 0):
                        kps = pab_ps.tile([128, 512], f32, tag="kps")
                        for k in range(8):
                            nc.tensor.matmul(kps[:], wks[:, k, kp * 128:(kp + 1) * 128],
                                             xc[:, k, :], start=(k == 0), stop=(k == 7))
                        kraw = pab_sb.tile([128, 512], f32r, tag="kraw")
                        nc.any.tensor_copy(kraw[:], kps[:])
                        ksw = pab_ps1.tile([128, 512], f32, tag="ksw")
                        nc.tensor.matmul(ksw[:], perm[:], kraw[:], start=True, stop=True)
                        if absub < 4:
                            continue
                        k2 = pab_sb.tile([128, 512], bf16, tag="k2")
                        nc.scalar.activation(k2[:], kps[:], AF.Square)
                        # per-token sumsq token-major, then transpose to row-major
                        sstk = pab_ps1.tile([128, 4, 2], f32, tag="sstk")
                        for kt in range(4):
                            ksl = slice(kt * 128, (kt + 1) * 128)
                            nc.tensor.matmul(sstk[:, kt, :], k2[:, ksl], och[:, 0:2],
                                             start=True, stop=True)
                        sstk_sb = pab_sb.tile([128, 4, 2], bf16, tag="sstk_sb")
                        nc.any.tensor_copy(sstk_sb[:], sstk[:])
                        ssrow = pab_ps1.tile([2, 512], f32, tag="ssrow")
                        for kt in range(4):
                            ksl = slice(kt * 128, (kt + 1) * 128)
                            nc.tensor.matmul(ssrow[0:2, ksl], sstk_sb[:, kt, :],
                                             idt[:], start=True, stop=True)
                        rmsk = pab_sb.tile([2, 512], f32, tag="rmsk")
                        nc.scalar.activation(rmsk[:], ssrow[0:2, :], AF.Sqrt,
                                             scale=1.0 / HD, bias=epsq[0:2, :])
                        invk = pab_sb.tile([2, 512], f32, tag="invk")
                        nc.vector.reciprocal_approx_fast(invk[:], rmsk[:])
                        repk = pab_ps1.tile([128, 512], f32, tag="repk")
                        nc.tensor.matmul(repk[:], selkt[:], invk[:],
                                         start=True, stop=True)
                        if absub < 5:
                            continue
                        t1 = pab_sb.tile([128, 512], f32, tag="t1k")
                        nc.vector.tensor_tensor(t1[:], kraw[:], cosf[:, sl], ALU.mult)
                        t2 = pab_sb.tile([128, 512], f32, tag="t2k")
                        nc.vector.tensor_tensor(t2[:], ksw[:], sinf[:, sl], ALU.mult)
                        t3 = pab_sb.tile([128, 512], f32, tag="t3k")
                        nc.vector.tensor_tensor(t3[:], t1[:], t2[:], ALU.add)
                        nc.vector.tensor_tensor(kT[:, kp, sl], t3[:], repk[:], ALU.mult)
                # ones column of V
                nc.vector.tensor_copy(
                    v_all[:, :, :, 64:66],
                    ocb[:, 0, None, None].to_broadcast([128, 4, NT, 2]))

        # ------------- Phase C: Q for own tokens --------------------------
        if ph_on("c"):
            with tc.tile_pool(name="pc_x", bufs=1) as pc_x, \
                 tc.tile_pool(name="pc_sb", bufs=3) as pc_sb, \
                 tc.tile_pool(name="pc_w", bufs=3) as pc_w, \
                 tc.tile_pool(name="pc_ps", bufs=2, space="PSUM") as pc_ps, \
                 tc.tile_pool(name="pc_ps1", bufs=1, space="PSUM") as pc_ps1:
                xqs = pc_x.tile([128, 8, OWN], bf16, tag="xqs")
                nc.sync.dma_start(xqs[:], xq[:])
                for p in range(8):
                    wqs = pc_w.tile([128, 8, 128], bf16, tag="wqs")
                    nc.sync.dma_start(wqs[:], wq[p])
                    qps = pc_ps.tile([128, OWN], f32, tag="qps")
                    for k in range(8):
                        nc.tensor.matmul(qps[:], wqs[:, k, :], xqs[:, k, :],
                                         start=(k == 0), stop=(k == 7))
                    qraw = pc_sb.tile([128, OWN], f32r, tag="qraw")
                    nc.any.tensor_copy(qraw[:], qps[:])
                    qsw = pc_ps.tile([128, OWN], f32, tag="qsw")
                    nc.tensor.matmul(qsw[:], perm[:], qraw[:], start=True, stop=True)
                    q2 = pc_sb.tile([128, OWN], bf16, tag="q2")
                    nc.scalar.activation(q2[:], qps[:], AF.Square)
                    sstq = pc_ps1.tile([128, 4, 2], f32, tag="sstq")
                    for kt in range(4):
                        ksl = slice(kt * 128, (kt + 1) * 128)
                        nc.tensor.matmul(sstq[:, kt, :], q2[:, ksl], och[:, 0:2],
                                         start=True, stop=True)
                    sstq_sb = pc_sb.tile([128, 4, 2], bf16, tag="sstq_sb")
                    nc.any.tensor_copy(sstq_sb[:], sstq[:])
                    ssqrow = pc_ps1.tile([2, OWN], f32, tag="ssqrow")
                    for kt in range(4):
                        ksl = slice(kt * 128, (kt + 1) * 128)
                        nc.tensor.matmul(ssqrow[0:2, ksl], sstq_sb[:, kt, :],
                                         idt[:], start=True, stop=True)
                    rmsq = pc_sb.tile([2, OWN], f32, tag="rmsq")
                    nc.scalar.activation(rmsq[:], ssqrow[0:2, :], AF.Sqrt,
                                         scale=1.0 / HD, bias=epsq[0:2, :])
                    invq = pc_sb.tile([2, OWN], f32, tag="invq")
                    nc.vector.reciprocal_approx_fast(invq[:], rmsq[:])
                    repq = pc_ps1.tile([128, OWN], f32, tag="repq")
                    nc.tensor.matmul(repq[:], selg[:, p, :], invq[:],
                                     start=True, stop=True)
                    t1 = pc_sb.tile([128, OWN], f32, tag="t1q")
                    nc.vector.tensor_tensor(t1[:], qraw[:], coso[:], ALU.mult)
                    t2 = pc_sb.tile([128, OWN], f32, tag="t2q")
                    nc.vector.tensor_tensor(t2[:], qsw[:], sino[:], ALU.mult)
                    t3 = pc_sb.tile([128, OWN], f32, tag="t3q")
                    nc.vector.tensor_tensor(t3[:], t1[:], t2[:], ALU.add)
                    nc.vector.tensor_tensor(qT[:, p, :], t3[:], repq[:], ALU.mult)

            rope_stack.close()

        # ------------- Phase D: attention ---------------------------------
        if ph_on("d"):
            xpr = big.tile([128, 8, OWN], bf16, tag="xpr")
            xpb = big.tile([128, 8, OWN], f32, tag="xpb")
            with tc.tile_pool(name="pd_m", bufs=1) as pd_m, \
                 tc.tile_pool(name="pd_pt", bufs=6) as pd_pt, \
                 tc.tile_pool(name="pd_sb", bufs=2) as pd_sb, \
                 tc.tile_pool(name="pd_s", bufs=2, space="PSUM") as pd_s, \
                 tc.tile_pool(name="pd_y", bufs=1, space="PSUM") as pd_y, \
                 tc.tile_pool(name="pd_r", bufs=1, space="PSUM") as pd_r:
                masks = pd_m.tile([128, 4, 4, 128], bf16, tag="masks")
                nc.sync.dma_start(masks[:], maskM[:])
                for t in range(4):
                    qsl = slice(t * 128, (t + 1) * 128)
                    n_chunks = t + 1
                    n_kvt = 4 * n_chunks
                    for half in range(2):
                        gA, gB = 2 * half, 2 * half + 1
                        yA = pd_y.tile([66, 4, 128], f32, tag="yA")
                        yB = pd_y.tile([66, 4, 128], f32, tag="yB")
                        qsA = qT[0:64, 4 * half:4 * half + 4, qsl]
                        qsB = qT[64:128, 4 * half:4 * half + 4, qsl]
                        for c in range(n_chunks):
                            pts = []
                            for i in range(4):
                                ks = slice((4 * c + i) * 128, (4 * c + i + 1) * 128)
                                psAB = pd_s.tile([128, 2, 4, 128], f32, tag="psAB")
                                nc.tensor.matmul(psAB[:, 0, :, :],
                                                 kT[0:64, half, ks], qsA,
                                                 start=True, stop=True,
                                                 tile_position=(0, 0))
                                nc.tensor.matmul(psAB[:, 1, :, :],
                                                 kT[64:128, half, ks], qsB,
                                                 start=True, stop=True,
                                                 tile_position=(64, 0))
                                ptAB = pd_pt.tile([128, 2, 4, 128], bf16, tag="ptAB")
                                nc.scalar.activation(ptAB[:], psAB[:], AF.Exp)
                                if c == t:
                                    mbc = masks[:, t, i, None, None, :].to_broadcast(
                                        [128, 2, 4, 128])
                                    eng = nc.vector if i % 2 == 0 else nc.gpsimd
                                    eng.tensor_tensor(ptAB[:], ptAB[:], mbc, ALU.mult)
                                pts.append(ptAB)
                            for i in range(4):
                                kvt = 4 * c + i
                                nc.tensor.matmul(yA[:], v_all[:, gA, kvt, :],
                                                 pts[i][:, 0, :, :], start=(kvt == 0),
                                                 stop=(kvt == n_kvt - 1))
                                nc.tensor.matmul(yB[:], v_all[:, gB, kvt, :],
                                                 pts[i][:, 1, :, :], start=(kvt == 0),
                                                 stop=(kvt == n_kvt - 1))
                        for g, y in ((gA, yA), (gB, yB)):
                            dsb = pd_sb.tile([2, 4, 128], f32, tag="dsb")
                            nc.vector.tensor_copy(dsb[:], y[64:66, :, :])
                            invs = pd_sb.tile([2, 4, 128], f32, tag="invs")
                            nc.vector.reciprocal_approx_fast(invs[:], dsb[:])
                            ysb = pd_sb.tile([64, 4, 128], f32, tag="ysb")
                            nc.vector.tensor_copy(ysb[:], y[0:64, :, :])
                            repy = pd_r.tile([64, 4, 128], f32, tag="repy")
                            nc.tensor.matmul(repy[:].rearrange("p a b -> p (a b)"),
                                             o10t[:, 0:64],
                                             invs[:].rearrange("p a b -> p (a b)"),
                                             start=True, stop=True)
                            for i in range(4):
                                h = 4 * g + i
                                chunk, part = h // 2, (h % 2) * 64
                                nc.vector.tensor_tensor(
                                    y_all[part:part + 64, chunk, qsl],
                                    ysb[:, i, :], repy[:, i, :], ALU.mult)

        # ------------- Phase E: Wo + post-norm + residual -----------------
        if ph_on("e"):
            with tc.tile_pool(name="pe_sb", bufs=2) as pe_sb, \
                 tc.tile_pool(name="pe_ao", bufs=1) as pe_ao, \
                 tc.tile_pool(name="pe_w", bufs=3) as pe_w, \
                 tc.tile_pool(name="pe_ps", bufs=2, space="PSUM") as pe_ps, \
                 tc.tile_pool(name="pe_ss", bufs=1, space="PSUM") as pe_ss:
                ao = pe_ao.tile([128, 8, OWN], f32, tag="ao")
                ssa = pe_ss.tile([2, OWN], f32, tag="ssa")
                for o in range(8):
                    wos = pe_w.tile([128, 8, 128], bf16, tag="wos")
                    nc.sync.dma_start(wos[:], wo[o])
                    aps = pe_ps.tile([128, OWN], f32, tag="aps")
                    for k in range(8):
                        nc.tensor.matmul(aps[:], wos[:, k, :], y_all[:, k, :],
                                         start=(k == 0), stop=(k == 7))
                    nc.any.tensor_copy(ao[:, o, :], aps[:])
                    a2 = pe_sb.tile([128, OWN], bf16, tag="a2")
                    nc.scalar.activation(a2[:], aps[:], AF.Square)
                    nc.tensor.matmul(ssa[:], ocb[:, 0:2], a2[:],
                                     start=(o == 0), stop=(o == 7))
                rmsa = pe_sb.tile([2, OWN], f32, tag="rmsa")
                nc.scalar.activation(rmsa[:], ssa[0:2, :], AF.Sqrt,
                                     scale=1.0 / D, bias=eps6[0:2, :])
                inva = pe_sb.tile([2, OWN], f32, tag="inva")
                nc.vector.reciprocal_approx_fast(inva[:], rmsa[:])
                repa = pe_ss.tile([128, OWN], f32, tag="repa")
                nc.tensor.matmul(repa[:], o10t[:], inva[:], start=True, stop=True)
                for o in range(8):
                    t1 = pe_sb.tile([128, OWN], f32, tag="t1e")
                    nc.vector.tensor_tensor(t1[:], ao[:, o, :], repa[:], ALU.mult)
                    nc.vector.scalar_tensor_tensor(
                        xpb[:, o, :], t1[:], gat[:, o, None], xrs[:, o, :],
                        ALU.mult, ALU.add)
                    nc.any.tensor_copy(xpr[:, o, :], xpb[:, o, :])
                    nc.vector.tensor_scalar_add(xpb[:, o, :], xpb[:, o, :],
                                                bml[:, o, None])

        # ------------- Phase F: MLP ---------------------------------------
        if ph_on("f"):
            mout = big.tile([128, 8, OWN], f32, tag="xrs_mout")
            with tc.tile_pool(name="pf_h2", bufs=1) as pf_h2, \
                 tc.tile_pool(name="pf_sb", bufs=2) as pf_sb, \
                 tc.tile_pool(name="pf_wf", bufs=3) as pf_wf, \
                 tc.tile_pool(name="pf_wp", bufs=3) as pf_wp, \
                 tc.tile_pool(name="pf_ps", bufs=2, space="PSUM") as pf_ps, \
                 tc.tile_pool(name="pf_mo", bufs=1, space="PSUM") as pf_mo:
                h2 = pf_h2.tile([128, 32, OWN], bf16, tag="h2")
                for hc in range(32):
                    wfs = pf_wf.tile([128, 8, 128], bf16, tag="wfs")
                    nc.sync.dma_start(wfs[:], wfc[hc])
                    hps = pf_ps.tile([128, OWN], f32, tag="hps")
                    for k in range(8):
                        nc.tensor.matmul(hps[:], wfs[:, k, :], xpr[:, k, :],
                                         start=(k == 0), stop=(k == 7))
                    hr = pf_sb.tile([128, OWN], bf16, tag="hr")
                    nc.scalar.activation(hr[:], hps[:], AF.Relu)
                    nc.vector.tensor_tensor(h2[:, hc, :], hr[:], hr[:], ALU.mult)
                ssm = pf_ps.tile([2, OWN], f32, tag="ssm")
                for ohalf in range(2):
                    mo_ps = [pf_mo.tile([128, OWN], f32, name=f"mo{oi}", tag=f"mo{oi}")
                             for oi in range(4)]
                    for hc in range(32):
                        wps = pf_wp.tile([128, 4, 128], bf16, tag="wps")
                        nc.sync.dma_start(wps[:], wprojq[hc, ohalf])
                        for oi in range(4):
                            nc.tensor.matmul(mo_ps[oi][:], wps[:, oi, :], h2[:, hc, :],
                                             start=(hc == 0), stop=(hc == 31))
                    for oi in range(4):
                        o = ohalf * 4 + oi
                        nc.any.tensor_copy(mout[:, o, :], mo_ps[oi][:])
                        m2 = pf_sb.tile([128, OWN], bf16, tag="m2")
                        nc.scalar.activation(m2[:], mo_ps[oi][:], AF.Square)
                        nc.tensor.matmul(ssm[:], ocb[:, 0:2], m2[:],
                                         start=(o == 0), stop=(o == 7))
                rmsm = pf_sb.tile([2, OWN], f32, tag="rmsm")
                nc.scalar.activation(rmsm[:], ssm[0:2, :], AF.Sqrt, scale=1.0 / D,
                                     bias=eps6[0:2, :])
                invm = pf_sb.tile([2, OWN], f32, tag="invm")
                nc.vector.reciprocal_approx_fast(invm[:], rmsm[:])
                repm = pf_ps.tile([128, OWN], f32, tag="hps")
                nc.tensor.matmul(repm[:], o10t[:], invm[:], start=True, stop=True)
                for o in range(8):
                    t1 = pf_sb.tile([128, OWN], f32, tag="t1f")
                    nc.vector.tensor_tensor(t1[:], mout[:, o, :], repm[:], ALU.mult)
                    outv = pf_sb.tile([128, OWN], f32, tag="outv")
                    nc.vector.scalar_tensor_tensor(
                        outv[:], t1[:], gml[:, o, None], xpb[:, o, :],
                        ALU.mult, ALU.add)
                    nc.sync.dma_start(out_t[:, o, :], outv[:])

        if not ph_on("f"):
            with tc.tile_pool(name="dummy", bufs=1) as dp:
                zout = dp.tile([128, 8, OWN], f32, tag="zout")
                nc.vector.memset(zout[:], 0.0)
                nc.sync.dma_start(out_t[:], zout[:])
            rope_stack.close()

    nc.finalize()
    return nc


def _feat_major(a):
    """[F, T] -> device layout [128, F//128, T]."""
    F, T = a.shape
    return np.ascontiguousarray(a.reshape(F // 128, 128, T).transpose(1, 0, 2))


def _vec_dev(v):
    return np.ascontiguousarray(v.reshape(-1, 128).T)


def _bf(a):
    return np.ascontiguousarray(a.astype(ml_dtypes.bfloat16))


_CACHE = {}
_RUN_KW = {}


def kernel(x, attn_norm_w, mlp_norm_w, attn_post_norm_w, mlp_post_norm_w,
           attn_scale, mlp_scale, attn_mod_gain, attn_mod_bias,
           mlp_mod_gain, mlp_mod_bias, Wq, Wk, Wv, Wo, q_gain, fc_w, proj_w):
    x = np.asarray(x, np.float32)
    q_gain = np.asarray(q_gain, np.float32)

    if "nc" not in _CACHE:
        _CACHE["nc"] = build(q_gain)
    nc = _CACHE["nc"]

    anw = np.asarray(attn_norm_w, np.float32)
    mnw = np.asarray(mlp_norm_w, np.float32)
    wq_eff = np.asarray(Wq, np.float32) * anw[None, :]
    wk_eff = np.asarray(Wk, np.float32) * anw[None, :]
    wv_eff = np.asarray(Wv, np.float32) * anw[None, :]
    fc_eff = np.asarray(fc_w, np.float32) * mnw[None, :]

    perm = np.zeros(D, np.int64)
    for p, (a, b) in enumerate(PAIRS):
        perm[p * 128:p * 128 + 64] = np.arange(a * 64, a * 64 + 64)
        perm[p * 128 + 64:(p + 1) * 128] = np.arange(b * 64, b * 64 + 64)
    WqTp = wq_eff.T[:, perm]                                  # [D_in, D_out-perm]
    wq_dev = _bf(np.stack([_feat_major(WqTp[:, p * 128:(p + 1) * 128]) for p in range(8)]))
    wk_dev = _bf(_feat_major(wk_eff.T))
    wv_dev = _bf(_feat_major(wv_eff.T))
    WoT = np.asarray(Wo, np.float32).T
    wo_dev = _bf(np.stack([_feat_major(WoT[:, o * 128:(o + 1) * 128]) for o in range(8)]))
    fcT = fc_eff.T
    wfc_dev = _bf(np.stack([_feat_major(fcT[:, h * 128:(h + 1) * 128]) for h in range(32)]))
    projT = np.asarray(proj_w, np.float32).T                  # [4096, 1024]
    wproj_dev = _bf(np.ascontiguousarray(
        projT.reshape(32, 128, 2, 4, 128).transpose(0, 2, 1, 3, 4)))

    inv_freq = 1.0 / (ROPE_BASE ** (np.arange(0, HD, 2, dtype=np.float32) / HD))
    tpos = np.arange(S, dtype=np.float32)
    freqs = np.outer(tpos, inv_freq).astype(np.float32)
    cosT = np.ascontiguousarray(np.tile(np.cos(freqs).T, (4, 1)))   # [128, S]
    sin1 = np.sin(freqs).T                                          # [32, S]
    sinS = np.ascontiguousarray(
        np.concatenate([sin1, -sin1, sin1, -sin1], axis=0))         # [128, S] sign-folded

    # swap permutation i <-> i^32 (within each 64-wide head)
    permM_h = np.zeros((128, 128), np.float32)
    for i in range(128):
        permM_h[i, i ^ 32] = 1.0

    oc_h_v = np.zeros((128, 2), np.float32)
    oc_h_v[0:64, 0] = 1.0
    oc_h_v[64:128, 1] = 1.0
    selg2_v = np.zeros((2, 8, 128), np.float32)
    for p, (a, b) in enumerate(PAIRS):
        selg2_v[0, p, 0:64] = q_gain[a] / 8.0
        selg2_v[1, p, 64:128] = q_gain[b] / 8.0
    ones10_v = np.concatenate([np.ones((1, 128), np.float32),
                               np.zeros((1, 128), np.float32)])

    gat_v = (np.asarray(attn_post_norm_w, np.float32)
             * np.asarray(attn_mod_gain, np.float32)
             * np.asarray(attn_scale, np.float32))
    bat_v = np.asarray(attn_mod_bias, np.float32) * np.asarray(attn_scale, np.float32)
    gml_v = (np.asarray(mlp_post_norm_w, np.float32)
             * np.asarray(mlp_mod_gain, np.float32)
             * np.asarray(mlp_scale, np.float32))
    bml_v = np.asarray(mlp_mod_bias, np.float32) * np.asarray(mlp_scale, np.float32)

    shared = {
        "wq": wq_dev, "wk": wk_dev, "wv": wv_dev, "wo": wo_dev,
        "wfc": wfc_dev, "wprojq": wproj_dev,
        "cosF": cosT, "sinF": sinS,
        "permM": permM_h,
        "oc_h": _bf(oc_h_v),
        "onescb": _bf(np.ones((128, 2), np.float32)),
        "selg2": selg2_v,
        "identM": _bf(np.eye(128, dtype=np.float32)),
        "selk": np.stack([np.concatenate([np.ones(64, np.float32), np.zeros(64, np.float32)]),
                          np.concatenate([np.zeros(64, np.float32), np.ones(64, np.float32)])]),
        "o10": ones10_v,
        "g_attn": _vec_dev(gat_v), "g_mlp": _vec_dev(gml_v),
        "b_mlp": _vec_dev(bml_v),
    }

    in_maps = []
    owners = []
    for c in range(8):
        b, j = c // 4, c % 4
        rows = np.concatenate(
            [np.arange((j + 4 * t) * 128, (j + 4 * t + 1) * 128) for t in range(4)])
        owners.append((b, rows))
        xb = x[b].T
        x_own = xb[:, rows]
        mask = np.zeros((4, 4, 128, 128), np.float32)
        for t in range(4):
            m = j + 4 * t
            q_idx = m * 128 + np.arange(128)
            for ktl in range(4):
                kv_idx = 512 * t + 128 * ktl + np.arange(128)
                mask[t, ktl] = (kv_idx[:, None] <= q_idx[None, :])
        m_in = {
            "xT": _bf(_feat_major(xb)),
            "xq": _bf(_feat_major(x_own)),
            "xres": _feat_major(x_own + bat_v[:, None]),
            "cosO": np.ascontiguousarray(cosT[:, rows]),
            "sinO": np.ascontiguousarray(sinS[:, rows]),
            "maskM": _bf(np.ascontiguousarray(mask.transpose(2, 0, 1, 3))),
        }
        m_in.update(shared)
        in_maps.append(m_in)

    res = run_bass_kernel_spmd(nc, in_maps, core_ids=list(range(8)),
                               **_RUN_KW)
    _CACHE["last_result"] = res

    out = np.empty((B, S, D), np.float32)
    for c in range(8):
        b, rows = owners[c]
        o = res.results[c]["out"]
        out[b, rows, :] = o.transpose(2, 1, 0).reshape(OWN, D)
    return out


# revision 51
# speedup vs baseline: 1.2750x; 1.1248x over previous
"""Trainium2 Bass kernel for one dense transformer block (B=2, S=2048, D=1024,
16 q-heads / 4 kv-heads GQA, squared-ReLU MLP), data-parallel over 8 NeuronCores.

Sharding: core c = (b, j), b = c // 4, j = c % 4, owns q-token tiles
{j, j+4, j+8, j+12} (128 tokens each) of batch b. K/V are computed for the full
sequence on every core (no collectives). The kv range for own q-tile t is
padded to 512*(t+1); causality enforced with per-core 0/1 masks on the last
512-wide kv chunk.

Numerical identities used (exact up to negligible eps rescaling):
  - per-head q/k rmsnorm is scale-invariant per token, so the block input
    rmsnorm cancels inside it -> Q/K project from raw (norm-weight-folded) x
  - the MLP input rmsnorm cancels through relu()^2 -> proj -> post-rmsnorm
  - V is projected from raw x and rescaled by 1/rms1(x) per token
  - no softmax max-subtraction (logits bounded by |q||k|/8 = 8)
  - softmax denominator = ones-column appended to V in the AV matmul
  - K's 1/rms is applied as a per-partition AP scale inside the exp
    activation (kv tokens are partitions in the score tiles); Q's 1/rms and
    q_gain/8 ride a replicate matmul onto qT

v2 perf changes vs baseline:
  - no DVE reciprocal with f32r destination (was ~7.7ns/elem); all recips are
    fp32->fp32 on DVE, replicates via small fp32 matmuls
  - rope via a feature-swap permutation matmul + 3 full-width DVE ops
    (was 12 narrow DVE ops)
  - bf16 weights + x + V/p/mask/y/h2 paths (half DMA, FWL weight loads,
    2x DVE); q/k/scores stay f32r
"""

import os

import numpy as np
import ml_dtypes

import concourse.bass as bass
from concourse import bacc
import concourse.tile as tile
import concourse.mybir as mybir
from concourse.bass_utils import run_bass_kernel_spmd

f32 = mybir.dt.float32
f32r = mybir.dt.float32r
bf16 = mybir.dt.bfloat16
AF = mybir.ActivationFunctionType
ALU = mybir.AluOpType

B, S, D = 2, 2048, 1024
H, HKV, HD = 16, 4, 64
MLP_HID = 4 * D
KV = HKV * HD
NT = 16
OWN = 512
EPS_BLOCK = 1e-6
EPS_QK = float(np.finfo(np.float32).eps)
ROPE_BASE = 10000.0

PAIRS = [(0, 4), (1, 5), (2, 6), (3, 7), (8, 12), (9, 13), (10, 14), (11, 15)]

PHASE_ORDER = ["ab", "c", "d", "e", "f"]


def build(q_gain):
    max_ph = os.environ.get("KERNEL_PHASES", "f")
    ph_on = lambda p: PHASE_ORDER.index(p) <= PHASE_ORDER.index(max_ph)
    bacc.Bacc.move_matmul_waits_to_ldweights = lambda self: None
    nc = bacc.Bacc(None)

    def dram_in(name, shape, dt):
        return nc.dram_tensor(name, list(shape), dt, kind="ExternalInput")

    xT = dram_in("xT", (128, 8, S), bf16)
    xq = dram_in("xq", (128, 8, OWN), bf16)
    xres = dram_in("xres", (128, 8, OWN), f32)
    wq = dram_in("wq", (8, 128, 8, 128), bf16)
    wk = dram_in("wk", (128, 8, KV), bf16)
    wv = dram_in("wv", (128, 8, KV), bf16)
    wo = dram_in("wo", (8, 128, 8, 128), bf16)
    wfc = dram_in("wfc", (32, 128, 8, 128), bf16)
    wprojq = dram_in("wprojq", (32, 2, 128, 4, 128), bf16)
    cosF = dram_in("cosF", (128, S), f32)
    sinF = dram_in("sinF", (128, S), f32)   # sign-folded: +sin rows 0-31/64-95, -sin rows 32-63/96-127
    cosO = dram_in("cosO", (128, OWN), f32)
    sinO = dram_in("sinO", (128, OWN), f32)
    maskM = dram_in("maskM", (128, 4, 4, 128), bf16)
    permM = dram_in("permM", (128, 128), f32r)     # swap rows i <-> i^32
    oc_h = dram_in("oc_h", (128, 2), bf16)         # col0: top-64 ones; col1: bottom-64 ones
    onescb = dram_in("onescb", (128, 2), bf16)     # all ones
    selg2 = dram_in("selg2", (2, 8, 128), bf16)     # row0 -> cols 0-63 * gA/8, row1 -> cols 64-127 * gB/8
    identM = dram_in("identM", (128, 128), bf16)   # 128x128 identity
    selk = dram_in("selk", (2, 128), bf16)          # row0 -> cols 0-63 ones, row1 -> cols 64-127 ones
    o10 = dram_in("o10", (2, 128), f32)
    o10bf = dram_in("o10bf", (2, 128), bf16)            # row0 ones, row1 zeros
    g_attn = dram_in("g_attn", (128, 8), f32)
    g_mlp = dram_in("g_mlp", (128, 8), f32)
    b_mlp = dram_in("b_mlp", (128, 8), f32)

    out_t = nc.dram_tensor("out", [128, 8, OWN], f32, kind="ExternalOutput")

    with tile.TileContext(nc) as tc, \
         tc.tile_pool(name="cst", bufs=1) as cst, \
         tc.tile_pool(name="big", bufs=1) as big:
        och = cst.tile([128, 2], bf16, tag="och")
        nc.sync.dma_start(och[:], oc_h[:])
        ocb = cst.tile([128, 2], bf16, tag="ocb")
        nc.sync.dma_start(ocb[:], onescb[:])
        selg = cst.tile([2, 8, 128], bf16, tag="selg")
        nc.sync.dma_start(selg[:], selg2[:])
        idt = cst.tile([128, 128], bf16, tag="idt")
        nc.sync.dma_start(idt[:], identM[:])
        selkt = cst.tile([2, 128], bf16, tag="selkt")
        nc.sync.dma_start(selkt[:], selk[:])
        o10t = cst.tile([2, 128], f32, tag="o10t")
        nc.sync.dma_start(o10t[:], o10[:])
        o10b = cst.tile([2, 128], bf16, tag="o10b")
        nc.sync.dma_start(o10b[:], o10bf[:])
        perm = cst.tile([128, 128], f32r, tag="perm")
        nc.sync.dma_start(perm[:], permM[:])
        eps6 = cst.tile([128, 1], f32, tag="eps6")
        nc.vector.memset(eps6[:], EPS_BLOCK)
        epsq = cst.tile([128, 1], f32, tag="epsq")
        nc.vector.memset(epsq[:], EPS_QK)
        gat = cst.tile([128, 8], f32, tag="gat")
        nc.sync.dma_start(gat[:], g_attn[:])
        gml = cst.tile([128, 8], f32, tag="gml")
        nc.sync.dma_start(gml[:], g_mlp[:])
        bml = cst.tile([128, 8], f32, tag="bml")
        nc.sync.dma_start(bml[:], b_mlp[:])
        from contextlib import ExitStack
        rope_stack = ExitStack()
        ropep = rope_stack.enter_context(tc.tile_pool(name="ropep", bufs=1))
        cosf = ropep.tile([128, S], f32, tag="cosf")
        sinf = ropep.tile([128, S], f32, tag="sinf")
        coso = ropep.tile([128, OWN], f32, tag="coso")
        sino = ropep.tile([128, OWN], f32, tag="sino")

        kT = big.tile([128, 2, S], f32r, tag="kT")
        v_all = big.tile([128, 4, NT, 66], bf16, tag="v_all")
        qT = big.tile([128, 8, OWN], f32r, tag="qT")
        y_all = big.tile([128, 8, OWN], bf16, tag="y_all")
        xrs = big.tile([128, 8, OWN], f32, tag="xrs_mout")
        invr1 = big.tile([128, NT], f32, tag="invr1")
        rms_st = big.tile([128, NT], f32, tag="rms_st")

        # ------------- Phase AB: rms1, K, V over full sequence ------------
        absub = int(os.environ.get("KERNEL_AB_SUB", "99"))
        if ph_on("ab"):
            with tc.tile_pool(name="pab_x", bufs=3) as pab_x, \
                 tc.tile_pool(name="pab_sb", bufs=2) as pab_sb, \
                 tc.tile_pool(name="pab_w", bufs=1) as pab_w, \
                 tc.tile_pool(name="pab_ps", bufs=2, space="PSUM") as pab_ps, \
                 tc.tile_pool(name="pab_ps1", bufs=1, space="PSUM") as pab_ps1:
                wvs = pab_w.tile([128, 8, KV], bf16, tag="wvs")
                nc.sync.dma_start(wvs[:], wv[:])
                wks = pab_w.tile([128, 8, KV], bf16, tag="wks")
                nc.sync.dma_start(wks[:], wk[:])
                nc.sync.dma_start(cosf[:], cosF[:])
                nc.sync.dma_start(sinf[:], sinF[:])
                nc.sync.dma_start(coso[:], cosO[:])
                nc.sync.dma_start(sino[:], sinO[:])
                nc.sync.dma_start(xrs[:], xres[:])
                for ci in range(4):
                    sl = slice(ci * 512, (ci + 1) * 512)
                    xc = pab_x.tile([128, 8, 512], bf16, tag="xc")
                    nc.sync.dma_start(xc[:], xT[:, :, sl])
                    # token-major sumsq -> invr1 for the 4 token tiles of the chunk
                    for kt in range(4):
                        x2 = pab_sb.tile([128, 8, 128], bf16, tag="x2")
                        nc.scalar.activation(x2[:], xc[:, :, kt * 128:(kt + 1) * 128],
                                             AF.Square)
                        ssp = pab_ps1.tile([128, 2], f32, tag="sstk")
                        for k in range(8):
                            nc.tensor.matmul(ssp[:], x2[:, k, :], ocb[:, 0:2],
                                             start=(k == 0), stop=(k == 7))
                        nc.scalar.activation(rms_st[:, ci * 4 + kt, None], ssp[:, 0:1],
                                             AF.Sqrt, scale=1.0 / D, bias=eps6[:])
                    nc.vector.reciprocal(invr1[:, ci * 4:(ci + 1) * 4],
                                         rms_st[:, ci * 4:(ci + 1) * 4])
                    # V token-major for the 4 token tiles
                    for kt in range(4 if absub >= 2 else 0):
                        gkt = ci * 4 + kt
                        vps = pab_ps.tile([128, KV], f32, tag="vps")
                        for k in range(8):
                            nc.tensor.matmul(vps[:], xc[:, k, kt * 128:(kt + 1) * 128],
                                             wvs[:, k, :], start=(k == 0), stop=(k == 7))
                        nc.vector.tensor_scalar_mul(
                            v_all[:, :, gkt, 0:64],
                            vps[:].rearrange("p (g d) -> p g d", g=4),
                            invr1[:, gkt, None])
                    # K feature-major for both kv pairs
                    for kp in range(2 if absub >= 3 else 0):
                        kps = pab_ps.tile([128, 512], f32, tag="kps")
                        for k in range(8):
                            nc.tensor.matmul(kps[:], wks[:, k, kp * 128:(kp + 1) * 128],
                                             xc[:, k, :], start=(k == 0), stop=(k == 7))
                        kraw = pab_sb.tile([128, 512], f32r, tag="kraw")
                        nc.any.tensor_copy(kraw[:], kps[:])
                        ksw = pab_ps1.tile([128, 512], f32, tag="ksw")
                        nc.tensor.matmul(ksw[:], perm[:], kraw[:], start=True, stop=True)
                        if absub < 4:
                            continue
                        k2 = pab_sb.tile([128, 512], bf16, tag="k2")
                        nc.scalar.activation(k2[:], kps[:], AF.Square)
                        # per-token sumsq token-major, then transpose to row-major
                        sstk = pab_ps1.tile([128, 4, 2], f32, tag="sstk")
                        for kt in range(4):
                            ksl = slice(kt * 128, (kt + 1) * 128)
                            nc.tensor.matmul(sstk[:, kt, :], k2[:, ksl], och[:, 0:2],
                                             start=True, stop=True)
                        sstk_sb = pab_sb.tile([128, 4, 2], bf16, tag="sstk_sb")
                        nc.any.tensor_copy(sstk_sb[:], sstk[:])
                        ssrow = pab_ps1.tile([2, 512], f32, tag="ssrow")
                        for kt in range(4):
                            ksl = slice(kt * 128, (kt + 1) * 128)
                            nc.tensor.matmul(ssrow[0:2, ksl], sstk_sb[:, kt, :],
                                             idt[:], start=True, stop=True)
                        rmsk = pab_sb.tile([2, 512], f32, tag="rmsk")
                        nc.scalar.activation(rmsk[:], ssrow[0:2, :], AF.Sqrt,
                                             scale=1.0 / HD, bias=epsq[0:2, :])
                        invk = pab_sb.tile([2, 512], f32, tag="invk")
                        nc.vector.reciprocal_approx_fast(invk[:], rmsk[:])
                        invkb = pab_sb.tile([2, 512], bf16, tag="invkb")
                        nc.vector.tensor_copy(invkb[:], invk[:])
                        repk = pab_ps1.tile([128, 512], f32, tag="repk")
                        nc.tensor.matmul(repk[:], selkt[:], invkb[:],
                                         start=True, stop=True)
                        if absub < 5:
                            continue
                        t1 = pab_sb.tile([128, 512], f32, tag="t1k")
                        nc.vector.tensor_tensor(t1[:], kraw[:], cosf[:, sl], ALU.mult)
                        t2 = pab_sb.tile([128, 512], f32, tag="t2k")
                        nc.vector.tensor_tensor(t2[:], ksw[:], sinf[:, sl], ALU.mult)
                        t3 = pab_sb.tile([128, 512], f32, tag="t3k")
                        nc.vector.tensor_tensor(t3[:], t1[:], t2[:], ALU.add)
                        nc.vector.tensor_tensor(kT[:, kp, sl], t3[:], repk[:], ALU.mult)
                # ones column of V
                nc.vector.tensor_copy(
                    v_all[:, :, :, 64:66],
                    ocb[:, 0, None, None].to_broadcast([128, 4, NT, 2]))

        # ------------- Phase C: Q for own tokens --------------------------
        if ph_on("c"):
            with tc.tile_pool(name="pc_x", bufs=1) as pc_x, \
                 tc.tile_pool(name="pc_sb", bufs=3) as pc_sb, \
                 tc.tile_pool(name="pc_w", bufs=3) as pc_w, \
                 tc.tile_pool(name="pc_ps", bufs=2, space="PSUM") as pc_ps, \
                 tc.tile_pool(name="pc_ps1", bufs=1, space="PSUM") as pc_ps1:
                xqs = pc_x.tile([128, 8, OWN], bf16, tag="xqs")
                nc.sync.dma_start(xqs[:], xq[:])
                for p in range(8):
                    wqs = pc_w.tile([128, 8, 128], bf16, tag="wqs")
                    nc.sync.dma_start(wqs[:], wq[p])
                    qps = pc_ps.tile([128, OWN], f32, tag="qps")
                    for k in range(8):
                        nc.tensor.matmul(qps[:], wqs[:, k, :], xqs[:, k, :],
                                         start=(k == 0), stop=(k == 7))
                    qraw = pc_sb.tile([128, OWN], f32r, tag="qraw")
                    nc.any.tensor_copy(qraw[:], qps[:])
                    qsw = pc_ps.tile([128, OWN], f32, tag="qsw")
                    nc.tensor.matmul(qsw[:], perm[:], qraw[:], start=True, stop=True)
                    q2 = pc_sb.tile([128, OWN], bf16, tag="q2")
                    nc.scalar.activation(q2[:], qps[:], AF.Square)
                    sstq = pc_ps1.tile([128, 4, 2], f32, tag="sstq")
                    for kt in range(4):
                        ksl = slice(kt * 128, (kt + 1) * 128)
                        nc.tensor.matmul(sstq[:, kt, :], q2[:, ksl], och[:, 0:2],
                                         start=True, stop=True)
                    sstq_sb = pc_sb.tile([128, 4, 2], bf16, tag="sstq_sb")
                    nc.any.tensor_copy(sstq_sb[:], sstq[:])
                    ssqrow = pc_ps1.tile([2, OWN], f32, tag="ssqrow")
                    for kt in range(4):
                        ksl = slice(kt * 128, (kt + 1) * 128)
                        nc.tensor.matmul(ssqrow[0:2, ksl], sstq_sb[:, kt, :],
                                         idt[:], start=True, stop=True)
                    rmsq = pc_sb.tile([2, OWN], f32, tag="rmsq")
                    nc.scalar.activation(rmsq[:], ssqrow[0:2, :], AF.Sqrt,
                                         scale=1.0 / HD, bias=epsq[0:2, :])
                    invq = pc_sb.tile([2, OWN], f32, tag="invq")
                    nc.vector.reciprocal_approx_fast(invq[:], rmsq[:])
                    invqb = pc_sb.tile([2, OWN], bf16, tag="invqb")
                    nc.vector.tensor_copy(invqb[:], invq[:])
                    repq = pc_ps1.tile([128, OWN], f32, tag="repq")
                    nc.tensor.matmul(repq[:], selg[:, p, :], invqb[:],
                                     start=True, stop=True)
                    t1 = pc_sb.tile([128, OWN], f32, tag="t1q")
                    nc.vector.tensor_tensor(t1[:], qraw[:], coso[:], ALU.mult)
                    t2 = pc_sb.tile([128, OWN], f32, tag="t2q")
                    nc.vector.tensor_tensor(t2[:], qsw[:], sino[:], ALU.mult)
                    t3 = pc_sb.tile([128, OWN], f32, tag="t3q")
                    nc.vector.tensor_tensor(t3[:], t1[:], t2[:], ALU.add)
                    nc.vector.tensor_tensor(qT[:, p, :], t3[:], repq[:], ALU.mult)

            rope_stack.close()

        # ------------- Phase D: attention ---------------------------------
        if ph_on("d"):
            xpr = big.tile([128, 8, OWN], bf16, tag="xpr")
            xpb = big.tile([128, 8, OWN], f32, tag="xpb")
            with tc.tile_pool(name="pd_m", bufs=1) as pd_m, \
                 tc.tile_pool(name="pd_pt", bufs=6) as pd_pt, \
                 tc.tile_pool(name="pd_sb", bufs=2) as pd_sb, \
                 tc.tile_pool(name="pd_s", bufs=2, space="PSUM") as pd_s, \
                 tc.tile_pool(name="pd_y", bufs=1, space="PSUM") as pd_y, \
                 tc.tile_pool(name="pd_r", bufs=1, space="PSUM") as pd_r:
                masks = pd_m.tile([128, 4, 4, 128], bf16, tag="masks")
                nc.sync.dma_start(masks[:], maskM[:])
                for t in range(4):
                    qsl = slice(t * 128, (t + 1) * 128)
                    n_chunks = t + 1
                    n_kvt = 4 * n_chunks
                    for half in range(2):
                        gA, gB = 2 * half, 2 * half + 1
                        yA = pd_y.tile([66, 4, 128], f32, tag="yA")
                        yB = pd_y.tile([66, 4, 128], f32, tag="yB")
                        qsA = qT[0:64, 4 * half:4 * half + 4, qsl]
                        qsB = qT[64:128, 4 * half:4 * half + 4, qsl]
                        for c in range(n_chunks):
                            pts = []
                            for i in range(4):
                                ks = slice((4 * c + i) * 128, (4 * c + i + 1) * 128)
                                psAB = pd_s.tile([128, 2, 4, 128], f32, tag="psAB")
                                nc.tensor.matmul(psAB[:, 0, :, :],
                                                 kT[0:64, half, ks], qsA,
                                                 start=True, stop=True,
                                                 tile_position=(0, 0))
                                nc.tensor.matmul(psAB[:, 1, :, :],
                                                 kT[64:128, half, ks], qsB,
                                                 start=True, stop=True,
                                                 tile_position=(64, 0))
                                ptAB = pd_pt.tile([128, 2, 4, 128], bf16, tag="ptAB")
                                nc.scalar.activation(ptAB[:], psAB[:], AF.Exp)
                                if c == t:
                                    mbc = masks[:, t, i, None, None, :].to_broadcast(
                                        [128, 2, 4, 128])
                                    eng = nc.vector if i % 2 == 0 else nc.gpsimd
                                    eng.tensor_tensor(ptAB[:], ptAB[:], mbc, ALU.mult)
                                pts.append(ptAB)
                            for i in range(4):
                                kvt = 4 * c + i
                                nc.tensor.matmul(yA[:], v_all[:, gA, kvt, :],
                                                 pts[i][:, 0, :, :], start=(kvt == 0),
                                                 stop=(kvt == n_kvt - 1))
                                nc.tensor.matmul(yB[:], v_all[:, gB, kvt, :],
                                                 pts[i][:, 1, :, :], start=(kvt == 0),
                                                 stop=(kvt == n_kvt - 1))
                        for g, y in ((gA, yA), (gB, yB)):
                            dsb = pd_sb.tile([2, 4, 128], f32, tag="dsb")
                            nc.vector.tensor_copy(dsb[:], y[64:66, :, :])
                            invs = pd_sb.tile([2, 4, 128], f32, tag="invs")
                            nc.vector.reciprocal_approx_fast(invs[:], dsb[:])
                            invsb = pd_sb.tile([2, 4, 128], bf16, tag="invsb")
                            nc.vector.tensor_copy(invsb[:], invs[:])
                            ysb = pd_sb.tile([64, 4, 128], f32, tag="ysb")
                            nc.vector.tensor_copy(ysb[:], y[0:64, :, :])
                            repy = pd_r.tile([64, 4, 128], f32, tag="repy")
                            nc.tensor.matmul(repy[:].rearrange("p a b -> p (a b)"),
                                             o10b[:, 0:64],
                                             invsb[:].rearrange("p a b -> p (a b)"),
                                             start=True, stop=True)
                            for i in range(4):
                                h = 4 * g + i
                                chunk, part = h // 2, (h % 2) * 64
                                nc.vector.tensor_tensor(
                                    y_all[part:part + 64, chunk, qsl],
                                    ysb[:, i, :], repy[:, i, :], ALU.mult)

        # ------------- Phase E: Wo + post-norm + residual -----------------
        if ph_on("e"):
            with tc.tile_pool(name="pe_sb", bufs=2) as pe_sb, \
                 tc.tile_pool(name="pe_ao", bufs=1) as pe_ao, \
                 tc.tile_pool(name="pe_w", bufs=3) as pe_w, \
                 tc.tile_pool(name="pe_ps", bufs=2, space="PSUM") as pe_ps, \
                 tc.tile_pool(name="pe_ss", bufs=1, space="PSUM") as pe_ss:
                ao = pe_ao.tile([128, 8, OWN], f32, tag="ao")
                ssa = pe_ss.tile([2, OWN], f32, tag="ssa")
                for o in range(8):
                    wos = pe_w.tile([128, 8, 128], bf16, tag="wos")
                    nc.sync.dma_start(wos[:], wo[o])
                    aps = pe_ps.tile([128, OWN], f32, tag="aps")
                    for k in range(8):
                        nc.tensor.matmul(aps[:], wos[:, k, :], y_all[:, k, :],
                                         start=(k == 0), stop=(k == 7))
                    nc.any.tensor_copy(ao[:, o, :], aps[:])
                    a2 = pe_sb.tile([128, OWN], bf16, tag="a2")
                    nc.scalar.activation(a2[:], aps[:], AF.Square)
                    nc.tensor.matmul(ssa[:], ocb[:, 0:2], a2[:],
                                     start=(o == 0), stop=(o == 7))
                rmsa = pe_sb.tile([2, OWN], f32, tag="rmsa")
                nc.scalar.activation(rmsa[:], ssa[0:2, :], AF.Sqrt,
                                     scale=1.0 / D, bias=eps6[0:2, :])
                inva = pe_sb.tile([2, OWN], f32, tag="inva")
                nc.vector.reciprocal_approx_fast(inva[:], rmsa[:])
                repa = pe_ss.tile([128, OWN], f32, tag="repa")
                nc.tensor.matmul(repa[:], o10t[:], inva[:], start=True, stop=True)
                for o in range(8):
                    t1 = pe_sb.tile([128, OWN], f32, tag="t1e")
                    nc.vector.tensor_tensor(t1[:], ao[:, o, :], repa[:], ALU.mult)
                    nc.vector.scalar_tensor_tensor(
                        xpb[:, o, :], t1[:], gat[:, o, None], xrs[:, o, :],
                        ALU.mult, ALU.add)
                    nc.any.tensor_copy(xpr[:, o, :], xpb[:, o, :])
                    nc.vector.tensor_scalar_add(xpb[:, o, :], xpb[:, o, :],
                                                bml[:, o, None])

        # ------------- Phase F: MLP ---------------------------------------
        if ph_on("f"):
            mout = big.tile([128, 8, OWN], f32, tag="xrs_mout")
            with tc.tile_pool(name="pf_h2", bufs=1) as pf_h2, \
                 tc.tile_pool(name="pf_sb", bufs=2) as pf_sb, \
                 tc.tile_pool(name="pf_wf", bufs=3) as pf_wf, \
                 tc.tile_pool(name="pf_wp", bufs=3) as pf_wp, \
                 tc.tile_pool(name="pf_ps", bufs=2, space="PSUM") as pf_ps, \
                 tc.tile_pool(name="pf_mo", bufs=1, space="PSUM") as pf_mo:
                h2 = pf_h2.tile([128, 32, OWN], bf16, tag="h2")
                for hc in range(32):
                    wfs = pf_wf.tile([128, 8, 128], bf16, tag="wfs")
                    nc.sync.dma_start(wfs[:], wfc[hc])
                    hps = pf_ps.tile([128, OWN], f32, tag="hps")
                    for k in range(8):
                        nc.tensor.matmul(hps[:], wfs[:, k, :], xpr[:, k, :],
                                         start=(k == 0), stop=(k == 7))
                    hr = pf_sb.tile([128, OWN], bf16, tag="hr")
                    nc.scalar.activation(hr[:], hps[:], AF.Relu)
                    nc.vector.tensor_tensor(h2[:, hc, :], hr[:], hr[:], ALU.mult)
                ssm = pf_ps.tile([2, OWN], f32, tag="ssm")
                for ohalf in range(2):
                    mo_ps = [pf_mo.tile([128, OWN], f32, name=f"mo{oi}", tag=f"mo{oi}")
                             for oi in range(4)]
                    for hc in range(32):
                        wps = pf_wp.tile([128, 4, 128], bf16, tag="wps")
                        nc.sync.dma_start(wps[:], wprojq[hc, ohalf])
                        for oi in range(4):
                            nc.tensor.matmul(mo_ps[oi][:], wps[:, oi, :], h2[:, hc, :],
                                             start=(hc == 0), stop=(hc == 31))
                    for oi in range(4):
                        o = ohalf * 4 + oi
                        nc.any.tensor_copy(mout[:, o, :], mo_ps[oi][:])
                        m2 = pf_sb.tile([128, OWN], bf16, tag="m2")
                        nc.scalar.activation(m2[:], mo_ps[oi][:], AF.Square)
                        nc.tensor.matmul(ssm[:], ocb[:, 0:2], m2[:],
                                         start=(o == 0), stop=(o == 7))
                rmsm = pf_sb.tile([2, OWN], f32, tag="rmsm")
                nc.scalar.activation(rmsm[:], ssm[0:2, :], AF.Sqrt, scale=1.0 / D,
                                     bias=eps6[0:2, :])
                invm = pf_sb.tile([2, OWN], f32, tag="invm")
                nc.vector.reciprocal_approx_fast(invm[:], rmsm[:])
                repm = pf_ps.tile([128, OWN], f32, tag="hps")
                nc.tensor.matmul(repm[:], o10t[:], invm[:], start=True, stop=True)
                for o in range(8):
                    t1 = pf_sb.tile([128, OWN], f32, tag="t1f")
                    nc.vector.tensor_tensor(t1[:], mout[:, o, :], repm[:], ALU.mult)
                    outv = pf_sb.tile([128, OWN], f32, tag="outv")
                    nc.vector.scalar_tensor_tensor(
                        outv[:], t1[:], gml[:, o, None], xpb[:, o, :],
                        ALU.mult, ALU.add)
                    nc.sync.dma_start(out_t[:, o, :], outv[:])

        if not ph_on("f"):
            with tc.tile_pool(name="dummy", bufs=1) as dp:
                zout = dp.tile([128, 8, OWN], f32, tag="zout")
                nc.vector.memset(zout[:], 0.0)
                nc.sync.dma_start(out_t[:], zout[:])
            rope_stack.close()

    nc.finalize()
    return nc


def _feat_major(a):
    """[F, T] -> device layout [128, F//128, T]."""
    F, T = a.shape
    return np.ascontiguousarray(a.reshape(F // 128, 128, T).transpose(1, 0, 2))


def _vec_dev(v):
    return np.ascontiguousarray(v.reshape(-1, 128).T)


def _bf(a):
    return np.ascontiguousarray(a.astype(ml_dtypes.bfloat16))


_CACHE = {}
_RUN_KW = {}


def kernel(x, attn_norm_w, mlp_norm_w, attn_post_norm_w, mlp_post_norm_w,
           attn_scale, mlp_scale, attn_mod_gain, attn_mod_bias,
           mlp_mod_gain, mlp_mod_bias, Wq, Wk, Wv, Wo, q_gain, fc_w, proj_w):
    x = np.asarray(x, np.float32)
    q_gain = np.asarray(q_gain, np.float32)

    if "nc" not in _CACHE:
        _CACHE["nc"] = build(q_gain)
    nc = _CACHE["nc"]

    anw = np.asarray(attn_norm_w, np.float32)
    mnw = np.asarray(mlp_norm_w, np.float32)
    wq_eff = np.asarray(Wq, np.float32) * anw[None, :]
    wk_eff = np.asarray(Wk, np.float32) * anw[None, :]
    wv_eff = np.asarray(Wv, np.float32) * anw[None, :]
    fc_eff = np.asarray(fc_w, np.float32) * mnw[None, :]

    perm = np.zeros(D, np.int64)
    for p, (a, b) in enumerate(PAIRS):
        perm[p * 128:p * 128 + 64] = np.arange(a * 64, a * 64 + 64)
        perm[p * 128 + 64:(p + 1) * 128] = np.arange(b * 64, b * 64 + 64)
    WqTp = wq_eff.T[:, perm]                                  # [D_in, D_out-perm]
    wq_dev = _bf(np.stack([_feat_major(WqTp[:, p * 128:(p + 1) * 128]) for p in range(8)]))
    wk_dev = _bf(_feat_major(wk_eff.T))
    wv_dev = _bf(_feat_major(wv_eff.T))
    WoT = np.asarray(Wo, np.float32).T
    wo_dev = _bf(np.stack([_feat_major(WoT[:, o * 128:(o + 1) * 128]) for o in range(8)]))
    fcT = fc_eff.T
    wfc_dev = _bf(np.stack([_feat_major(fcT[:, h * 128:(h + 1) * 128]) for h in range(32)]))
    projT = np.asarray(proj_w, np.float32).T                  # [4096, 1024]
    wproj_dev = _bf(np.ascontiguousarray(
        projT.reshape(32, 128, 2, 4, 128).transpose(0, 2, 1, 3, 4)))

    inv_freq = 1.0 / (ROPE_BASE ** (np.arange(0, HD, 2, dtype=np.float32) / HD))
    tpos = np.arange(S, dtype=np.float32)
    freqs = np.outer(tpos, inv_freq).astype(np.float32)
    cosT = np.ascontiguousarray(np.tile(np.cos(freqs).T, (4, 1)))   # [128, S]
    sin1 = np.sin(freqs).T                                          # [32, S]
    sinS = np.ascontiguousarray(
        np.concatenate([sin1, -sin1, sin1, -sin1], axis=0))         # [128, S] sign-folded

    # swap permutation i <-> i^32 (within each 64-wide head)
    permM_h = np.zeros((128, 128), np.float32)
    for i in range(128):
        permM_h[i, i ^ 32] = 1.0

    oc_h_v = np.zeros((128, 2), np.float32)
    oc_h_v[0:64, 0] = 1.0
    oc_h_v[64:128, 1] = 1.0
    selg2_v = np.zeros((2, 8, 128), np.float32)
    for p, (a, b) in enumerate(PAIRS):
        selg2_v[0, p, 0:64] = q_gain[a] / 8.0
        selg2_v[1, p, 64:128] = q_gain[b] / 8.0
    ones10_v = np.concatenate([np.ones((1, 128), np.float32),
                               np.zeros((1, 128), np.float32)])

    gat_v = (np.asarray(attn_post_norm_w, np.float32)
             * np.asarray(attn_mod_gain, np.float32)
             * np.asarray(attn_scale, np.float32))
    bat_v = np.asarray(attn_mod_bias, np.float32) * np.asarray(attn_scale, np.float32)
    gml_v = (np.asarray(mlp_post_norm_w, np.float32)
             * np.asarray(mlp_mod_gain, np.float32)
             * np.asarray(mlp_scale, np.float32))
    bml_v = np.asarray(mlp_mod_bias, np.float32) * np.asarray(mlp_scale, np.float32)

    _bf_selk = _bf(np.stack([np.concatenate([np.ones(64, np.float32), np.zeros(64, np.float32)]),
                          np.concatenate([np.zeros(64, np.float32), np.ones(64, np.float32)])]))
    shared = {
        "wq": wq_dev, "wk": wk_dev, "wv": wv_dev, "wo": wo_dev,
        "wfc": wfc_dev, "wprojq": wproj_dev,
        "cosF": cosT, "sinF": sinS,
        "permM": permM_h,
        "oc_h": _bf(oc_h_v),
        "onescb": _bf(np.ones((128, 2), np.float32)),
        "selg2": _bf(selg2_v),
        "identM": _bf(np.eye(128, dtype=np.float32)),
        "selk": _bf_selk,
        "o10": ones10_v,
        "o10bf": _bf(ones10_v),
        "g_attn": _vec_dev(gat_v), "g_mlp": _vec_dev(gml_v),
        "b_mlp": _vec_dev(bml_v),
    }

    in_maps = []
    owners = []
    for c in range(8):
        b, j = c // 4, c % 4
        rows = np.concatenate(
            [np.arange((j + 4 * t) * 128, (j + 4 * t + 1) * 128) for t in range(4)])
        owners.append((b, rows))
        xb = x[b].T
        x_own = xb[:, rows]
        mask = np.zeros((4, 4, 128, 128), np.float32)
        for t in range(4):
            m = j + 4 * t
            q_idx = m * 128 + np.arange(128)
            for ktl in range(4):
                kv_idx = 512 * t + 128 * ktl + np.arange(128)
                mask[t, ktl] = (kv_idx[:, None] <= q_idx[None, :])
        m_in = {
            "xT": _bf(_feat_major(xb)),
            "xq": _bf(_feat_major(x_own)),
            "xres": _feat_major(x_own + bat_v[:, None]),
            "cosO": np.ascontiguousarray(cosT[:, rows]),
            "sinO": np.ascontiguousarray(sinS[:, rows]),
            "maskM": _bf(np.ascontiguousarray(mask.transpose(2, 0, 1, 3))),
        }
        m_in.update(shared)
        in_maps.append(m_in)

    res = run_bass_kernel_spmd(nc, in_maps, core_ids=list(range(8)),
                               **_RUN_KW)
    _CACHE["last_result"] = res

    out = np.empty((B, S, D), np.float32)
    for c in range(8):
        b, rows = owners[c]
        o = res.results[c]["out"]
        out[b, rows, :] = o.transpose(2, 1, 0).reshape(OWN, D)
    return out


# revision 52
# speedup vs baseline: 1.2897x; 1.0116x over previous
"""Trainium2 Bass kernel for one dense transformer block (B=2, S=2048, D=1024,
16 q-heads / 4 kv-heads GQA, squared-ReLU MLP), data-parallel over 8 NeuronCores.

Sharding: core c = (b, j), b = c // 4, j = c % 4, owns q-token tiles
{j, j+4, j+8, j+12} (128 tokens each) of batch b. K/V are computed for the full
sequence on every core (no collectives). The kv range for own q-tile t is
padded to 512*(t+1); causality enforced with per-core 0/1 masks on the last
512-wide kv chunk.

Numerical identities used (exact up to negligible eps rescaling):
  - per-head q/k rmsnorm is scale-invariant per token, so the block input
    rmsnorm cancels inside it -> Q/K project from raw (norm-weight-folded) x
  - the MLP input rmsnorm cancels through relu()^2 -> proj -> post-rmsnorm
  - V is projected from raw x and rescaled by 1/rms1(x) per token
  - no softmax max-subtraction (logits bounded by |q||k|/8 = 8)
  - softmax denominator = ones-column appended to V in the AV matmul
  - K's 1/rms is applied as a per-partition AP scale inside the exp
    activation (kv tokens are partitions in the score tiles); Q's 1/rms and
    q_gain/8 ride a replicate matmul onto qT

v2 perf changes vs baseline:
  - no DVE reciprocal with f32r destination (was ~7.7ns/elem); all recips are
    fp32->fp32 on DVE, replicates via small fp32 matmuls
  - rope via a feature-swap permutation matmul + 3 full-width DVE ops
    (was 12 narrow DVE ops)
  - bf16 weights + x + V/p/mask/y/h2 paths (half DMA, FWL weight loads,
    2x DVE); q/k/scores stay f32r
"""

import os

import numpy as np
import ml_dtypes

import concourse.bass as bass
from concourse import bacc
import concourse.tile as tile
import concourse.mybir as mybir
from concourse.bass_utils import run_bass_kernel_spmd

f32 = mybir.dt.float32
f32r = mybir.dt.float32r
bf16 = mybir.dt.bfloat16
AF = mybir.ActivationFunctionType
ALU = mybir.AluOpType

B, S, D = 2, 2048, 1024
H, HKV, HD = 16, 4, 64
MLP_HID = 4 * D
KV = HKV * HD
NT = 16
OWN = 512
EPS_BLOCK = 1e-6
EPS_QK = float(np.finfo(np.float32).eps)
ROPE_BASE = 10000.0

PAIRS = [(0, 4), (1, 5), (2, 6), (3, 7), (8, 12), (9, 13), (10, 14), (11, 15)]

PHASE_ORDER = ["ab", "c", "d", "e", "f"]


def build(q_gain):
    max_ph = os.environ.get("KERNEL_PHASES", "f")
    ph_on = lambda p: PHASE_ORDER.index(p) <= PHASE_ORDER.index(max_ph)
    bacc.Bacc.move_matmul_waits_to_ldweights = lambda self: None
    nc = bacc.Bacc(None)

    def dram_in(name, shape, dt):
        return nc.dram_tensor(name, list(shape), dt, kind="ExternalInput")

    xT = dram_in("xT", (128, 8, S), bf16)
    xq = dram_in("xq", (128, 8, OWN), bf16)
    xres = dram_in("xres", (128, 8, OWN), f32)
    wq = dram_in("wq", (8, 128, 8, 128), bf16)
    wk = dram_in("wk", (128, 8, KV), bf16)
    wv = dram_in("wv", (128, 8, KV), bf16)
    wo = dram_in("wo", (8, 128, 8, 128), bf16)
    wfc = dram_in("wfc", (32, 128, 8, 128), bf16)
    wprojq = dram_in("wprojq", (32, 2, 128, 4, 128), bf16)
    cosF = dram_in("cosF", (128, S), bf16)
    sinF = dram_in("sinF", (128, S), bf16)   # sign-folded: +sin rows 0-31/64-95, -sin rows 32-63/96-127
    cosO = dram_in("cosO", (128, OWN), bf16)
    sinO = dram_in("sinO", (128, OWN), bf16)
    maskM = dram_in("maskM", (128, 4, 4, 128), bf16)
    permM = dram_in("permM", (128, 128), bf16)     # swap rows i <-> i^32
    oc_h = dram_in("oc_h", (128, 2), bf16)         # col0: top-64 ones; col1: bottom-64 ones
    onescb = dram_in("onescb", (128, 2), bf16)     # all ones
    selg2 = dram_in("selg2", (2, 8, 128), bf16)     # row0 -> cols 0-63 * gA/8, row1 -> cols 64-127 * gB/8
    identM = dram_in("identM", (128, 128), bf16)   # 128x128 identity
    selk = dram_in("selk", (2, 128), bf16)          # row0 -> cols 0-63 ones, row1 -> cols 64-127 ones
    o10 = dram_in("o10", (2, 128), f32)
    o10bf = dram_in("o10bf", (2, 128), bf16)            # row0 ones, row1 zeros
    g_attn = dram_in("g_attn", (128, 8), f32)
    g_mlp = dram_in("g_mlp", (128, 8), f32)
    b_mlp = dram_in("b_mlp", (128, 8), f32)

    out_t = nc.dram_tensor("out", [128, 8, OWN], f32, kind="ExternalOutput")

    with tile.TileContext(nc) as tc, \
         tc.tile_pool(name="cst", bufs=1) as cst, \
         tc.tile_pool(name="big", bufs=1) as big:
        och = cst.tile([128, 2], bf16, tag="och")
        nc.sync.dma_start(och[:], oc_h[:])
        ocb = cst.tile([128, 2], bf16, tag="ocb")
        nc.sync.dma_start(ocb[:], onescb[:])
        selg = cst.tile([2, 8, 128], bf16, tag="selg")
        nc.sync.dma_start(selg[:], selg2[:])
        idt = cst.tile([128, 128], bf16, tag="idt")
        nc.sync.dma_start(idt[:], identM[:])
        selkt = cst.tile([2, 128], bf16, tag="selkt")
        nc.sync.dma_start(selkt[:], selk[:])
        o10t = cst.tile([2, 128], f32, tag="o10t")
        nc.sync.dma_start(o10t[:], o10[:])
        o10b = cst.tile([2, 128], bf16, tag="o10b")
        nc.sync.dma_start(o10b[:], o10bf[:])
        perm = cst.tile([128, 128], bf16, tag="perm")
        nc.sync.dma_start(perm[:], permM[:])
        eps6 = cst.tile([128, 1], f32, tag="eps6")
        nc.vector.memset(eps6[:], EPS_BLOCK)
        epsq = cst.tile([128, 1], f32, tag="epsq")
        nc.vector.memset(epsq[:], EPS_QK)
        gat = cst.tile([128, 8], f32, tag="gat")
        nc.sync.dma_start(gat[:], g_attn[:])
        gml = cst.tile([128, 8], f32, tag="gml")
        nc.sync.dma_start(gml[:], g_mlp[:])
        bml = cst.tile([128, 8], f32, tag="bml")
        nc.sync.dma_start(bml[:], b_mlp[:])
        from contextlib import ExitStack
        rope_stack = ExitStack()
        ropep = rope_stack.enter_context(tc.tile_pool(name="ropep", bufs=1))
        cosf = ropep.tile([128, S], bf16, tag="cosf")
        sinf = ropep.tile([128, S], bf16, tag="sinf")
        coso = ropep.tile([128, OWN], bf16, tag="coso")
        sino = ropep.tile([128, OWN], bf16, tag="sino")

        kT = big.tile([128, 2, S], f32r, tag="kT")
        v_all = big.tile([128, 4, NT, 66], bf16, tag="v_all")
        qT = big.tile([128, 8, OWN], f32r, tag="qT")
        y_all = big.tile([128, 8, OWN], bf16, tag="y_all")
        xrs = big.tile([128, 8, OWN], f32, tag="xrs_mout")
        invr1 = big.tile([128, NT], f32, tag="invr1")
        rms_st = big.tile([128, NT], f32, tag="rms_st")

        # ------------- Phase AB: rms1, K, V over full sequence ------------
        absub = int(os.environ.get("KERNEL_AB_SUB", "99"))
        if ph_on("ab"):
            with tc.tile_pool(name="pab_x", bufs=3) as pab_x, \
                 tc.tile_pool(name="pab_sb", bufs=2) as pab_sb, \
                 tc.tile_pool(name="pab_w", bufs=1) as pab_w, \
                 tc.tile_pool(name="pab_ps", bufs=2, space="PSUM") as pab_ps, \
                 tc.tile_pool(name="pab_ps1", bufs=1, space="PSUM") as pab_ps1:
                wvs = pab_w.tile([128, 8, KV], bf16, tag="wvs")
                nc.sync.dma_start(wvs[:], wv[:])
                wks = pab_w.tile([128, 8, KV], bf16, tag="wks")
                nc.sync.dma_start(wks[:], wk[:])
                nc.sync.dma_start(cosf[:], cosF[:])
                nc.sync.dma_start(sinf[:], sinF[:])
                nc.sync.dma_start(coso[:], cosO[:])
                nc.sync.dma_start(sino[:], sinO[:])
                nc.sync.dma_start(xrs[:], xres[:])
                for ci in range(4):
                    sl = slice(ci * 512, (ci + 1) * 512)
                    xc = pab_x.tile([128, 8, 512], bf16, tag="xc")
                    nc.sync.dma_start(xc[:], xT[:, :, sl])
                    # token-major sumsq -> invr1 for the 4 token tiles of the chunk
                    for kt in range(4):
                        x2 = pab_sb.tile([128, 8, 128], bf16, tag="x2")
                        nc.scalar.activation(x2[:], xc[:, :, kt * 128:(kt + 1) * 128],
                                             AF.Square)
                        ssp = pab_ps1.tile([128, 2], f32, tag="sstk")
                        for k in range(8):
                            nc.tensor.matmul(ssp[:], x2[:, k, :], ocb[:, 0:2],
                                             start=(k == 0), stop=(k == 7))
                        nc.scalar.activation(rms_st[:, ci * 4 + kt, None], ssp[:, 0:1],
                                             AF.Sqrt, scale=1.0 / D, bias=eps6[:])
                    nc.vector.reciprocal(invr1[:, ci * 4:(ci + 1) * 4],
                                         rms_st[:, ci * 4:(ci + 1) * 4])
                    # V token-major for the 4 token tiles
                    for kt in range(4 if absub >= 2 else 0):
                        gkt = ci * 4 + kt
                        vps = pab_ps.tile([128, KV], f32, tag="vps")
                        for k in range(8):
                            nc.tensor.matmul(vps[:], xc[:, k, kt * 128:(kt + 1) * 128],
                                             wvs[:, k, :], start=(k == 0), stop=(k == 7))
                        nc.vector.tensor_scalar_mul(
                            v_all[:, :, gkt, 0:64],
                            vps[:].rearrange("p (g d) -> p g d", g=4),
                            invr1[:, gkt, None])
                    # K feature-major for both kv pairs
                    for kp in range(2 if absub >= 3 else 0):
                        kps = pab_ps.tile([128, 512], f32, tag="kps")
                        for k in range(8):
                            nc.tensor.matmul(kps[:], wks[:, k, kp * 128:(kp + 1) * 128],
                                             xc[:, k, :], start=(k == 0), stop=(k == 7))
                        kraw = pab_sb.tile([128, 512], bf16, tag="kraw")
                        nc.any.tensor_copy(kraw[:], kps[:])
                        ksw = pab_ps1.tile([128, 512], f32, tag="ksw")
                        nc.tensor.matmul(ksw[:], perm[:], kraw[:], start=True, stop=True)
                        if absub < 4:
                            continue
                        k2 = pab_sb.tile([128, 512], bf16, tag="k2")
                        nc.scalar.activation(k2[:], kps[:], AF.Square)
                        # per-token sumsq token-major, then transpose to row-major
                        sstk = pab_ps1.tile([128, 4, 2], f32, tag="sstk")
                        for kt in range(4):
                            ksl = slice(kt * 128, (kt + 1) * 128)
                            nc.tensor.matmul(sstk[:, kt, :], k2[:, ksl], och[:, 0:2],
                                             start=True, stop=True)
                        sstk_sb = pab_sb.tile([128, 4, 2], bf16, tag="sstk_sb")
                        nc.any.tensor_copy(sstk_sb[:], sstk[:])
                        ssrow = pab_ps1.tile([2, 512], f32, tag="ssrow")
                        for kt in range(4):
                            ksl = slice(kt * 128, (kt + 1) * 128)
                            nc.tensor.matmul(ssrow[0:2, ksl], sstk_sb[:, kt, :],
                                             idt[:], start=True, stop=True)
                        rmsk = pab_sb.tile([2, 512], f32, tag="rmsk")
                        nc.scalar.activation(rmsk[:], ssrow[0:2, :], AF.Sqrt,
                                             scale=1.0 / HD, bias=epsq[0:2, :])
                        invk = pab_sb.tile([2, 512], f32, tag="invk")
                        nc.vector.reciprocal_approx_fast(invk[:], rmsk[:])
                        invkb = pab_sb.tile([2, 512], bf16, tag="invkb")
                        nc.vector.tensor_copy(invkb[:], invk[:])
                        repk = pab_ps1.tile([128, 512], f32, tag="repk")
                        nc.tensor.matmul(repk[:], selkt[:], invkb[:],
                                         start=True, stop=True)
                        if absub < 5:
                            continue
                        t1 = pab_sb.tile([128, 512], bf16, tag="t1k")
                        nc.vector.tensor_tensor(t1[:], kraw[:], cosf[:, sl], ALU.mult)
                        t2 = pab_sb.tile([128, 512], bf16, tag="t2k")
                        nc.vector.tensor_tensor(t2[:], ksw[:], sinf[:, sl], ALU.mult)
                        t3 = pab_sb.tile([128, 512], bf16, tag="t3k")
                        nc.vector.tensor_tensor(t3[:], t1[:], t2[:], ALU.add)
                        nc.vector.tensor_tensor(kT[:, kp, sl], t3[:], repk[:], ALU.mult)
                # ones column of V
                nc.vector.tensor_copy(
                    v_all[:, :, :, 64:66],
                    ocb[:, 0, None, None].to_broadcast([128, 4, NT, 2]))

        # ------------- Phase C: Q for own tokens --------------------------
        if ph_on("c"):
            with tc.tile_pool(name="pc_x", bufs=1) as pc_x, \
                 tc.tile_pool(name="pc_sb", bufs=3) as pc_sb, \
                 tc.tile_pool(name="pc_w", bufs=3) as pc_w, \
                 tc.tile_pool(name="pc_ps", bufs=2, space="PSUM") as pc_ps, \
                 tc.tile_pool(name="pc_ps1", bufs=1, space="PSUM") as pc_ps1:
                xqs = pc_x.tile([128, 8, OWN], bf16, tag="xqs")
                nc.sync.dma_start(xqs[:], xq[:])
                for p in range(8):
                    wqs = pc_w.tile([128, 8, 128], bf16, tag="wqs")
                    nc.sync.dma_start(wqs[:], wq[p])
                    qps = pc_ps.tile([128, OWN], f32, tag="qps")
                    for k in range(8):
                        nc.tensor.matmul(qps[:], wqs[:, k, :], xqs[:, k, :],
                                         start=(k == 0), stop=(k == 7))
                    qraw = pc_sb.tile([128, OWN], bf16, tag="qraw")
                    nc.any.tensor_copy(qraw[:], qps[:])
                    qsw = pc_ps.tile([128, OWN], f32, tag="qsw")
                    nc.tensor.matmul(qsw[:], perm[:], qraw[:], start=True, stop=True)
                    q2 = pc_sb.tile([128, OWN], bf16, tag="q2")
                    nc.scalar.activation(q2[:], qps[:], AF.Square)
                    sstq = pc_ps1.tile([128, 4, 2], f32, tag="sstq")
                    for kt in range(4):
                        ksl = slice(kt * 128, (kt + 1) * 128)
                        nc.tensor.matmul(sstq[:, kt, :], q2[:, ksl], och[:, 0:2],
                                         start=True, stop=True)
                    sstq_sb = pc_sb.tile([128, 4, 2], bf16, tag="sstq_sb")
                    nc.any.tensor_copy(sstq_sb[:], sstq[:])
                    ssqrow = pc_ps1.tile([2, OWN], f32, tag="ssqrow")
                    for kt in range(4):
                        ksl = slice(kt * 128, (kt + 1) * 128)
                        nc.tensor.matmul(ssqrow[0:2, ksl], sstq_sb[:, kt, :],
                                         idt[:], start=True, stop=True)
                    rmsq = pc_sb.tile([2, OWN], f32, tag="rmsq")
                    nc.scalar.activation(rmsq[:], ssqrow[0:2, :], AF.Sqrt,
                                         scale=1.0 / HD, bias=epsq[0:2, :])
                    invq = pc_sb.tile([2, OWN], f32, tag="invq")
                    nc.vector.reciprocal_approx_fast(invq[:], rmsq[:])
                    invqb = pc_sb.tile([2, OWN], bf16, tag="invqb")
                    nc.vector.tensor_copy(invqb[:], invq[:])
                    repq = pc_ps1.tile([128, OWN], f32, tag="repq")
                    nc.tensor.matmul(repq[:], selg[:, p, :], invqb[:],
                                     start=True, stop=True)
                    t1 = pc_sb.tile([128, OWN], bf16, tag="t1q")
                    nc.vector.tensor_tensor(t1[:], qraw[:], coso[:], ALU.mult)
                    t2 = pc_sb.tile([128, OWN], bf16, tag="t2q")
                    nc.vector.tensor_tensor(t2[:], qsw[:], sino[:], ALU.mult)
                    t3 = pc_sb.tile([128, OWN], bf16, tag="t3q")
                    nc.vector.tensor_tensor(t3[:], t1[:], t2[:], ALU.add)
                    nc.vector.tensor_tensor(qT[:, p, :], t3[:], repq[:], ALU.mult)

            rope_stack.close()

        # ------------- Phase D: attention ---------------------------------
        if ph_on("d"):
            xpr = big.tile([128, 8, OWN], bf16, tag="xpr")
            xpb = big.tile([128, 8, OWN], f32, tag="xpb")
            with tc.tile_pool(name="pd_m", bufs=1) as pd_m, \
                 tc.tile_pool(name="pd_pt", bufs=6) as pd_pt, \
                 tc.tile_pool(name="pd_sb", bufs=2) as pd_sb, \
                 tc.tile_pool(name="pd_s", bufs=2, space="PSUM") as pd_s, \
                 tc.tile_pool(name="pd_y", bufs=1, space="PSUM") as pd_y, \
                 tc.tile_pool(name="pd_r", bufs=1, space="PSUM") as pd_r:
                masks = pd_m.tile([128, 4, 4, 128], bf16, tag="masks")
                nc.sync.dma_start(masks[:], maskM[:])
                for t in range(4):
                    qsl = slice(t * 128, (t + 1) * 128)
                    n_chunks = t + 1
                    n_kvt = 4 * n_chunks
                    for half in range(2):
                        gA, gB = 2 * half, 2 * half + 1
                        yA = pd_y.tile([66, 4, 128], f32, tag="yA")
                        yB = pd_y.tile([66, 4, 128], f32, tag="yB")
                        qsA = qT[0:64, 4 * half:4 * half + 4, qsl]
                        qsB = qT[64:128, 4 * half:4 * half + 4, qsl]
                        for c in range(n_chunks):
                            pts = []
                            for i in range(4):
                                ks = slice((4 * c + i) * 128, (4 * c + i + 1) * 128)
                                psAB = pd_s.tile([128, 2, 4, 128], f32, tag="psAB")
                                nc.tensor.matmul(psAB[:, 0, :, :],
                                                 kT[0:64, half, ks], qsA,
                                                 start=True, stop=True,
                                                 tile_position=(0, 0))
                                nc.tensor.matmul(psAB[:, 1, :, :],
                                                 kT[64:128, half, ks], qsB,
                                                 start=True, stop=True,
                                                 tile_position=(64, 0))
                                ptAB = pd_pt.tile([128, 2, 4, 128], bf16, tag="ptAB")
                                nc.scalar.activation(ptAB[:], psAB[:], AF.Exp)
                                if c == t:
                                    mbc = masks[:, t, i, None, None, :].to_broadcast(
                                        [128, 2, 4, 128])
                                    eng = nc.vector if i % 2 == 0 else nc.gpsimd
                                    eng.tensor_tensor(ptAB[:], ptAB[:], mbc, ALU.mult)
                                pts.append(ptAB)
                            for i in range(4):
                                kvt = 4 * c + i
                                nc.tensor.matmul(yA[:], v_all[:, gA, kvt, :],
                                                 pts[i][:, 0, :, :], start=(kvt == 0),
                                                 stop=(kvt == n_kvt - 1))
                                nc.tensor.matmul(yB[:], v_all[:, gB, kvt, :],
                                                 pts[i][:, 1, :, :], start=(kvt == 0),
                                                 stop=(kvt == n_kvt - 1))
                        for g, y in ((gA, yA), (gB, yB)):
                            dsb = pd_sb.tile([2, 4, 128], f32, tag="dsb")
                            nc.vector.tensor_copy(dsb[:], y[64:66, :, :])
                            invs = pd_sb.tile([2, 4, 128], f32, tag="invs")
                            nc.vector.reciprocal_approx_fast(invs[:], dsb[:])
                            invsb = pd_sb.tile([2, 4, 128], bf16, tag="invsb")
                            nc.vector.tensor_copy(invsb[:], invs[:])
                            ysb = pd_sb.tile([64, 4, 128], f32, tag="ysb")
                            nc.vector.tensor_copy(ysb[:], y[0:64, :, :])
                            repy = pd_r.tile([64, 4, 128], f32, tag="repy")
                            nc.tensor.matmul(repy[:].rearrange("p a b -> p (a b)"),
                                             o10b[:, 0:64],
                                             invsb[:].rearrange("p a b -> p (a b)"),
                                             start=True, stop=True)
                            for i in range(4):
                                h = 4 * g + i
                                chunk, part = h // 2, (h % 2) * 64
                                nc.vector.tensor_tensor(
                                    y_all[part:part + 64, chunk, qsl],
                                    ysb[:, i, :], repy[:, i, :], ALU.mult)

        # ------------- Phase E: Wo + post-norm + residual -----------------
        if ph_on("e"):
            with tc.tile_pool(name="pe_sb", bufs=2) as pe_sb, \
                 tc.tile_pool(name="pe_ao", bufs=1) as pe_ao, \
                 tc.tile_pool(name="pe_w", bufs=3) as pe_w, \
                 tc.tile_pool(name="pe_ps", bufs=2, space="PSUM") as pe_ps, \
                 tc.tile_pool(name="pe_ss", bufs=1, space="PSUM") as pe_ss:
                ao = pe_ao.tile([128, 8, OWN], f32, tag="ao")
                ssa = pe_ss.tile([2, OWN], f32, tag="ssa")
                for o in range(8):
                    wos = pe_w.tile([128, 8, 128], bf16, tag="wos")
                    nc.sync.dma_start(wos[:], wo[o])
                    aps = pe_ps.tile([128, OWN], f32, tag="aps")
                    for k in range(8):
                        nc.tensor.matmul(aps[:], wos[:, k, :], y_all[:, k, :],
                                         start=(k == 0), stop=(k == 7))
                    nc.any.tensor_copy(ao[:, o, :], aps[:])
                    a2 = pe_sb.tile([128, OWN], bf16, tag="a2")
                    nc.scalar.activation(a2[:], aps[:], AF.Square)
                    nc.tensor.matmul(ssa[:], ocb[:, 0:2], a2[:],
                                     start=(o == 0), stop=(o == 7))
                rmsa = pe_sb.tile([2, OWN], f32, tag="rmsa")
                nc.scalar.activation(rmsa[:], ssa[0:2, :], AF.Sqrt,
                                     scale=1.0 / D, bias=eps6[0:2, :])
                inva = pe_sb.tile([2, OWN], f32, tag="inva")
                nc.vector.reciprocal_approx_fast(inva[:], rmsa[:])
                repa = pe_ss.tile([128, OWN], f32, tag="repa")
                nc.tensor.matmul(repa[:], o10t[:], inva[:], start=True, stop=True)
                for o in range(8):
                    t1 = pe_sb.tile([128, OWN], f32, tag="t1e")
                    nc.vector.tensor_tensor(t1[:], ao[:, o, :], repa[:], ALU.mult)
                    nc.vector.scalar_tensor_tensor(
                        xpb[:, o, :], t1[:], gat[:, o, None], xrs[:, o, :],
                        ALU.mult, ALU.add)
                    nc.any.tensor_copy(xpr[:, o, :], xpb[:, o, :])
                    nc.vector.tensor_scalar_add(xpb[:, o, :], xpb[:, o, :],
                                                bml[:, o, None])

        # ------------- Phase F: MLP ---------------------------------------
        if ph_on("f"):
            mout = big.tile([128, 8, OWN], f32, tag="xrs_mout")
            with tc.tile_pool(name="pf_h2", bufs=1) as pf_h2, \
                 tc.tile_pool(name="pf_sb", bufs=2) as pf_sb, \
                 tc.tile_pool(name="pf_wf", bufs=3) as pf_wf, \
                 tc.tile_pool(name="pf_wp", bufs=3) as pf_wp, \
                 tc.tile_pool(name="pf_ps", bufs=2, space="PSUM") as pf_ps, \
                 tc.tile_pool(name="pf_mo", bufs=1, space="PSUM") as pf_mo:
                h2 = pf_h2.tile([128, 32, OWN], bf16, tag="h2")
                for hc in range(32):
                    wfs = pf_wf.tile([128, 8, 128], bf16, tag="wfs")
                    nc.sync.dma_start(wfs[:], wfc[hc])
                    hps = pf_ps.tile([128, OWN], f32, tag="hps")
                    for k in range(8):
                        nc.tensor.matmul(hps[:], wfs[:, k, :], xpr[:, k, :],
                                         start=(k == 0), stop=(k == 7))
                    hr = pf_sb.tile([128, OWN], bf16, tag="hr")
                    nc.scalar.activation(hr[:], hps[:], AF.Relu)
                    nc.vector.tensor_tensor(h2[:, hc, :], hr[:], hr[:], ALU.mult)
                ssm = pf_ps.tile([2, OWN], f32, tag="ssm")
                for ohalf in range(2):
                    mo_ps = [pf_mo.tile([128, OWN], f32, name=f"mo{oi}", tag=f"mo{oi}")
                             for oi in range(4)]
                    for hc in range(32):
                        wps = pf_wp.tile([128, 4, 128], bf16, tag="wps")
                        nc.sync.dma_start(wps[:], wprojq[hc, ohalf])
                        for oi in range(4):
                            nc.tensor.matmul(mo_ps[oi][:], wps[:, oi, :], h2[:, hc, :],
                                             start=(hc == 0), stop=(hc == 31))
                    for oi in range(4):
                        o = ohalf * 4 + oi
                        nc.any.tensor_copy(mout[:, o, :], mo_ps[oi][:])
                        m2 = pf_sb.tile([128, OWN], bf16, tag="m2")
                        nc.scalar.activation(m2[:], mo_ps[oi][:], AF.Square)
                        nc.tensor.matmul(ssm[:], ocb[:, 0:2], m2[:],
                                         start=(o == 0), stop=(o == 7))
                rmsm = pf_sb.tile([2, OWN], f32, tag="rmsm")
                nc.scalar.activation(rmsm[:], ssm[0:2, :], AF.Sqrt, scale=1.0 / D,
                                     bias=eps6[0:2, :])
                invm = pf_sb.tile([2, OWN], f32, tag="invm")
                nc.vector.reciprocal_approx_fast(invm[:], rmsm[:])
                repm = pf_ps.tile([128, OWN], f32, tag="hps")
                nc.tensor.matmul(repm[:], o10t[:], invm[:], start=True, stop=True)
                for o in range(8):
                    t1 = pf_sb.tile([128, OWN], f32, tag="t1f")
                    nc.vector.tensor_tensor(t1[:], mout[:, o, :], repm[:], ALU.mult)
                    outv = pf_sb.tile([128, OWN], f32, tag="outv")
                    nc.vector.scalar_tensor_tensor(
                        outv[:], t1[:], gml[:, o, None], xpb[:, o, :],
                        ALU.mult, ALU.add)
                    nc.sync.dma_start(out_t[:, o, :], outv[:])

        if not ph_on("f"):
            with tc.tile_pool(name="dummy", bufs=1) as dp:
                zout = dp.tile([128, 8, OWN], f32, tag="zout")
                nc.vector.memset(zout[:], 0.0)
                nc.sync.dma_start(out_t[:], zout[:])
            rope_stack.close()

    nc.finalize()
    return nc


def _feat_major(a):
    """[F, T] -> device layout [128, F//128, T]."""
    F, T = a.shape
    return np.ascontiguousarray(a.reshape(F // 128, 128, T).transpose(1, 0, 2))


def _vec_dev(v):
    return np.ascontiguousarray(v.reshape(-1, 128).T)


def _bf(a):
    return np.ascontiguousarray(a.astype(ml_dtypes.bfloat16))


_CACHE = {}
_RUN_KW = {}


def kernel(x, attn_norm_w, mlp_norm_w, attn_post_norm_w, mlp_post_norm_w,
           attn_scale, mlp_scale, attn_mod_gain, attn_mod_bias,
           mlp_mod_gain, mlp_mod_bias, Wq, Wk, Wv, Wo, q_gain, fc_w, proj_w):
    x = np.asarray(x, np.float32)
    q_gain = np.asarray(q_gain, np.float32)

    if "nc" not in _CACHE:
        _CACHE["nc"] = build(q_gain)
    nc = _CACHE["nc"]

    anw = np.asarray(attn_norm_w, np.float32)
    mnw = np.asarray(mlp_norm_w, np.float32)
    wq_eff = np.asarray(Wq, np.float32) * anw[None, :]
    wk_eff = np.asarray(Wk, np.float32) * anw[None, :]
    wv_eff = np.asarray(Wv, np.float32) * anw[None, :]
    fc_eff = np.asarray(fc_w, np.float32) * mnw[None, :]

    perm = np.zeros(D, np.int64)
    for p, (a, b) in enumerate(PAIRS):
        perm[p * 128:p * 128 + 64] = np.arange(a * 64, a * 64 + 64)
        perm[p * 128 + 64:(p + 1) * 128] = np.arange(b * 64, b * 64 + 64)
    WqTp = wq_eff.T[:, perm]                                  # [D_in, D_out-perm]
    wq_dev = _bf(np.stack([_feat_major(WqTp[:, p * 128:(p + 1) * 128]) for p in range(8)]))
    wk_dev = _bf(_feat_major(wk_eff.T))
    wv_dev = _bf(_feat_major(wv_eff.T))
    WoT = np.asarray(Wo, np.float32).T
    wo_dev = _bf(np.stack([_feat_major(WoT[:, o * 128:(o + 1) * 128]) for o in range(8)]))
    fcT = fc_eff.T
    wfc_dev = _bf(np.stack([_feat_major(fcT[:, h * 128:(h + 1) * 128]) for h in range(32)]))
    projT = np.asarray(proj_w, np.float32).T                  # [4096, 1024]
    wproj_dev = _bf(np.ascontiguousarray(
        projT.reshape(32, 128, 2, 4, 128).transpose(0, 2, 1, 3, 4)))

    inv_freq = 1.0 / (ROPE_BASE ** (np.arange(0, HD, 2, dtype=np.float32) / HD))
    tpos = np.arange(S, dtype=np.float32)
    freqs = np.outer(tpos, inv_freq).astype(np.float32)
    cosT = np.ascontiguousarray(np.tile(np.cos(freqs).T, (4, 1)))   # [128, S]
    sin1 = np.sin(freqs).T                                          # [32, S]
    sinS = np.ascontiguousarray(
        np.concatenate([sin1, -sin1, sin1, -sin1], axis=0))         # [128, S] sign-folded

    # swap permutation i <-> i^32 (within each 64-wide head)
    permM_h = np.zeros((128, 128), np.float32)
    for i in range(128):
        permM_h[i, i ^ 32] = 1.0

    oc_h_v = np.zeros((128, 2), np.float32)
    oc_h_v[0:64, 0] = 1.0
    oc_h_v[64:128, 1] = 1.0
    selg2_v = np.zeros((2, 8, 128), np.float32)
    for p, (a, b) in enumerate(PAIRS):
        selg2_v[0, p, 0:64] = q_gain[a] / 8.0
        selg2_v[1, p, 64:128] = q_gain[b] / 8.0
    ones10_v = np.concatenate([np.ones((1, 128), np.float32),
                               np.zeros((1, 128), np.float32)])

    gat_v = (np.asarray(attn_post_norm_w, np.float32)
             * np.asarray(attn_mod_gain, np.float32)
             * np.asarray(attn_scale, np.float32))
    bat_v = np.asarray(attn_mod_bias, np.float32) * np.asarray(attn_scale, np.float32)
    gml_v = (np.asarray(mlp_post_norm_w, np.float32)
             * np.asarray(mlp_mod_gain, np.float32)
             * np.asarray(mlp_scale, np.float32))
    bml_v = np.asarray(mlp_mod_bias, np.float32) * np.asarray(mlp_scale, np.float32)

    _bf_selk = _bf(np.stack([np.concatenate([np.ones(64, np.float32), np.zeros(64, np.float32)]),
                          np.concatenate([np.zeros(64, np.float32), np.ones(64, np.float32)])]))
    shared = {
        "wq": wq_dev, "wk": wk_dev, "wv": wv_dev, "wo": wo_dev,
        "wfc": wfc_dev, "wprojq": wproj_dev,
        "cosF": _bf(cosT), "sinF": _bf(sinS),
        "permM": _bf(permM_h),
        "oc_h": _bf(oc_h_v),
        "onescb": _bf(np.ones((128, 2), np.float32)),
        "selg2": _bf(selg2_v),
        "identM": _bf(np.eye(128, dtype=np.float32)),
        "selk": _bf_selk,
        "o10": ones10_v,
        "o10bf": _bf(ones10_v),
        "g_attn": _vec_dev(gat_v), "g_mlp": _vec_dev(gml_v),
        "b_mlp": _vec_dev(bml_v),
    }

    in_maps = []
    owners = []
    for c in range(8):
        b, j = c // 4, c % 4
        rows = np.concatenate(
            [np.arange((j + 4 * t) * 128, (j + 4 * t + 1) * 128) for t in range(4)])
        owners.append((b, rows))
        xb = x[b].T
        x_own = xb[:, rows]
        mask = np.zeros((4, 4, 128, 128), np.float32)
        for t in range(4):
            m = j + 4 * t
            q_idx = m * 128 + np.arange(128)
            for ktl in range(4):
                kv_idx = 512 * t + 128 * ktl + np.arange(128)
                mask[t, ktl] = (kv_idx[:, None] <= q_idx[None, :])
        m_in = {
            "xT": _bf(_feat_major(xb)),
            "xq": _bf(_feat_major(x_own)),
            "xres": _feat_major(x_own + bat_v[:, None]),
            "cosO": _bf(cosT[:, rows]),
            "sinO": _bf(sinS[:, rows]),
            "maskM": _bf(np.ascontiguousarray(mask.transpose(2, 0, 1, 3))),
        }
        m_in.update(shared)
        in_maps.append(m_in)

    res = run_bass_kernel_spmd(nc, in_maps, core_ids=list(range(8)),
                               **_RUN_KW)
    _CACHE["last_result"] = res

    out = np.empty((B, S, D), np.float32)
    for c in range(8):
        b, rows = owners[c]
        o = res.results[c]["out"]
        out[b, rows, :] = o.transpose(2, 1, 0).reshape(OWN, D)
    return out


# revision 53
# speedup vs baseline: 1.3108x; 1.0163x over previous
"""Trainium2 Bass kernel for one dense transformer block (B=2, S=2048, D=1024,
16 q-heads / 4 kv-heads GQA, squared-ReLU MLP), data-parallel over 8 NeuronCores.

Sharding: core c = (b, j), b = c // 4, j = c % 4, owns q-token tiles
{j, j+4, j+8, j+12} (128 tokens each) of batch b. K/V are computed for the full
sequence on every core (no collectives). The kv range for own q-tile t is
padded to 512*(t+1); causality enforced with per-core 0/1 masks on the last
512-wide kv chunk.

Numerical identities used (exact up to negligible eps rescaling):
  - per-head q/k rmsnorm is scale-invariant per token, so the block input
    rmsnorm cancels inside it -> Q/K project from raw (norm-weight-folded) x
  - the MLP input rmsnorm cancels through relu()^2 -> proj -> post-rmsnorm
  - V is projected from raw x and rescaled by 1/rms1(x) per token
  - no softmax max-subtraction (logits bounded by |q||k|/8 = 8)
  - softmax denominator = ones-column appended to V in the AV matmul
  - K's 1/rms is applied as a per-partition AP scale inside the exp
    activation (kv tokens are partitions in the score tiles); Q's 1/rms and
    q_gain/8 ride a replicate matmul onto qT

v2 perf changes vs baseline:
  - no DVE reciprocal with f32r destination (was ~7.7ns/elem); all recips are
    fp32->fp32 on DVE, replicates via small fp32 matmuls
  - rope via a feature-swap permutation matmul + 3 full-width DVE ops
    (was 12 narrow DVE ops)
  - bf16 weights + x + V/p/mask/y/h2 paths (half DMA, FWL weight loads,
    2x DVE); q/k/scores stay f32r
"""

import os

import numpy as np
import ml_dtypes

import concourse.bass as bass
from concourse import bacc
import concourse.tile as tile
import concourse.mybir as mybir
from concourse.bass_utils import run_bass_kernel_spmd

f32 = mybir.dt.float32
f32r = mybir.dt.float32r
bf16 = mybir.dt.bfloat16
AF = mybir.ActivationFunctionType
ALU = mybir.AluOpType

B, S, D = 2, 2048, 1024
H, HKV, HD = 16, 4, 64
MLP_HID = 4 * D
KV = HKV * HD
NT = 16
OWN = 512
EPS_BLOCK = 1e-6
EPS_QK = float(np.finfo(np.float32).eps)
ROPE_BASE = 10000.0

PAIRS = [(0, 4), (1, 5), (2, 6), (3, 7), (8, 12), (9, 13), (10, 14), (11, 15)]

PHASE_ORDER = ["ab", "c", "d", "e", "f"]


def build(q_gain):
    max_ph = os.environ.get("KERNEL_PHASES", "f")
    ph_on = lambda p: PHASE_ORDER.index(p) <= PHASE_ORDER.index(max_ph)
    bacc.Bacc.move_matmul_waits_to_ldweights = lambda self: None
    nc = bacc.Bacc(None)

    def dram_in(name, shape, dt):
        return nc.dram_tensor(name, list(shape), dt, kind="ExternalInput")

    xT = dram_in("xT", (128, 8, S), bf16)
    xq = dram_in("xq", (128, 8, OWN), bf16)
    xres = dram_in("xres", (128, 8, OWN), f32)
    wq = dram_in("wq", (8, 128, 8, 128), bf16)
    wk = dram_in("wk", (128, 8, KV), bf16)
    wv = dram_in("wv", (128, 8, KV), bf16)
    wo = dram_in("wo", (8, 128, 8, 128), bf16)
    wfc = dram_in("wfc", (32, 128, 8, 128), bf16)
    wprojq = dram_in("wprojq", (32, 2, 128, 4, 128), bf16)
    cosF = dram_in("cosF", (128, S), bf16)
    sinF = dram_in("sinF", (128, S), bf16)   # sign-folded: +sin rows 0-31/64-95, -sin rows 32-63/96-127
    cosO = dram_in("cosO", (128, OWN), bf16)
    sinO = dram_in("sinO", (128, OWN), bf16)
    maskM = dram_in("maskM", (128, 4, 4, 128), bf16)
    permM = dram_in("permM", (128, 128), bf16)     # swap rows i <-> i^32
    oc_h = dram_in("oc_h", (128, 2), bf16)         # col0: top-64 ones; col1: bottom-64 ones
    onescb = dram_in("onescb", (128, 2), bf16)     # all ones
    selg2 = dram_in("selg2", (2, 8, 128), bf16)     # row0 -> cols 0-63 * gA/8, row1 -> cols 64-127 * gB/8
    identM = dram_in("identM", (128, 128), bf16)   # 128x128 identity
    selk = dram_in("selk", (2, 128), bf16)          # row0 -> cols 0-63 ones, row1 -> cols 64-127 ones
    o10 = dram_in("o10", (2, 128), f32)
    o10bf = dram_in("o10bf", (2, 128), bf16)            # row0 ones, row1 zeros
    g_attn = dram_in("g_attn", (128, 8), f32)
    g_mlp = dram_in("g_mlp", (128, 8), f32)
    b_mlp = dram_in("b_mlp", (128, 8), f32)

    out_t = nc.dram_tensor("out", [128, 8, OWN], f32, kind="ExternalOutput")

    with tile.TileContext(nc) as tc, \
         tc.tile_pool(name="cst", bufs=1) as cst, \
         tc.tile_pool(name="big", bufs=1) as big:
        och = cst.tile([128, 2], bf16, tag="och")
        nc.sync.dma_start(och[:], oc_h[:])
        ocb = cst.tile([128, 2], bf16, tag="ocb")
        nc.sync.dma_start(ocb[:], onescb[:])
        selg = cst.tile([2, 8, 128], bf16, tag="selg")
        idt = cst.tile([128, 128], bf16, tag="idt")
        selkt = cst.tile([2, 128], bf16, tag="selkt")
        o10t = cst.tile([2, 128], f32, tag="o10t")
        o10b = cst.tile([2, 128], bf16, tag="o10b")
        perm = cst.tile([128, 128], bf16, tag="perm")
        nc.sync.dma_start(perm[:], permM[:])
        eps6 = cst.tile([128, 1], f32, tag="eps6")
        nc.vector.memset(eps6[:], EPS_BLOCK)
        epsq = cst.tile([128, 1], f32, tag="epsq")
        nc.vector.memset(epsq[:], EPS_QK)
        gat = cst.tile([128, 8], f32, tag="gat")
        gml = cst.tile([128, 8], f32, tag="gml")
        bml = cst.tile([128, 8], f32, tag="bml")
        from contextlib import ExitStack
        rope_stack = ExitStack()
        ropep = rope_stack.enter_context(tc.tile_pool(name="ropep", bufs=1))
        cosf = ropep.tile([128, S], bf16, tag="cosf")
        sinf = ropep.tile([128, S], bf16, tag="sinf")
        coso = ropep.tile([128, OWN], bf16, tag="coso")
        sino = ropep.tile([128, OWN], bf16, tag="sino")

        kT = big.tile([128, 2, S], f32r, tag="kT")
        v_all = big.tile([128, 4, NT, 66], bf16, tag="v_all")
        qT = big.tile([128, 8, OWN], f32r, tag="qT")
        y_all = big.tile([128, 8, OWN], bf16, tag="y_all")
        xrs = big.tile([128, 8, OWN], f32, tag="xrs_mout")
        invr1 = big.tile([128, NT], f32, tag="invr1")
        rms_st = big.tile([128, NT], f32, tag="rms_st")

        # ------------- Phase AB: rms1, K, V over full sequence ------------
        absub = int(os.environ.get("KERNEL_AB_SUB", "99"))
        if ph_on("ab"):
            with tc.tile_pool(name="pab_x", bufs=3) as pab_x, \
                 tc.tile_pool(name="pab_sb", bufs=2) as pab_sb, \
                 tc.tile_pool(name="pab_w", bufs=1) as pab_w, \
                 tc.tile_pool(name="pab_ps", bufs=2, space="PSUM") as pab_ps, \
                 tc.tile_pool(name="pab_ps1", bufs=1, space="PSUM") as pab_ps1:
                wvs = pab_w.tile([128, 8, KV], bf16, tag="wvs")
                nc.sync.dma_start(wvs[:], wv[:])
                wks = pab_w.tile([128, 8, KV], bf16, tag="wks")
                nc.sync.dma_start(wks[:], wk[:])
                nc.sync.dma_start(idt[:], identM[:])
                nc.sync.dma_start(selkt[:], selk[:])
                nc.sync.dma_start(cosf[:], cosF[:])
                nc.sync.dma_start(sinf[:], sinF[:])
                nc.sync.dma_start(coso[:], cosO[:])
                nc.sync.dma_start(sino[:], sinO[:])
                nc.sync.dma_start(xrs[:], xres[:])
                for ci in range(4):
                    sl = slice(ci * 512, (ci + 1) * 512)
                    xc = pab_x.tile([128, 8, 512], bf16, tag="xc")
                    nc.sync.dma_start(xc[:], xT[:, :, sl])
                    # token-major sumsq -> invr1 for the 4 token tiles of the chunk
                    for kt in range(4):
                        x2 = pab_sb.tile([128, 8, 128], bf16, tag="x2")
                        nc.scalar.activation(x2[:], xc[:, :, kt * 128:(kt + 1) * 128],
                                             AF.Square)
                        ssp = pab_ps1.tile([128, 2], f32, tag="sstk")
                        for k in range(8):
                            nc.tensor.matmul(ssp[:], x2[:, k, :], ocb[:, 0:2],
                                             start=(k == 0), stop=(k == 7))
                        nc.scalar.activation(rms_st[:, ci * 4 + kt, None], ssp[:, 0:1],
                                             AF.Sqrt, scale=1.0 / D, bias=eps6[:])
                    nc.vector.reciprocal(invr1[:, ci * 4:(ci + 1) * 4],
                                         rms_st[:, ci * 4:(ci + 1) * 4])
                    # V token-major for the 4 token tiles
                    for kt in range(4 if absub >= 2 else 0):
                        gkt = ci * 4 + kt
                        vps = pab_ps.tile([128, KV], f32, tag="vps")
                        for k in range(8):
                            nc.tensor.matmul(vps[:], xc[:, k, kt * 128:(kt + 1) * 128],
                                             wvs[:, k, :], start=(k == 0), stop=(k == 7))
                        nc.vector.tensor_scalar_mul(
                            v_all[:, :, gkt, 0:64],
                            vps[:].rearrange("p (g d) -> p g d", g=4),
                            invr1[:, gkt, None])
                    # K feature-major for both kv pairs
                    for kp in range(2 if absub >= 3 else 0):
                        kps = pab_ps.tile([128, 512], f32, tag="kps")
                        for k in range(8):
                            nc.tensor.matmul(kps[:], wks[:, k, kp * 128:(kp + 1) * 128],
                                             xc[:, k, :], start=(k == 0), stop=(k == 7))
                        kraw = pab_sb.tile([128, 512], bf16, tag="kraw")
                        nc.any.tensor_copy(kraw[:], kps[:])
                        ksw = pab_ps1.tile([128, 512], f32, tag="ksw")
                        nc.tensor.matmul(ksw[:], perm[:], kraw[:], start=True, stop=True)
                        if absub < 4:
                            continue
                        k2 = pab_sb.tile([128, 512], bf16, tag="k2")
                        nc.scalar.activation(k2[:], kps[:], AF.Square)
                        # per-token sumsq token-major, then transpose to row-major
                        sstk = pab_ps1.tile([128, 4, 2], f32, tag="sstk")
                        for kt in range(4):
                            ksl = slice(kt * 128, (kt + 1) * 128)
                            nc.tensor.matmul(sstk[:, kt, :], k2[:, ksl], och[:, 0:2],
                                             start=True, stop=True)
                        sstk_sb = pab_sb.tile([128, 4, 2], bf16, tag="sstk_sb")
                        nc.any.tensor_copy(sstk_sb[:], sstk[:])
                        ssrow = pab_ps1.tile([2, 512], f32, tag="ssrow")
                        for kt in range(4):
                            ksl = slice(kt * 128, (kt + 1) * 128)
                            nc.tensor.matmul(ssrow[0:2, ksl], sstk_sb[:, kt, :],
                                             idt[:], start=True, stop=True)
                        rmsk = pab_sb.tile([2, 512], f32, tag="rmsk")
                        nc.scalar.activation(rmsk[:], ssrow[0:2, :], AF.Sqrt,
                                             scale=1.0 / HD, bias=epsq[0:2, :])
                        invk = pab_sb.tile([2, 512], f32, tag="invk")
                        nc.vector.reciprocal_approx_fast(invk[:], rmsk[:])
                        invkb = pab_sb.tile([2, 512], bf16, tag="invkb")
                        nc.vector.tensor_copy(invkb[:], invk[:])
                        repk = pab_ps1.tile([128, 512], f32, tag="repk")
                        nc.tensor.matmul(repk[:], selkt[:], invkb[:],
                                         start=True, stop=True)
                        if absub < 5:
                            continue
                        t1 = pab_sb.tile([128, 512], bf16, tag="t1k")
                        nc.vector.tensor_tensor(t1[:], kraw[:], cosf[:, sl], ALU.mult)
                        t2 = pab_sb.tile([128, 512], bf16, tag="t2k")
                        nc.vector.tensor_tensor(t2[:], ksw[:], sinf[:, sl], ALU.mult)
                        t3 = pab_sb.tile([128, 512], bf16, tag="t3k")
                        nc.vector.tensor_tensor(t3[:], t1[:], t2[:], ALU.add)
                        nc.vector.tensor_tensor(kT[:, kp, sl], t3[:], repk[:], ALU.mult)
                # ones column of V
                nc.vector.tensor_copy(
                    v_all[:, :, :, 64:66],
                    ocb[:, 0, None, None].to_broadcast([128, 4, NT, 2]))

        # ------------- Phase C: Q for own tokens --------------------------
        if ph_on("c"):
            with tc.tile_pool(name="pc_x", bufs=1) as pc_x, \
                 tc.tile_pool(name="pc_sb", bufs=3) as pc_sb, \
                 tc.tile_pool(name="pc_w", bufs=3) as pc_w, \
                 tc.tile_pool(name="pc_ps", bufs=2, space="PSUM") as pc_ps, \
                 tc.tile_pool(name="pc_ps1", bufs=1, space="PSUM") as pc_ps1:
                xqs = pc_x.tile([128, 8, OWN], bf16, tag="xqs")
                nc.sync.dma_start(xqs[:], xq[:])
                nc.sync.dma_start(selg[:], selg2[:])
                nc.sync.dma_start(o10t[:], o10[:])
                nc.sync.dma_start(o10b[:], o10bf[:])
                nc.sync.dma_start(gat[:], g_attn[:])
                nc.sync.dma_start(gml[:], g_mlp[:])
                nc.sync.dma_start(bml[:], b_mlp[:])
                for p in range(8):
                    wqs = pc_w.tile([128, 8, 128], bf16, tag="wqs")
                    nc.sync.dma_start(wqs[:], wq[p])
                    qps = pc_ps.tile([128, OWN], f32, tag="qps")
                    for k in range(8):
                        nc.tensor.matmul(qps[:], wqs[:, k, :], xqs[:, k, :],
                                         start=(k == 0), stop=(k == 7))
                    qraw = pc_sb.tile([128, OWN], bf16, tag="qraw")
                    nc.any.tensor_copy(qraw[:], qps[:])
                    qsw = pc_ps.tile([128, OWN], f32, tag="qsw")
                    nc.tensor.matmul(qsw[:], perm[:], qraw[:], start=True, stop=True)
                    q2 = pc_sb.tile([128, OWN], bf16, tag="q2")
                    nc.scalar.activation(q2[:], qps[:], AF.Square)
                    sstq = pc_ps1.tile([128, 4, 2], f32, tag="sstq")
                    for kt in range(4):
                        ksl = slice(kt * 128, (kt + 1) * 128)
                        nc.tensor.matmul(sstq[:, kt, :], q2[:, ksl], och[:, 0:2],
                                         start=True, stop=True)
                    sstq_sb = pc_sb.tile([128, 4, 2], bf16, tag="sstq_sb")
                    nc.any.tensor_copy(sstq_sb[:], sstq[:])
                    ssqrow = pc_ps1.tile([2, OWN], f32, tag="ssqrow")
                    for kt in range(4):
                        ksl = slice(kt * 128, (kt + 1) * 128)
                        nc.tensor.matmul(ssqrow[0:2, ksl], sstq_sb[:, kt, :],
                                         idt[:], start=True, stop=True)
                    rmsq = pc_sb.tile([2, OWN], f32, tag="rmsq")
                    nc.scalar.activation(rmsq[:], ssqrow[0:2, :], AF.Sqrt,
                                         scale=1.0 / HD, bias=epsq[0:2, :])
                    invq = pc_sb.tile([2, OWN], f32, tag="invq")
                    nc.vector.reciprocal_approx_fast(invq[:], rmsq[:])
                    invqb = pc_sb.tile([2, OWN], bf16, tag="invqb")
                    nc.vector.tensor_copy(invqb[:], invq[:])
                    repq = pc_ps1.tile([128, OWN], f32, tag="repq")
                    nc.tensor.matmul(repq[:], selg[:, p, :], invqb[:],
                                     start=True, stop=True)
                    t1 = pc_sb.tile([128, OWN], bf16, tag="t1q")
                    nc.vector.tensor_tensor(t1[:], qraw[:], coso[:], ALU.mult)
                    t2 = pc_sb.tile([128, OWN], bf16, tag="t2q")
                    nc.vector.tensor_tensor(t2[:], qsw[:], sino[:], ALU.mult)
                    t3 = pc_sb.tile([128, OWN], bf16, tag="t3q")
                    nc.vector.tensor_tensor(t3[:], t1[:], t2[:], ALU.add)
                    nc.vector.tensor_tensor(qT[:, p, :], t3[:], repq[:], ALU.mult)

            rope_stack.close()

        # ------------- Phase D: attention ---------------------------------
        if ph_on("d"):
            xpr = big.tile([128, 8, OWN], bf16, tag="xpr")
            xpb = big.tile([128, 8, OWN], f32, tag="xpb")
            with tc.tile_pool(name="pd_m", bufs=1) as pd_m, \
                 tc.tile_pool(name="pd_pt", bufs=6) as pd_pt, \
                 tc.tile_pool(name="pd_sb", bufs=2) as pd_sb, \
                 tc.tile_pool(name="pd_s", bufs=2, space="PSUM") as pd_s, \
                 tc.tile_pool(name="pd_y", bufs=1, space="PSUM") as pd_y, \
                 tc.tile_pool(name="pd_r", bufs=1, space="PSUM") as pd_r:
                masks = pd_m.tile([128, 4, 4, 128], bf16, tag="masks")
                nc.sync.dma_start(masks[:], maskM[:])
                for t in range(4):
                    qsl = slice(t * 128, (t + 1) * 128)
                    n_chunks = t + 1
                    n_kvt = 4 * n_chunks
                    for half in range(2):
                        gA, gB = 2 * half, 2 * half + 1
                        yA = pd_y.tile([66, 4, 128], f32, tag="yA")
                        yB = pd_y.tile([66, 4, 128], f32, tag="yB")
                        qsA = qT[0:64, 4 * half:4 * half + 4, qsl]
                        qsB = qT[64:128, 4 * half:4 * half + 4, qsl]
                        for c in range(n_chunks):
                            pts = []
                            for i in range(4):
                                ks = slice((4 * c + i) * 128, (4 * c + i + 1) * 128)
                                psAB = pd_s.tile([128, 2, 4, 128], f32, tag="psAB")
                                nc.tensor.matmul(psAB[:, 0, :, :],
                                                 kT[0:64, half, ks], qsA,
                                                 start=True, stop=True,
                                                 tile_position=(0, 0))
                                nc.tensor.matmul(psAB[:, 1, :, :],
                                                 kT[64:128, half, ks], qsB,
                                                 start=True, stop=True,
                                                 tile_position=(64, 0))
                                ptAB = pd_pt.tile([128, 2, 4, 128], bf16, tag="ptAB")
                                nc.scalar.activation(ptAB[:], psAB[:], AF.Exp)
                                if c == t:
                                    mbc = masks[:, t, i, None, None, :].to_broadcast(
                                        [128, 2, 4, 128])
                                    eng = nc.vector if i % 2 == 0 else nc.gpsimd
                                    eng.tensor_tensor(ptAB[:], ptAB[:], mbc, ALU.mult)
                                pts.append(ptAB)
                            for i in range(4):
                                kvt = 4 * c + i
                                nc.tensor.matmul(yA[:], v_all[:, gA, kvt, :],
                                                 pts[i][:, 0, :, :], start=(kvt == 0),
                                                 stop=(kvt == n_kvt - 1))
                                nc.tensor.matmul(yB[:], v_all[:, gB, kvt, :],
                                                 pts[i][:, 1, :, :], start=(kvt == 0),
                                                 stop=(kvt == n_kvt - 1))
                        for g, y in ((gA, yA), (gB, yB)):
                            dsb = pd_sb.tile([2, 4, 128], f32, tag="dsb")
                            nc.vector.tensor_copy(dsb[:], y[64:66, :, :])
                            invs = pd_sb.tile([2, 4, 128], f32, tag="invs")
                            nc.vector.reciprocal_approx_fast(invs[:], dsb[:])
                            invsb = pd_sb.tile([2, 4, 128], bf16, tag="invsb")
                            nc.vector.tensor_copy(invsb[:], invs[:])
                            ysb = pd_sb.tile([64, 4, 128], f32, tag="ysb")
                            nc.vector.tensor_copy(ysb[:], y[0:64, :, :])
                            repy = pd_r.tile([64, 4, 128], f32, tag="repy")
                            nc.tensor.matmul(repy[:].rearrange("p a b -> p (a b)"),
                                             o10b[:, 0:64],
                                             invsb[:].rearrange("p a b -> p (a b)"),
                                             start=True, stop=True)
                            for i in range(4):
                                h = 4 * g + i
                                chunk, part = h // 2, (h % 2) * 64
                                nc.vector.tensor_tensor(
                                    y_all[part:part + 64, chunk, qsl],
                                    ysb[:, i, :], repy[:, i, :], ALU.mult)

        # ------------- Phase E: Wo + post-norm + residual -----------------
        if ph_on("e"):
            with tc.tile_pool(name="pe_sb", bufs=2) as pe_sb, \
                 tc.tile_pool(name="pe_ao", bufs=1) as pe_ao, \
                 tc.tile_pool(name="pe_w", bufs=3) as pe_w, \
                 tc.tile_pool(name="pe_ps", bufs=2, space="PSUM") as pe_ps, \
                 tc.tile_pool(name="pe_ss", bufs=1, space="PSUM") as pe_ss:
                ao = pe_ao.tile([128, 8, OWN], f32, tag="ao")
                ssa = pe_ss.tile([2, OWN], f32, tag="ssa")
                for o in range(8):
                    wos = pe_w.tile([128, 8, 128], bf16, tag="wos")
                    nc.sync.dma_start(wos[:], wo[o])
                    aps = pe_ps.tile([128, OWN], f32, tag="aps")
                    for k in range(8):
                        nc.tensor.matmul(aps[:], wos[:, k, :], y_all[:, k, :],
                                         start=(k == 0), stop=(k == 7))
                    nc.any.tensor_copy(ao[:, o, :], aps[:])
                    a2 = pe_sb.tile([128, OWN], bf16, tag="a2")
                    nc.scalar.activation(a2[:], aps[:], AF.Square)
                    nc.tensor.matmul(ssa[:], ocb[:, 0:2], a2[:],
                                     start=(o == 0), stop=(o == 7))
                rmsa = pe_sb.tile([2, OWN], f32, tag="rmsa")
                nc.scalar.activation(rmsa[:], ssa[0:2, :], AF.Sqrt,
                                     scale=1.0 / D, bias=eps6[0:2, :])
                inva = pe_sb.tile([2, OWN], f32, tag="inva")
                nc.vector.reciprocal_approx_fast(inva[:], rmsa[:])
                invab = pe_sb.tile([2, OWN], bf16, tag="invab")
                nc.vector.tensor_copy(invab[:], inva[:])
                repa = pe_ss.tile([128, OWN], f32, tag="repa")
                nc.tensor.matmul(repa[:], o10b[:], invab[:], start=True, stop=True)
                for o in range(8):
                    t1 = pe_sb.tile([128, OWN], f32, tag="t1e")
                    nc.vector.tensor_tensor(t1[:], ao[:, o, :], repa[:], ALU.mult)
                    nc.vector.scalar_tensor_tensor(
                        xpb[:, o, :], t1[:], gat[:, o, None], xrs[:, o, :],
                        ALU.mult, ALU.add)
                    nc.any.tensor_copy(xpr[:, o, :], xpb[:, o, :])
                    nc.vector.tensor_scalar_add(xpb[:, o, :], xpb[:, o, :],
                                                bml[:, o, None])

        # ------------- Phase F: MLP ---------------------------------------
        if ph_on("f"):
            mout = big.tile([128, 8, OWN], f32, tag="xrs_mout")
            with tc.tile_pool(name="pf_h2", bufs=1) as pf_h2, \
                 tc.tile_pool(name="pf_sb", bufs=2) as pf_sb, \
                 tc.tile_pool(name="pf_wf", bufs=3) as pf_wf, \
                 tc.tile_pool(name="pf_wp", bufs=3) as pf_wp, \
                 tc.tile_pool(name="pf_ps", bufs=2, space="PSUM") as pf_ps, \
                 tc.tile_pool(name="pf_mo", bufs=1, space="PSUM") as pf_mo:
                h2 = pf_h2.tile([128, 32, OWN], bf16, tag="h2")
                for hc in range(32):
                    wfs = pf_wf.tile([128, 8, 128], bf16, tag="wfs")
                    nc.sync.dma_start(wfs[:], wfc[hc])
                    hps = pf_ps.tile([128, OWN], f32, tag="hps")
                    for k in range(8):
                        nc.tensor.matmul(hps[:], wfs[:, k, :], xpr[:, k, :],
                                         start=(k == 0), stop=(k == 7))
                    hr = pf_sb.tile([128, OWN], bf16, tag="hr")
                    nc.scalar.activation(hr[:], hps[:], AF.Relu)
                    nc.vector.tensor_tensor(h2[:, hc, :], hr[:], hr[:], ALU.mult)
                ssm = pf_ps.tile([2, OWN], f32, tag="ssm")
                for ohalf in range(2):
                    mo_ps = [pf_mo.tile([128, OWN], f32, name=f"mo{oi}", tag=f"mo{oi}")
                             for oi in range(4)]
                    for hc in range(32):
                        wps = pf_wp.tile([128, 4, 128], bf16, tag="wps")
                        nc.sync.dma_start(wps[:], wprojq[hc, ohalf])
                        for oi in range(4):
                            nc.tensor.matmul(mo_ps[oi][:], wps[:, oi, :], h2[:, hc, :],
                                             start=(hc == 0), stop=(hc == 31))
                    for oi in range(4):
                        o = ohalf * 4 + oi
                        nc.any.tensor_copy(mout[:, o, :], mo_ps[oi][:])
                        m2 = pf_sb.tile([128, OWN], bf16, tag="m2")
                        nc.scalar.activation(m2[:], mo_ps[oi][:], AF.Square)
                        nc.tensor.matmul(ssm[:], ocb[:, 0:2], m2[:],
                                         start=(o == 0), stop=(o == 7))
                rmsm = pf_sb.tile([2, OWN], f32, tag="rmsm")
                nc.scalar.activation(rmsm[:], ssm[0:2, :], AF.Sqrt, scale=1.0 / D,
                                     bias=eps6[0:2, :])
                invm = pf_sb.tile([2, OWN], f32, tag="invm")
                nc.vector.reciprocal_approx_fast(invm[:], rmsm[:])
                invmb = pf_sb.tile([2, OWN], bf16, tag="invmb")
                nc.vector.tensor_copy(invmb[:], invm[:])
                repm = pf_ps.tile([128, OWN], f32, tag="hps")
                nc.tensor.matmul(repm[:], o10b[:], invmb[:], start=True, stop=True)
                for o in range(8):
                    t1 = pf_sb.tile([128, OWN], f32, tag="t1f")
                    nc.vector.tensor_tensor(t1[:], mout[:, o, :], repm[:], ALU.mult)
                    outv = pf_sb.tile([128, OWN], f32, tag="outv")
                    nc.vector.scalar_tensor_tensor(
                        outv[:], t1[:], gml[:, o, None], xpb[:, o, :],
                        ALU.mult, ALU.add)
                    nc.sync.dma_start(out_t[:, o, :], outv[:])

        if not ph_on("f"):
            with tc.tile_pool(name="dummy", bufs=1) as dp:
                zout = dp.tile([128, 8, OWN], f32, tag="zout")
                nc.vector.memset(zout[:], 0.0)
                nc.sync.dma_start(out_t[:], zout[:])
            rope_stack.close()

    nc.finalize()
    return nc


def _feat_major(a):
    """[F, T] -> device layout [128, F//128, T]."""
    F, T = a.shape
    return np.ascontiguousarray(a.reshape(F // 128, 128, T).transpose(1, 0, 2))


def _vec_dev(v):
    return np.ascontiguousarray(v.reshape(-1, 128).T)


def _bf(a):
    return np.ascontiguousarray(a.astype(ml_dtypes.bfloat16))


_CACHE = {}
_RUN_KW = {}


def kernel(x, attn_norm_w, mlp_norm_w, attn_post_norm_w, mlp_post_norm_w,
           attn_scale, mlp_scale, attn_mod_gain, attn_mod_bias,
           mlp_mod_gain, mlp_mod_bias, Wq, Wk, Wv, Wo, q_gain, fc_w, proj_w):
    x = np.asarray(x, np.float32)
    q_gain = np.asarray(q_gain, np.float32)

    if "nc" not in _CACHE:
        _CACHE["nc"] = build(q_gain)
    nc = _CACHE["nc"]

    anw = np.asarray(attn_norm_w, np.float32)
    mnw = np.asarray(mlp_norm_w, np.float32)
    wq_eff = np.asarray(Wq, np.float32) * anw[None, :]
    wk_eff = np.asarray(Wk, np.float32) * anw[None, :]
    wv_eff = np.asarray(Wv, np.float32) * anw[None, :]
    fc_eff = np.asarray(fc_w, np.float32) * mnw[None, :]

    perm = np.zeros(D, np.int64)
    for p, (a, b) in enumerate(PAIRS):
        perm[p * 128:p * 128 + 64] = np.arange(a * 64, a * 64 + 64)
        perm[p * 128 + 64:(p + 1) * 128] = np.arange(b * 64, b * 64 + 64)
    WqTp = wq_eff.T[:, perm]                                  # [D_in, D_out-perm]
    wq_dev = _bf(np.stack([_feat_major(WqTp[:, p * 128:(p + 1) * 128]) for p in range(8)]))
    wk_dev = _bf(_feat_major(wk_eff.T))
    wv_dev = _bf(_feat_major(wv_eff.T))
    WoT = np.asarray(Wo, np.float32).T
    wo_dev = _bf(np.stack([_feat_major(WoT[:, o * 128:(o + 1) * 128]) for o in range(8)]))
    fcT = fc_eff.T
    wfc_dev = _bf(np.stack([_feat_major(fcT[:, h * 128:(h + 1) * 128]) for h in range(32)]))
    projT = np.asarray(proj_w, np.float32).T                  # [4096, 1024]
    wproj_dev = _bf(np.ascontiguousarray(
        projT.reshape(32, 128, 2, 4, 128).transpose(0, 2, 1, 3, 4)))

    inv_freq = 1.0 / (ROPE_BASE ** (np.arange(0, HD, 2, dtype=np.float32) / HD))
    tpos = np.arange(S, dtype=np.float32)
    freqs = np.outer(tpos, inv_freq).astype(np.float32)
    cosT = np.ascontiguousarray(np.tile(np.cos(freqs).T, (4, 1)))   # [128, S]
    sin1 = np.sin(freqs).T                                          # [32, S]
    sinS = np.ascontiguousarray(
        np.concatenate([sin1, -sin1, sin1, -sin1], axis=0))         # [128, S] sign-folded

    # swap permutation i <-> i^32 (within each 64-wide head)
    permM_h = np.zeros((128, 128), np.float32)
    for i in range(128):
        permM_h[i, i ^ 32] = 1.0

    oc_h_v = np.zeros((128, 2), np.float32)
    oc_h_v[0:64, 0] = 1.0
    oc_h_v[64:128, 1] = 1.0
    selg2_v = np.zeros((2, 8, 128), np.float32)
    for p, (a, b) in enumerate(PAIRS):
        selg2_v[0, p, 0:64] = q_gain[a] / 8.0
        selg2_v[1, p, 64:128] = q_gain[b] / 8.0
    ones10_v = np.concatenate([np.ones((1, 128), np.float32),
                               np.zeros((1, 128), np.float32)])

    gat_v = (np.asarray(attn_post_norm_w, np.float32)
             * np.asarray(attn_mod_gain, np.float32)
             * np.asarray(attn_scale, np.float32))
    bat_v = np.asarray(attn_mod_bias, np.float32) * np.asarray(attn_scale, np.float32)
    gml_v = (np.asarray(mlp_post_norm_w, np.float32)
             * np.asarray(mlp_mod_gain, np.float32)
             * np.asarray(mlp_scale, np.float32))
    bml_v = np.asarray(mlp_mod_bias, np.float32) * np.asarray(mlp_scale, np.float32)

    _bf_selk = _bf(np.stack([np.concatenate([np.ones(64, np.float32), np.zeros(64, np.float32)]),
                          np.concatenate([np.zeros(64, np.float32), np.ones(64, np.float32)])]))
    shared = {
        "wq": wq_dev, "wk": wk_dev, "wv": wv_dev, "wo": wo_dev,
        "wfc": wfc_dev, "wprojq": wproj_dev,
        "cosF": _bf(cosT), "sinF": _bf(sinS),
        "permM": _bf(permM_h),
        "oc_h": _bf(oc_h_v),
        "onescb": _bf(np.ones((128, 2), np.float32)),
        "selg2": _bf(selg2_v),
        "identM": _bf(np.eye(128, dtype=np.float32)),
        "selk": _bf_selk,
        "o10": ones10_v,
        "o10bf": _bf(ones10_v),
        "g_attn": _vec_dev(gat_v), "g_mlp": _vec_dev(gml_v),
        "b_mlp": _vec_dev(bml_v),
    }

    in_maps = []
    owners = []
    for c in range(8):
        b, j = c // 4, c % 4
        rows = np.concatenate(
            [np.arange((j + 4 * t) * 128, (j + 4 * t + 1) * 128) for t in range(4)])
        owners.append((b, rows))
        xb = x[b].T
        x_own = xb[:, rows]
        mask = np.zeros((4, 4, 128, 128), np.float32)
        for t in range(4):
            m = j + 4 * t
            q_idx = m * 128 + np.arange(128)
            for ktl in range(4):
                kv_idx = 512 * t + 128 * ktl + np.arange(128)
                mask[t, ktl] = (kv_idx[:, None] <= q_idx[None, :])
        m_in = {
            "xT": _bf(_feat_major(xb)),
            "xq": _bf(_feat_major(x_own)),
            "xres": _feat_major(x_own + bat_v[:, None]),
            "cosO": _bf(cosT[:, rows]),
            "sinO": _bf(sinS[:, rows]),
            "maskM": _bf(np.ascontiguousarray(mask.transpose(2, 0, 1, 3))),
        }
        m_in.update(shared)
        in_maps.append(m_in)

    res = run_bass_kernel_spmd(nc, in_maps, core_ids=list(range(8)),
                               **_RUN_KW)
    _CACHE["last_result"] = res

    out = np.empty((B, S, D), np.float32)
    for c in range(8):
        b, rows = owners[c]
        o = res.results[c]["out"]
        out[b, rows, :] = o.transpose(2, 1, 0).reshape(OWN, D)
    return out


# revision 54
# speedup vs baseline: 1.3237x; 1.0098x over previous
"""Trainium2 Bass kernel for one dense transformer block (B=2, S=2048, D=1024,
16 q-heads / 4 kv-heads GQA, squared-ReLU MLP), data-parallel over 8 NeuronCores.

Sharding: core c = (b, j), b = c // 4, j = c % 4, owns q-token tiles
{j, j+4, j+8, j+12} (128 tokens each) of batch b. K/V are computed for the full
sequence on every core (no collectives). The kv range for own q-tile t is
padded to 512*(t+1); causality enforced with per-core 0/1 masks on the last
512-wide kv chunk.

Numerical identities used (exact up to negligible eps rescaling):
  - per-head q/k rmsnorm is scale-invariant per token, so the block input
    rmsnorm cancels inside it -> Q/K project from raw (norm-weight-folded) x
  - the MLP input rmsnorm cancels through relu()^2 -> proj -> post-rmsnorm
  - V is projected from raw x and rescaled by 1/rms1(x) per token
  - no softmax max-subtraction (logits bounded by |q||k|/8 = 8)
  - softmax denominator = ones-column appended to V in the AV matmul
  - K's 1/rms is applied as a per-partition AP scale inside the exp
    activation (kv tokens are partitions in the score tiles); Q's 1/rms and
    q_gain/8 ride a replicate matmul onto qT

v2 perf changes vs baseline:
  - no DVE reciprocal with f32r destination (was ~7.7ns/elem); all recips are
    fp32->fp32 on DVE, replicates via small fp32 matmuls
  - rope via a feature-swap permutation matmul + 3 full-width DVE ops
    (was 12 narrow DVE ops)
  - bf16 weights + x + V/p/mask/y/h2 paths (half DMA, FWL weight loads,
    2x DVE); q/k/scores stay f32r
"""

import os

import numpy as np
import ml_dtypes

import concourse.bass as bass
from concourse import bacc
import concourse.tile as tile
import concourse.mybir as mybir
from concourse.bass_utils import run_bass_kernel_spmd

f32 = mybir.dt.float32
f32r = mybir.dt.float32r
bf16 = mybir.dt.bfloat16
AF = mybir.ActivationFunctionType
ALU = mybir.AluOpType

B, S, D = 2, 2048, 1024
H, HKV, HD = 16, 4, 64
MLP_HID = 4 * D
KV = HKV * HD
NT = 16
OWN = 512
EPS_BLOCK = 1e-6
EPS_QK = float(np.finfo(np.float32).eps)
ROPE_BASE = 10000.0

PAIRS = [(0, 4), (1, 5), (2, 6), (3, 7), (8, 12), (9, 13), (10, 14), (11, 15)]

PHASE_ORDER = ["ab", "c", "d", "e", "f"]


def build(q_gain):
    max_ph = os.environ.get("KERNEL_PHASES", "f")
    ph_on = lambda p: PHASE_ORDER.index(p) <= PHASE_ORDER.index(max_ph)
    bacc.Bacc.move_matmul_waits_to_ldweights = lambda self: None
    nc = bacc.Bacc(None)

    def dram_in(name, shape, dt):
        return nc.dram_tensor(name, list(shape), dt, kind="ExternalInput")

    xT = dram_in("xT", (128, 8, S), bf16)
    xq = dram_in("xq", (128, 8, OWN), bf16)
    xres = dram_in("xres", (128, 8, OWN), f32)
    wq = dram_in("wq", (8, 128, 8, 128), bf16)
    wk = dram_in("wk", (128, 8, KV), bf16)
    wv = dram_in("wv", (128, 8, KV), bf16)
    wo = dram_in("wo", (8, 128, 8, 128), bf16)
    wfc = dram_in("wfc", (32, 128, 8, 128), bf16)
    wprojq = dram_in("wprojq", (32, 2, 128, 4, 128), bf16)
    cosF = dram_in("cosF", (128, S), bf16)
    sinF = dram_in("sinF", (128, S), bf16)   # sign-folded: +sin rows 0-31/64-95, -sin rows 32-63/96-127
    cosO = dram_in("cosO", (128, OWN), bf16)
    sinO = dram_in("sinO", (128, OWN), bf16)
    maskM = dram_in("maskM", (128, 4, 4, 128), bf16)
    permM = dram_in("permM", (128, 128), bf16)     # swap rows i <-> i^32
    oc_h = dram_in("oc_h", (128, 2), bf16)         # col0: top-64 ones; col1: bottom-64 ones
    onescb = dram_in("onescb", (128, 2), bf16)     # all ones
    selg2 = dram_in("selg2", (2, 8, 128), bf16)     # row0 -> cols 0-63 * gA/8, row1 -> cols 64-127 * gB/8
    identM = dram_in("identM", (128, 128), bf16)   # 128x128 identity
    selk = dram_in("selk", (2, 128), bf16)          # row0 -> cols 0-63 ones, row1 -> cols 64-127 ones
    o10 = dram_in("o10", (2, 128), f32)
    o10bf = dram_in("o10bf", (2, 128), bf16)            # row0 ones, row1 zeros
    g_attn = dram_in("g_attn", (128, 8), f32)
    g_mlp = dram_in("g_mlp", (128, 8), f32)
    b_mlp = dram_in("b_mlp", (128, 8), f32)

    out_t = nc.dram_tensor("out", [128, 8, OWN], f32, kind="ExternalOutput")

    with tile.TileContext(nc) as tc, \
         tc.tile_pool(name="cst", bufs=1) as cst, \
         tc.tile_pool(name="big", bufs=1) as big:
        och = cst.tile([128, 2], bf16, tag="och")
        ocb = cst.tile([128, 2], bf16, tag="ocb")
        selg = cst.tile([2, 8, 128], bf16, tag="selg")
        idt = cst.tile([128, 128], bf16, tag="idt")
        selkt = cst.tile([2, 128], bf16, tag="selkt")
        o10t = cst.tile([2, 128], f32, tag="o10t")
        o10b = cst.tile([2, 128], bf16, tag="o10b")
        perm = cst.tile([128, 128], bf16, tag="perm")
        eps6 = cst.tile([128, 1], f32, tag="eps6")
        nc.vector.memset(eps6[:], EPS_BLOCK)
        epsq = cst.tile([128, 1], f32, tag="epsq")
        nc.vector.memset(epsq[:], EPS_QK)
        gat = cst.tile([128, 8], f32, tag="gat")
        gml = cst.tile([128, 8], f32, tag="gml")
        bml = cst.tile([128, 8], f32, tag="bml")
        from contextlib import ExitStack
        rope_stack = ExitStack()
        ropep = rope_stack.enter_context(tc.tile_pool(name="ropep", bufs=1))
        cosf = ropep.tile([128, S], bf16, tag="cosf")
        sinf = ropep.tile([128, S], bf16, tag="sinf")
        coso = ropep.tile([128, OWN], bf16, tag="coso")
        sino = ropep.tile([128, OWN], bf16, tag="sino")

        kT = big.tile([128, 2, S], f32r, tag="kT")
        v_all = big.tile([128, 4, NT, 66], bf16, tag="v_all")
        qT = big.tile([128, 8, OWN], f32r, tag="qT")
        y_all = big.tile([128, 8, OWN], bf16, tag="y_all")
        xrs = big.tile([128, 8, OWN], f32, tag="xrs_mout")
        invr1 = big.tile([128, NT], f32, tag="invr1")
        rms_st = big.tile([128, NT], f32, tag="rms_st")

        # ------------- Phase AB: rms1, K, V over full sequence ------------
        absub = int(os.environ.get("KERNEL_AB_SUB", "99"))
        if ph_on("ab"):
            with tc.tile_pool(name="pab_x", bufs=3) as pab_x, \
                 tc.tile_pool(name="pab_sb", bufs=2) as pab_sb, \
                 tc.tile_pool(name="pab_w", bufs=1) as pab_w, \
                 tc.tile_pool(name="pab_ps", bufs=2, space="PSUM") as pab_ps, \
                 tc.tile_pool(name="pab_ps1", bufs=1, space="PSUM") as pab_ps1:
                xc0 = pab_x.tile([128, 8, 512], bf16, tag="xc")
                nc.sync.dma_start(xc0[:], xT[:, :, 0:512])
                nc.sync.dma_start(och[:], oc_h[:])
                nc.sync.dma_start(ocb[:], onescb[:])
                nc.sync.dma_start(perm[:], permM[:])
                wvs = pab_w.tile([128, 8, KV], bf16, tag="wvs")
                nc.sync.dma_start(wvs[:], wv[:])
                wks = pab_w.tile([128, 8, KV], bf16, tag="wks")
                nc.sync.dma_start(wks[:], wk[:])
                nc.sync.dma_start(idt[:], identM[:])
                nc.sync.dma_start(selkt[:], selk[:])
                nc.sync.dma_start(cosf[:], cosF[:])
                nc.sync.dma_start(sinf[:], sinF[:])
                nc.sync.dma_start(coso[:], cosO[:])
                nc.sync.dma_start(sino[:], sinO[:])
                nc.sync.dma_start(xrs[:], xres[:])
                for ci in range(4):
                    sl = slice(ci * 512, (ci + 1) * 512)
                    if ci == 0:
                        xc = xc0
                    else:
                        xc = pab_x.tile([128, 8, 512], bf16, tag="xc")
                        nc.sync.dma_start(xc[:], xT[:, :, sl])
                    # token-major sumsq -> invr1 for the 4 token tiles of the chunk
                    for kt in range(4):
                        x2 = pab_sb.tile([128, 8, 128], bf16, tag="x2")
                        nc.scalar.activation(x2[:], xc[:, :, kt * 128:(kt + 1) * 128],
                                             AF.Square)
                        ssp = pab_ps1.tile([128, 2], f32, tag="sstk")
                        for k in range(8):
                            nc.tensor.matmul(ssp[:], x2[:, k, :], ocb[:, 0:2],
                                             start=(k == 0), stop=(k == 7))
                        nc.scalar.activation(rms_st[:, ci * 4 + kt, None], ssp[:, 0:1],
                                             AF.Sqrt, scale=1.0 / D, bias=eps6[:])
                    nc.vector.reciprocal(invr1[:, ci * 4:(ci + 1) * 4],
                                         rms_st[:, ci * 4:(ci + 1) * 4])
                    # V token-major for the 4 token tiles
                    for kt in range(4 if absub >= 2 else 0):
                        gkt = ci * 4 + kt
                        vps = pab_ps.tile([128, KV], f32, tag="vps")
                        for k in range(8):
                            nc.tensor.matmul(vps[:], xc[:, k, kt * 128:(kt + 1) * 128],
                                             wvs[:, k, :], start=(k == 0), stop=(k == 7))
                        nc.vector.tensor_scalar_mul(
                            v_all[:, :, gkt, 0:64],
                            vps[:].rearrange("p (g d) -> p g d", g=4),
                            invr1[:, gkt, None])
                    # K feature-major for both kv pairs
                    for kp in range(2 if absub >= 3 else 0):
                        kps = pab_ps.tile([128, 512], f32, tag="kps")
                        for k in range(8):
                            nc.tensor.matmul(kps[:], wks[:, k, kp * 128:(kp + 1) * 128],
                                             xc[:, k, :], start=(k == 0), stop=(k == 7))
                        kraw = pab_sb.tile([128, 512], bf16, tag="kraw")
                        nc.any.tensor_copy(kraw[:], kps[:])
                        ksw = pab_ps1.tile([128, 512], f32, tag="ksw")
                        nc.tensor.matmul(ksw[:], perm[:], kraw[:], start=True, stop=True)
                        if absub < 4:
                            continue
                        k2 = pab_sb.tile([128, 512], bf16, tag="k2")
                        nc.scalar.activation(k2[:], kps[:], AF.Square)
                        # per-token sumsq token-major, then transpose to row-major
                        sstk = pab_ps1.tile([128, 4, 2], f32, tag="sstk")
                        for kt in range(4):
                            ksl = slice(kt * 128, (kt + 1) * 128)
                            nc.tensor.matmul(sstk[:, kt, :], k2[:, ksl], och[:, 0:2],
                                             start=True, stop=True)
                        sstk_sb = pab_sb.tile([128, 4, 2], bf16, tag="sstk_sb")
                        nc.any.tensor_copy(sstk_sb[:], sstk[:])
                        ssrow = pab_ps1.tile([2, 512], f32, tag="ssrow")
                        for kt in range(4):
                            ksl = slice(kt * 128, (kt + 1) * 128)
                            nc.tensor.matmul(ssrow[0:2, ksl], sstk_sb[:, kt, :],
                                             idt[:], start=True, stop=True)
                        rmsk = pab_sb.tile([2, 512], f32, tag="rmsk")
                        nc.scalar.activation(rmsk[:], ssrow[0:2, :], AF.Sqrt,
                                             scale=1.0 / HD, bias=epsq[0:2, :])
                        invk = pab_sb.tile([2, 512], f32, tag="invk")
                        nc.vector.reciprocal_approx_fast(invk[:], rmsk[:])
                        invkb = pab_sb.tile([2, 512], bf16, tag="invkb")
                        nc.vector.tensor_copy(invkb[:], invk[:])
                        repk = pab_ps1.tile([128, 512], f32, tag="repk")
                        nc.tensor.matmul(repk[:], selkt[:], invkb[:],
                                         start=True, stop=True)
                        if absub < 5:
                            continue
                        t1 = pab_sb.tile([128, 512], bf16, tag="t1k")
                        nc.vector.tensor_tensor(t1[:], kraw[:], cosf[:, sl], ALU.mult)
                        t2 = pab_sb.tile([128, 512], bf16, tag="t2k")
                        nc.vector.tensor_tensor(t2[:], ksw[:], sinf[:, sl], ALU.mult)
                        t3 = pab_sb.tile([128, 512], bf16, tag="t3k")
                        nc.vector.tensor_tensor(t3[:], t1[:], t2[:], ALU.add)
                        nc.vector.tensor_tensor(kT[:, kp, sl], t3[:], repk[:], ALU.mult)
                # ones column of V
                nc.vector.tensor_copy(
                    v_all[:, :, :, 64:66],
                    ocb[:, 0, None, None].to_broadcast([128, 4, NT, 2]))

        # ------------- Phase C: Q for own tokens --------------------------
        if ph_on("c"):
            with tc.tile_pool(name="pc_x", bufs=1) as pc_x, \
                 tc.tile_pool(name="pc_sb", bufs=3) as pc_sb, \
                 tc.tile_pool(name="pc_w", bufs=3) as pc_w, \
                 tc.tile_pool(name="pc_ps", bufs=2, space="PSUM") as pc_ps, \
                 tc.tile_pool(name="pc_ps1", bufs=1, space="PSUM") as pc_ps1:
                xqs = pc_x.tile([128, 8, OWN], bf16, tag="xqs")
                nc.sync.dma_start(xqs[:], xq[:])
                nc.sync.dma_start(selg[:], selg2[:])
                nc.sync.dma_start(o10t[:], o10[:])
                nc.sync.dma_start(o10b[:], o10bf[:])
                nc.sync.dma_start(gat[:], g_attn[:])
                nc.sync.dma_start(gml[:], g_mlp[:])
                nc.sync.dma_start(bml[:], b_mlp[:])
                for p in range(8):
                    wqs = pc_w.tile([128, 8, 128], bf16, tag="wqs")
                    nc.sync.dma_start(wqs[:], wq[p])
                    qps = pc_ps.tile([128, OWN], f32, tag="qps")
                    for k in range(8):
                        nc.tensor.matmul(qps[:], wqs[:, k, :], xqs[:, k, :],
                                         start=(k == 0), stop=(k == 7))
                    qraw = pc_sb.tile([128, OWN], bf16, tag="qraw")
                    nc.any.tensor_copy(qraw[:], qps[:])
                    qsw = pc_ps.tile([128, OWN], f32, tag="qsw")
                    nc.tensor.matmul(qsw[:], perm[:], qraw[:], start=True, stop=True)
                    q2 = pc_sb.tile([128, OWN], bf16, tag="q2")
                    nc.scalar.activation(q2[:], qps[:], AF.Square)
                    sstq = pc_ps1.tile([128, 4, 2], f32, tag="sstq")
                    for kt in range(4):
                        ksl = slice(kt * 128, (kt + 1) * 128)
                        nc.tensor.matmul(sstq[:, kt, :], q2[:, ksl], och[:, 0:2],
                                         start=True, stop=True)
                    sstq_sb = pc_sb.tile([128, 4, 2], bf16, tag="sstq_sb")
                    nc.any.tensor_copy(sstq_sb[:], sstq[:])
                    ssqrow = pc_ps1.tile([2, OWN], f32, tag="ssqrow")
                    for kt in range(4):
                        ksl = slice(kt * 128, (kt + 1) * 128)
                        nc.tensor.matmul(ssqrow[0:2, ksl], sstq_sb[:, kt, :],
                                         idt[:], start=True, stop=True)
                    rmsq = pc_sb.tile([2, OWN], f32, tag="rmsq")
                    nc.scalar.activation(rmsq[:], ssqrow[0:2, :], AF.Sqrt,
                                         scale=1.0 / HD, bias=epsq[0:2, :])
                    invq = pc_sb.tile([2, OWN], f32, tag="invq")
                    nc.vector.reciprocal_approx_fast(invq[:], rmsq[:])
                    invqb = pc_sb.tile([2, OWN], bf16, tag="invqb")
                    nc.vector.tensor_copy(invqb[:], invq[:])
                    repq = pc_ps1.tile([128, OWN], f32, tag="repq")
                    nc.tensor.matmul(repq[:], selg[:, p, :], invqb[:],
                                     start=True, stop=True)
                    t1 = pc_sb.tile([128, OWN], bf16, tag="t1q")
                    nc.vector.tensor_tensor(t1[:], qraw[:], coso[:], ALU.mult)
                    t2 = pc_sb.tile([128, OWN], bf16, tag="t2q")
                    nc.vector.tensor_tensor(t2[:], qsw[:], sino[:], ALU.mult)
                    t3 = pc_sb.tile([128, OWN], bf16, tag="t3q")
                    nc.vector.tensor_tensor(t3[:], t1[:], t2[:], ALU.add)
                    nc.vector.tensor_tensor(qT[:, p, :], t3[:], repq[:], ALU.mult)

            rope_stack.close()

        # ------------- Phase D: attention ---------------------------------
        if ph_on("d"):
            xpr = big.tile([128, 8, OWN], bf16, tag="xpr")
            xpb = big.tile([128, 8, OWN], f32, tag="xpb")
            with tc.tile_pool(name="pd_m", bufs=1) as pd_m, \
                 tc.tile_pool(name="pd_pt", bufs=6) as pd_pt, \
                 tc.tile_pool(name="pd_sb", bufs=2) as pd_sb, \
                 tc.tile_pool(name="pd_s", bufs=2, space="PSUM") as pd_s, \
                 tc.tile_pool(name="pd_y", bufs=1, space="PSUM") as pd_y, \
                 tc.tile_pool(name="pd_r", bufs=1, space="PSUM") as pd_r:
                masks = pd_m.tile([128, 4, 4, 128], bf16, tag="masks")
                nc.sync.dma_start(masks[:], maskM[:])
                for t in range(4):
                    qsl = slice(t * 128, (t + 1) * 128)
                    n_chunks = t + 1
                    n_kvt = 4 * n_chunks
                    for half in range(2):
                        gA, gB = 2 * half, 2 * half + 1
                        yA = pd_y.tile([66, 4, 128], f32, tag="yA")
                        yB = pd_y.tile([66, 4, 128], f32, tag="yB")
                        qsA = qT[0:64, 4 * half:4 * half + 4, qsl]
                        qsB = qT[64:128, 4 * half:4 * half + 4, qsl]
                        for c in range(n_chunks):
                            pts = []
                            for i in range(4):
                                ks = slice((4 * c + i) * 128, (4 * c + i + 1) * 128)
                                psAB = pd_s.tile([128, 2, 4, 128], f32, tag="psAB")
                                nc.tensor.matmul(psAB[:, 0, :, :],
                                                 kT[0:64, half, ks], qsA,
                                                 start=True, stop=True,
                                                 tile_position=(0, 0))
                                nc.tensor.matmul(psAB[:, 1, :, :],
                                                 kT[64:128, half, ks], qsB,
                                                 start=True, stop=True,
                                                 tile_position=(64, 0))
                                ptAB = pd_pt.tile([128, 2, 4, 128], bf16, tag="ptAB")
                                nc.scalar.activation(ptAB[:], psAB[:], AF.Exp)
                                if c == t:
                                    mbc = masks[:, t, i, None, None, :].to_broadcast(
                                        [128, 2, 4, 128])
                                    eng = nc.vector if i % 2 == 0 else nc.gpsimd
                                    eng.tensor_tensor(ptAB[:], ptAB[:], mbc, ALU.mult)
                                pts.append(ptAB)
                            for i in range(4):
                                kvt = 4 * c + i
                                nc.tensor.matmul(yA[:], v_all[:, gA, kvt, :],
                                                 pts[i][:, 0, :, :], start=(kvt == 0),
                                                 stop=(kvt == n_kvt - 1))
                                nc.tensor.matmul(yB[:], v_all[:, gB, kvt, :],
                                                 pts[i][:, 1, :, :], start=(kvt == 0),
                                                 stop=(kvt == n_kvt - 1))
                        for g, y in ((gA, yA), (gB, yB)):
                            dsb = pd_sb.tile([2, 4, 128], f32, tag="dsb")
                            nc.vector.tensor_copy(dsb[:], y[64:66, :, :])
                            invs = pd_sb.tile([2, 4, 128], f32, tag="invs")
                            nc.vector.reciprocal_approx_fast(invs[:], dsb[:])
                            invsb = pd_sb.tile([2, 4, 128], bf16, tag="invsb")
                            nc.vector.tensor_copy(invsb[:], invs[:])
                            ysb = pd_sb.tile([64, 4, 128], f32, tag="ysb")
                            nc.vector.tensor_copy(ysb[:], y[0:64, :, :])
                            repy = pd_r.tile([64, 4, 128], f32, tag="repy")
                            nc.tensor.matmul(repy[:].rearrange("p a b -> p (a b)"),
                                             o10b[:, 0:64],
                                             invsb[:].rearrange("p a b -> p (a b)"),
                                             start=True, stop=True)
                            for i in range(4):
                                h = 4 * g + i
                                chunk, part = h // 2, (h % 2) * 64
                                nc.vector.tensor_tensor(
                                    y_all[part:part + 64, chunk, qsl],
                                    ysb[:, i, :], repy[:, i, :], ALU.mult)

        # ------------- Phase E: Wo + post-norm + residual -----------------
        if ph_on("e"):
            with tc.tile_pool(name="pe_sb", bufs=2) as pe_sb, \
                 tc.tile_pool(name="pe_ao", bufs=1) as pe_ao, \
                 tc.tile_pool(name="pe_w", bufs=3) as pe_w, \
                 tc.tile_pool(name="pe_ps", bufs=2, space="PSUM") as pe_ps, \
                 tc.tile_pool(name="pe_ss", bufs=1, space="PSUM") as pe_ss:
                ao = pe_ao.tile([128, 8, OWN], f32, tag="ao")
                ssa = pe_ss.tile([2, OWN], f32, tag="ssa")
                for o in range(8):
                    wos = pe_w.tile([128, 8, 128], bf16, tag="wos")
                    nc.sync.dma_start(wos[:], wo[o])
                    aps = pe_ps.tile([128, OWN], f32, tag="aps")
                    for k in range(8):
                        nc.tensor.matmul(aps[:], wos[:, k, :], y_all[:, k, :],
                                         start=(k == 0), stop=(k == 7))
                    nc.any.tensor_copy(ao[:, o, :], aps[:])
                    a2 = pe_sb.tile([128, OWN], bf16, tag="a2")
                    nc.scalar.activation(a2[:], aps[:], AF.Square)
                    nc.tensor.matmul(ssa[:], ocb[:, 0:2], a2[:],
                                     start=(o == 0), stop=(o == 7))
                rmsa = pe_sb.tile([2, OWN], f32, tag="rmsa")
                nc.scalar.activation(rmsa[:], ssa[0:2, :], AF.Sqrt,
                                     scale=1.0 / D, bias=eps6[0:2, :])
                inva = pe_sb.tile([2, OWN], f32, tag="inva")
                nc.vector.reciprocal_approx_fast(inva[:], rmsa[:])
                invab = pe_sb.tile([2, OWN], bf16, tag="invab")
                nc.vector.tensor_copy(invab[:], inva[:])
                repa = pe_ss.tile([128, OWN], f32, tag="repa")
                nc.tensor.matmul(repa[:], o10b[:], invab[:], start=True, stop=True)
                for o in range(8):
                    t1 = pe_sb.tile([128, OWN], f32, tag="t1e")
                    nc.vector.tensor_tensor(t1[:], ao[:, o, :], repa[:], ALU.mult)
                    nc.vector.scalar_tensor_tensor(
                        xpb[:, o, :], t1[:], gat[:, o, None], xrs[:, o, :],
                        ALU.mult, ALU.add)
                    nc.any.tensor_copy(xpr[:, o, :], xpb[:, o, :])
                    nc.vector.tensor_scalar_add(xpb[:, o, :], xpb[:, o, :],
                                                bml[:, o, None])

        # ------------- Phase F: MLP ---------------------------------------
        if ph_on("f"):
            mout = big.tile([128, 8, OWN], f32, tag="xrs_mout")
            with tc.tile_pool(name="pf_h2", bufs=1) as pf_h2, \
                 tc.tile_pool(name="pf_sb", bufs=2) as pf_sb, \
                 tc.tile_pool(name="pf_wf", bufs=3) as pf_wf, \
                 tc.tile_pool(name="pf_wp", bufs=3) as pf_wp, \
                 tc.tile_pool(name="pf_ps", bufs=2, space="PSUM") as pf_ps, \
                 tc.tile_pool(name="pf_mo", bufs=1, space="PSUM") as pf_mo:
                h2 = pf_h2.tile([128, 32, OWN], bf16, tag="h2")
                for hc in range(32):
                    wfs = pf_wf.tile([128, 8, 128], bf16, tag="wfs")
                    nc.sync.dma_start(wfs[:], wfc[hc])
                    hps = pf_ps.tile([128, OWN], f32, tag="hps")
                    for k in range(8):
                        nc.tensor.matmul(hps[:], wfs[:, k, :], xpr[:, k, :],
                                         start=(k == 0), stop=(k == 7))
                    hr = pf_sb.tile([128, OWN], bf16, tag="hr")
                    nc.scalar.activation(hr[:], hps[:], AF.Relu)
                    nc.vector.tensor_tensor(h2[:, hc, :], hr[:], hr[:], ALU.mult)
                ssm = pf_ps.tile([2, OWN], f32, tag="ssm")
                for ohalf in range(2):
                    mo_ps = [pf_mo.tile([128, OWN], f32, name=f"mo{oi}", tag=f"mo{oi}")
                             for oi in range(4)]
                    for hc in range(32):
                        wps = pf_wp.tile([128, 4, 128], bf16, tag="wps")
                        nc.sync.dma_start(wps[:], wprojq[hc, ohalf])
                        for oi in range(4):
                            nc.tensor.matmul(mo_ps[oi][:], wps[:, oi, :], h2[:, hc, :],
                                             start=(hc == 0), stop=(hc == 31))
                    for oi in range(4):
                        o = ohalf * 4 + oi
                        nc.any.tensor_copy(mout[:, o, :], mo_ps[oi][:])
                        m2 = pf_sb.tile([128, OWN], bf16, tag="m2")
                        nc.scalar.activation(m2[:], mo_ps[oi][:], AF.Square)
                        nc.tensor.matmul(ssm[:], ocb[:, 0:2], m2[:],
                                         start=(o == 0), stop=(o == 7))
                rmsm = pf_sb.tile([2, OWN], f32, tag="rmsm")
                nc.scalar.activation(rmsm[:], ssm[0:2, :], AF.Sqrt, scale=1.0 / D,
                                     bias=eps6[0:2, :])
                invm = pf_sb.tile([2, OWN], f32, tag="invm")
                nc.vector.reciprocal_approx_fast(invm[:], rmsm[:])
                invmb = pf_sb.tile([2, OWN], bf16, tag="invmb")
                nc.vector.tensor_copy(invmb[:], invm[:])
                repm = pf_ps.tile([128, OWN], f32, tag="hps")
                nc.tensor.matmul(repm[:], o10b[:], invmb[:], start=True, stop=True)
                for o in range(8):
                    t1 = pf_sb.tile([128, OWN], f32, tag="t1f")
                    nc.vector.tensor_tensor(t1[:], mout[:, o, :], repm[:], ALU.mult)
                    outv = pf_sb.tile([128, OWN], f32, tag="outv")
                    nc.vector.scalar_tensor_tensor(
                        outv[:], t1[:], gml[:, o, None], xpb[:, o, :],
                        ALU.mult, ALU.add)
                    nc.sync.dma_start(out_t[:, o, :], outv[:])

        if not ph_on("f"):
            with tc.tile_pool(name="dummy", bufs=1) as dp:
                zout = dp.tile([128, 8, OWN], f32, tag="zout")
                nc.vector.memset(zout[:], 0.0)
                nc.sync.dma_start(out_t[:], zout[:])
            rope_stack.close()

    nc.finalize()
    return nc


def _feat_major(a):
    """[F, T] -> device layout [128, F//128, T]."""
    F, T = a.shape
    return np.ascontiguousarray(a.reshape(F // 128, 128, T).transpose(1, 0, 2))


def _vec_dev(v):
    return np.ascontiguousarray(v.reshape(-1, 128).T)


def _bf(a):
    return np.ascontiguousarray(a.astype(ml_dtypes.bfloat16))


_CACHE = {}
_RUN_KW = {}


def kernel(x, attn_norm_w, mlp_norm_w, attn_post_norm_w, mlp_post_norm_w,
           attn_scale, mlp_scale, attn_mod_gain, attn_mod_bias,
           mlp_mod_gain, mlp_mod_bias, Wq, Wk, Wv, Wo, q_gain, fc_w, proj_w):
    x = np.asarray(x, np.float32)
    q_gain = np.asarray(q_gain, np.float32)

    if "nc" not in _CACHE:
        _CACHE["nc"] = build(q_gain)
    nc = _CACHE["nc"]

    anw = np.asarray(attn_norm_w, np.float32)
    mnw = np.asarray(mlp_norm_w, np.float32)
    wq_eff = np.asarray(Wq, np.float32) * anw[None, :]
    wk_eff = np.asarray(Wk, np.float32) * anw[None, :]
    wv_eff = np.asarray(Wv, np.float32) * anw[None, :]
    fc_eff = np.asarray(fc_w, np.float32) * mnw[None, :]

    perm = np.zeros(D, np.int64)
    for p, (a, b) in enumerate(PAIRS):
        perm[p * 128:p * 128 + 64] = np.arange(a * 64, a * 64 + 64)
        perm[p * 128 + 64:(p + 1) * 128] = np.arange(b * 64, b * 64 + 64)
    WqTp = wq_eff.T[:, perm]                                  # [D_in, D_out-perm]
    wq_dev = _bf(np.stack([_feat_major(WqTp[:, p * 128:(p + 1) * 128]) for p in range(8)]))
    wk_dev = _bf(_feat_major(wk_eff.T))
    wv_dev = _bf(_feat_major(wv_eff.T))
    WoT = np.asarray(Wo, np.float32).T
    wo_dev = _bf(np.stack([_feat_major(WoT[:, o * 128:(o + 1) * 128]) for o in range(8)]))
    fcT = fc_eff.T
    wfc_dev = _bf(np.stack([_feat_major(fcT[:, h * 128:(h + 1) * 128]) for h in range(32)]))
    projT = np.asarray(proj_w, np.float32).T                  # [4096, 1024]
    wproj_dev = _bf(np.ascontiguousarray(
        projT.reshape(32, 128, 2, 4, 128).transpose(0, 2, 1, 3, 4)))

    inv_freq = 1.0 / (ROPE_BASE ** (np.arange(0, HD, 2, dtype=np.float32) / HD))
    tpos = np.arange(S, dtype=np.float32)
    freqs = np.outer(tpos, inv_freq).astype(np.float32)
    cosT = np.ascontiguousarray(np.tile(np.cos(freqs).T, (4, 1)))   # [128, S]
    sin1 = np.sin(freqs).T                                          # [32, S]
    sinS = np.ascontiguousarray(
        np.concatenate([sin1, -sin1, sin1, -sin1], axis=0))         # [128, S] sign-folded

    # swap permutation i <-> i^32 (within each 64-wide head)
    permM_h = np.zeros((128, 128), np.float32)
    for i in range(128):
        permM_h[i, i ^ 32] = 1.0

    oc_h_v = np.zeros((128, 2), np.float32)
    oc_h_v[0:64, 0] = 1.0
    oc_h_v[64:128, 1] = 1.0
    selg2_v = np.zeros((2, 8, 128), np.float32)
    for p, (a, b) in enumerate(PAIRS):
        selg2_v[0, p, 0:64] = q_gain[a] / 8.0
        selg2_v[1, p, 64:128] = q_gain[b] / 8.0
    ones10_v = np.concatenate([np.ones((1, 128), np.float32),
                               np.zeros((1, 128), np.float32)])

    gat_v = (np.asarray(attn_post_norm_w, np.float32)
             * np.asarray(attn_mod_gain, np.float32)
             * np.asarray(attn_scale, np.float32))
    bat_v = np.asarray(attn_mod_bias, np.float32) * np.asarray(attn_scale, np.float32)
    gml_v = (np.asarray(mlp_post_norm_w, np.float32)
             * np.asarray(mlp_mod_gain, np.float32)
             * np.asarray(mlp_scale, np.float32))
    bml_v = np.asarray(mlp_mod_bias, np.float32) * np.asarray(mlp_scale, np.float32)

    _bf_selk = _bf(np.stack([np.concatenate([np.ones(64, np.float32), np.zeros(64, np.float32)]),
                          np.concatenate([np.zeros(64, np.float32), np.ones(64, np.float32)])]))
    shared = {
        "wq": wq_dev, "wk": wk_dev, "wv": wv_dev, "wo": wo_dev,
        "wfc": wfc_dev, "wprojq": wproj_dev,
        "cosF": _bf(cosT), "sinF": _bf(sinS),
        "permM": _bf(permM_h),
        "oc_h": _bf(oc_h_v),
        "onescb": _bf(np.ones((128, 2), np.float32)),
        "selg2": _bf(selg2_v),
        "identM": _bf(np.eye(128, dtype=np.float32)),
        "selk": _bf_selk,
        "o10": ones10_v,
        "o10bf": _bf(ones10_v),
        "g_attn": _vec_dev(gat_v), "g_mlp": _vec_dev(gml_v),
        "b_mlp": _vec_dev(bml_v),
    }

    in_maps = []
    owners = []
    for c in range(8):
        b, j = c // 4, c % 4
        rows = np.concatenate(
            [np.arange((j + 4 * t) * 128, (j + 4 * t + 1) * 128) for t in range(4)])
        owners.append((b, rows))
        xb = x[b].T
        x_own = xb[:, rows]
        mask = np.zeros((4, 4, 128, 128), np.float32)
        for t in range(4):
            m = j + 4 * t
            q_idx = m * 128 + np.arange(128)
            for ktl in range(4):
                kv_idx = 512 * t + 128 * ktl + np.arange(128)
                mask[t, ktl] = (kv_idx[:, None] <= q_idx[None, :])
        m_in = {
            "xT": _bf(_feat_major(xb)),
            "xq": _bf(_feat_major(x_own)),
            "xres": _feat_major(x_own + bat_v[:, None]),
            "cosO": _bf(cosT[:, rows]),
            "sinO": _bf(sinS[:, rows]),
            "maskM": _bf(np.ascontiguousarray(mask.transpose(2, 0, 1, 3))),
        }
        m_in.update(shared)
        in_maps.append(m_in)

    res = run_bass_kernel_spmd(nc, in_maps, core_ids=list(range(8)),
                               **_RUN_KW)
    _CACHE["last_result"] = res

    out = np.empty((B, S, D), np.float32)
    for c in range(8):
        b, rows = owners[c]
        o = res.results[c]["out"]
        out[b, rows, :] = o.transpose(2, 1, 0).reshape(OWN, D)
    return out
